# revision 24
# baseline (speedup 1.0000x reference)
# Self-contained 8-core Trainium2 Bass kernel for the 2-layer GAT + mean-pool
# problem (nn_GAT_83820581749190).
#
# Sharding: destination nodes (and all their incident edges) are partitioned
# across the 8 cores, so each layer's attention softmax and aggregation
# complete locally per core. Each core builds a replicated layer-1 feature
# table [h1 | al_src | al_dst] (bf16, 512-byte rows) in HBM with a replicated
# x @ W1ext matmul, edge-gathers rows with the GPSIMD dma_gather custom op
# (int16 indices force a lo/hi table-half split), computes the edge softmax
# without segment-max (logits are small enough that exp cannot overflow), and
# aggregates per-destination with wide identity-matmul PSUM accumulation
# (destinations sit on partitions via degree-bucketed groups of 128; up to 4
# degree-lanes per matmul, reduced on Vector). Each destination's self-loop
# edge is forced to be the first lo-half edge, so the gathered block 0 carries
# the per-destination a_dst logits for free. Layer-1 group outputs are pushed
# through W2ext inline and written to a layer-2 feature table in L1-slot
# order; two split AllGathers (overlapped with the L1 tail) replicate it.
# Layer 2 repeats the gather/softmax/aggregate pattern (16 lanes per matmul);
# mean-pool is a matmul against a host-built one-hot graph matrix plus a tiny
# AllReduce.
import os
import numpy as np
import ml_dtypes

N = 50000
E = 800000
IN = 128
HID = 32
HEADS = 4
OUT = 10
GPOOL = 64
NEG = 0.2
NCORES = 8
S = N // NCORES
LO_MAX = 32767          # max usable int16 gather index
SPECIAL_ALS = -100.0    # al_src of pad rows: exp(0.2*(-100+ald)) ~ 2e-9
SB_BLOCK_BUDGET = 48    # max gather blocks per superblock
XCHUNK = 512
WPACK1 = 4              # layer-1 psum lanes per wide matmul
WPACK2 = 16             # layer-2 psum lanes per wide matmul
PHASES = int(os.environ.get("K_PHASES", "99"))  # 1=X only, 2=+L1, 3=+exchange, 4=+L2
L1STEP = int(os.environ.get("K_L1STEP", "99"))  # 1=gathers 2=+softmax 3=+exh 4=+agg 5=+epi
K_ELEM1 = int(os.environ.get("K_ELEM1", "256"))  # L1 gather elem (floor experiments)
K_SP = int(os.environ.get("K_SP", "0"))          # single_packet on gathers
K_IND = int(os.environ.get("K_IND", "0"))        # use indirect_dma_start (floor test)

bf16 = ml_dtypes.bfloat16


def _ceil_to(v, m):
    return (v + m - 1) // m * m


# ======================= host prep =========================================

def _boundary_aware_order(deg_lo, deg_hi):
    """Sort ids by (lo desc, hi desc), but fill 128-groups that straddle a
    lo-run boundary from the *small-hi tail* of the next run, keeping
    per-group max_lo + max_hi tight."""
    Sn = len(deg_lo)
    base = np.lexsort((-deg_hi, -deg_lo))
    glo = deg_lo[base]
    runs = []
    i = 0
    while i < Sn:
        j = i
        while j < Sn and glo[j] == glo[i]:
            j += 1
        runs.append(list(base[i:j]))
        i = j
    order = []
    ri = 0
    fronts = [0] * len(runs)
    backs = [len(r) for r in runs]
    while len(order) < Sn:
        while ri < len(runs) and fronts[ri] >= backs[ri]:
            ri += 1
        if ri >= len(runs):
            break
        need = 128 - (len(order) % 128)
        avail = backs[ri] - fronts[ri]
        if avail >= need:
            order.extend(runs[ri][fronts[ri]:fronts[ri] + need])
            fronts[ri] += need
        else:
            order.extend(runs[ri][fronts[ri]:backs[ri]])
            fronts[ri] = backs[ri]
            need -= avail
            rj = ri + 1
            while need > 0 and rj < len(runs):
                a = backs[rj] - fronts[rj]
                t = min(a, need)
                order.extend(reversed(runs[rj][backs[rj] - t:backs[rj]]))
                backs[rj] -= t
                need -= t
                rj += 1
    P = np.asarray(order, np.int64)
    Ppos = np.empty(Sn, np.int64)
    Ppos[P] = np.arange(Sn)
    return P, Ppos


def _run_groups(glo, ghi, max_rows=128):
    Sn = len(glo)
    ng = (Sn + max_rows - 1) // max_rows
    dlo = np.zeros(ng, np.int64)
    dhi = np.zeros(ng, np.int64)
    for g in range(ng):
        s, e = g * max_rows, min((g + 1) * max_rows, Sn)
        dlo[g] = glo[s:e].max()
        dhi[g] = ghi[s:e].max()
    return dlo, dhi


def _build_layer(src, dstl, is_lo):
    deg_lo = np.bincount(dstl[is_lo], minlength=S)
    deg_hi = np.bincount(dstl[~is_lo], minlength=S)
    P, Ppos = _boundary_aware_order(deg_lo, deg_hi)
    dlo, dhi = _run_groups(deg_lo[P], deg_hi[P])
    return dict(src=src, dstl=dstl, is_lo=is_lo, deg_lo=deg_lo, deg_hi=deg_hi,
                P=P, Ppos=Ppos, dlo=dlo, dhi=dhi)


def _emit_slots(l, DLO, DHI, idx_lo_of, idx_hi_of, special_lo, special_hi):
    NG = len(DLO)
    src, is_lo = l["src"], l["is_lo"]
    Ppos = l["Ppos"]
    nreal = len(l["P"])
    slot2cmp = np.full(NG * 128, -1, np.int64)
    slot2cmp[:nreal] = np.arange(nreal)
    idx_lo = [np.full((int(DLO[g]), 128), special_lo, np.int64)
              for g in range(NG)]
    idx_hi = [np.full((int(DHI[g]), 128), special_hi, np.int64)
              for g in range(NG)]
    slot_of_edge = Ppos[l["dstl"]]
    order = np.argsort(slot_of_edge, kind="stable")
    for mask, arrs, idx_fn in ((is_lo, idx_lo, idx_lo_of),
                               (~is_lo, idx_hi, idx_hi_of)):
        m = mask[order]
        so = slot_of_edge[order][m]
        sr = src[order][m]
        jj = np.arange(len(so)) - np.searchsorted(so, so, side="left")
        gg, kk = so // 128, so % 128
        vals = idx_fn(sr)
        for g in range(NG):
            sel = gg == g
            if sel.any():
                arrs[g][jj[sel], kk[sel]] = vals[sel]
    return idx_lo, idx_hi, slot2cmp


def _wrap16(idx):
    """[n] -> [128, n//16] int16: idx i at [i%16, i//16], replicated x8."""
    n = len(idx)
    assert n % 16 == 0
    w = np.ascontiguousarray(np.asarray(idx).reshape(n // 16, 16).T)
    w = w.astype(np.int16)
    return np.tile(w, (8, 1))


def _wrap_groups(arrs):
    segs = [_wrap16(a.reshape(-1)) if a.size else np.zeros((128, 0), np.int16)
            for a in arrs]
    return np.concatenate(segs, axis=1) if segs else np.zeros((128, 0), np.int16)


def host_prep(x, edge_index, batch, W1, a1_src, a1_dst, b1, W2, a2_src, a2_dst,
              b2, Wl, bl):
    x = np.asarray(x, np.float32)
    edge_index = np.asarray(edge_index, np.int64)
    batch = np.asarray(batch, np.int64)
    # self-loops FIRST: each dst's self edge is its first (lo) edge, so the
    # gathered lo block 0 carries the per-dst a_dst logit columns.
    src_all = np.concatenate([np.arange(N, dtype=np.int64), edge_index[0]])
    dst_all = np.concatenate([np.arange(N, dtype=np.int64), edge_index[1]])
    owner = dst_all // S

    a1_src = np.asarray(a1_src, np.float32)
    a1_dst = np.asarray(a1_dst, np.float32)
    W1 = np.asarray(W1, np.float32)
    W2 = np.asarray(W2, np.float32)
    As1 = np.zeros((HEADS * HID, HEADS), np.float32)
    Ad1 = np.zeros((HEADS * HID, HEADS), np.float32)
    for h in range(HEADS):
        As1[h * HID:(h + 1) * HID, h] = a1_src[h]
        Ad1[h * HID:(h + 1) * HID, h] = a1_dst[h]
    W1ext = np.concatenate([W1, W1 @ As1, W1 @ Ad1], axis=1)   # [128,136]
    W2ext = np.concatenate(
        [W2, W2 @ np.asarray(a2_src, np.float32)[0][:, None],
         W2 @ np.asarray(a2_dst, np.float32)[0][:, None],
         np.zeros((HEADS * HID, 2), np.float32)], axis=1)  # [128,36]

    cores = [dict(c=c) for c in range(NCORES)]
    for cd in cores:
        c = cd["c"]
        m = owner == c
        cd["src"] = src_all[m]
        cd["dstl"] = dst_all[m] - c * S

    # ---------- layer 1 ----------
    for cd in cores:
        c = cd["c"]
        pos_of = np.empty(N, np.int64)
        own = np.arange(c * S, (c + 1) * S)
        oth = np.concatenate([np.arange(0, c * S), np.arange((c + 1) * S, N)])
        pos_of[oth] = S + np.arange(N - S)
        pos_of[own] = 0
        is_lo1 = pos_of[cd["src"]] < LO_MAX
        l1 = _build_layer(cd["src"], cd["dstl"], is_lo1)
        pos_of[own] = l1["Ppos"]
        row_of = np.where(pos_of < LO_MAX, pos_of, pos_of + 1)
        cd["l1"] = l1
        cd["row_of"] = row_of
    NG1 = max(len(cd["l1"]["dlo"]) for cd in cores)
    DLO1 = np.zeros(NG1, np.int64)
    DHI1 = np.zeros(NG1, np.int64)
    for cd in cores:
        d = cd["l1"]
        DLO1[:len(d["dlo"])] = np.maximum(DLO1[:len(d["dlo"])], d["dlo"])
        DHI1[:len(d["dhi"])] = np.maximum(DHI1[:len(d["dhi"])], d["dhi"])
    assert (DLO1 >= 1).all()   # self edges are always lo
    for cd in cores:
        r = cd["row_of"]
        cd["idx1_lo"], cd["idx1_hi"], cd["slot2cmp1"] = _emit_slots(
            cd["l1"], DLO1, DHI1,
            lambda s, r=r: r[s], lambda s, r=r: r[s] - (LO_MAX + 1),
            LO_MAX, N + 1 - (LO_MAX + 1))
        # verify self-edge-first: lo block 0 of each group holds the dst's own
        # row for every real slot
        c = cd["c"]
        P1 = cd["l1"]["P"]
        for g in range(NG1):
            e = min(128, S - g * 128)
            if e <= 0:
                break
            dsts = P1[g * 128:g * 128 + e]
            assert (cd["idx1_lo"][g][0, :e] == r[dsts + c * S]).all()

    G128 = NG1 * 128
    H1 = (NG1 // 2) * 128   # first split-AllGather covers slots < H1
    # ---------- layer 2 ----------
    # table2 layout: [0]=patch-lo | [1 : 1+G128) own slots (AllGather input) |
    # [1+G128 : +8*H1) all cores' slots < H1 | [.. : +8*(G128-H1)) all cores'
    # slots >= H1 | [last]=patch-hi. Two contiguous AllGather segments.
    T2_ROWS = 1 + G128 + NCORES * G128 + 1
    B2 = 1 + G128 + NCORES * H1      # lo/hi boundary = start of 2nd AllGather
    assert B2 <= LO_MAX + 1
    row2_main = np.empty(N, np.int64)
    for cd in cores:
        c = cd["c"]
        pp1 = cd["l1"]["Ppos"]
        r = np.where(pp1 < H1,
                     1 + G128 + c * H1 + pp1,
                     1 + G128 + NCORES * H1 + c * (G128 - H1) + (pp1 - H1))
        row2_main[c * S:(c + 1) * S] = r
    for cd in cores:
        c = cd["c"]
        src = cd["src"]
        own_m = (src // S) == c
        row2 = row2_main[src].copy()
        row2[own_m] = 1 + cd["l1"]["Ppos"][src[own_m] - c * S]
        cd["row2"] = row2
        is_lo2 = row2 < B2
        cd["l2"] = _build_layer(src, cd["dstl"], is_lo2)
    NG2 = max(len(cd["l2"]["dlo"]) for cd in cores)
    DLO2 = np.zeros(NG2, np.int64)
    DHI2 = np.zeros(NG2, np.int64)
    for cd in cores:
        d = cd["l2"]
        DLO2[:len(d["dlo"])] = np.maximum(DLO2[:len(d["dlo"])], d["dlo"])
        DHI2[:len(d["dhi"])] = np.maximum(DHI2[:len(d["dhi"])], d["dhi"])
    assert (DLO2 >= 1).all()   # self edges (own rows <= G128) are always lo
    assert T2_ROWS - 1 - B2 <= 32767
    for cd in cores:
        l2 = cd["l2"]
        row2 = cd["row2"]
        pos_in_edges = {}
        # idx fns index by src node id; build per-edge instead: _emit_slots
        # passes src ids, but row2 is per-edge. Map via first occurrence is
        # wrong if a src repeats with different rows -- it cannot: row2 is a
        # function of src id only. Build a per-node map lazily.
        row2_of_node = np.empty(N, np.int64)
        row2_of_node[cd["src"]] = row2
        cd["idx2_lo"], cd["idx2_hi"], cd["slot2cmp2"] = _emit_slots(
            l2, DLO2, DHI2,
            lambda s, r=row2_of_node: r[s],
            lambda s, r=row2_of_node: r[s] - B2,
            0, T2_ROWS - 1 - B2)
        # verify self-edge-first in lo block 0
        c = cd["c"]
        P2 = l2["P"]
        own_row = 1 + cd["l1"]["Ppos"]
        for g in range(NG2):
            e = min(128, S - g * 128)
            if e <= 0:
                break
            dsts = P2[g * 128:g * 128 + e]
            assert (cd["idx2_lo"][g][0, :e] == own_row[dsts]).all()

    # ---------- aux ----------
    cnt = np.bincount(batch, minlength=GPOOL).astype(np.float32)
    recip_cnt = (1.0 / np.maximum(cnt, 1.0)).astype(np.float32)

    for cd in cores:
        c = cd["c"]
        gids = batch[c * S:(c + 1) * S]
        Mp = np.zeros((NG2 * 128, GPOOL), np.float32)
        s2c = cd["slot2cmp2"]
        real = s2c >= 0
        Mp[np.where(real)[0], gids[cd["l2"]["P"][s2c[real]]]] = 1.0
        cd["mpool"] = Mp.astype(bf16)

        xt = np.zeros((IN, _ceil_to(N + 2, XCHUNK)), np.float32)
        xt[:, cd["row_of"]] = x.T
        cd["xT"] = xt.astype(bf16)

        segs = []
        for g in range(NG1):
            a = np.concatenate(
                [cd["idx1_lo"][g],
                 cd["idx1_hi"][g] + (LO_MAX + 1)], axis=0)  # [d, 128] abs rows
            segs.append(np.ascontiguousarray(a.T))          # [128, d]
        cd["w_idx32"] = np.concatenate(segs, axis=1).astype(np.int32)
        cd["w_idx1lo"] = _wrap_groups(cd["idx1_lo"])
        cd["w_idx1hi"] = _wrap_groups(cd["idx1_hi"])
        cd["w_idx2lo"] = _wrap_groups(cd["idx2_lo"])
        cd["w_idx2hi"] = _wrap_groups(cd["idx2_hi"])

    patch1 = np.zeros((2, 256), np.float32)
    patch1[:, 128:132] = SPECIAL_ALS
    patch2 = np.zeros((2, 64), np.float32)
    patch2[:, 32] = SPECIAL_ALS

    return dict(cores=cores,
                DLO1=[int(v) for v in DLO1], DHI1=[int(v) for v in DHI1],
                DLO2=[int(v) for v in DLO2], DHI2=[int(v) for v in DHI2],
                W1ext=W1ext.astype(bf16), W2ext=W2ext.astype(bf16),
                Wl=np.asarray(Wl, np.float32),
                b1=np.tile(np.asarray(b1, np.float32).reshape(1, -1),
                           (128, 1)),
                b2=np.tile(np.asarray(b2, np.float32).reshape(1, -1),
                           (128, 1)),
                bl=np.tile(np.asarray(bl, np.float32).reshape(1, -1),
                           (GPOOL, 1)),
                rcnt=np.tile(recip_cnt.reshape(1, -1), (HID, 1)),
                patch1=patch1.astype(bf16), patch2=patch2,
                ident=np.eye(128, dtype=bf16))


def _pack_superblocks(DLO, DHI, budget=SB_BLOCK_BUDGET):
    sbs, cur, tot = [], [], 0
    for g in range(len(DLO)):
        d = int(DLO[g] + DHI[g])
        if cur and tot + d > budget:
            sbs.append(cur)
            cur, tot = [], 0
        cur.append(g)
        tot += d
    if cur:
        sbs.append(cur)
    return sbs


def make_sched(prep):
    DLO1, DHI1 = prep["DLO1"], prep["DHI1"]
    DLO2, DHI2 = prep["DLO2"], prep["DHI2"]
    assert all(a >= 1 for a in DLO1)
    assert all(a >= 1 for a in DLO2)
    return dict(DLO1=DLO1, DHI1=DHI1, DLO2=DLO2, DHI2=DHI2,
                SB1=_pack_superblocks(DLO1, DHI1),
                SB2=_pack_superblocks(DLO2, DHI2),
                HASB1=bool(np.any(prep["b1"])), HASB2=bool(np.any(prep["b2"])),
                HASBL=bool(np.any(prep["bl"])))


# ======================= bass kernel =======================================

def _emit_dummy_out(nc, tc, t_out, dt):
    with tc.tile_pool(name='dummy', bufs=1) as dp:
        d = dp.tile([GPOOL, OUT], dt.float32)
        nc.vector.memset(d[:], 0.0)
        nc.sync.dma_start(t_out[:, :], d[:])


def _chunks_desc(segs, wpack):
    """segs: list of (tile_idx, off, count). Returns wide-matmul chunks
    (tile_idx, off, k<=wpack) sorted largest-first."""
    out = []
    for ti, off, cnt in segs:
        rem = cnt
        o = off
        while rem > 0:
            k = min(wpack, rem)
            out.append((ti, o, k))
            o += k
            rem -= k
    out.sort(key=lambda t: -t[2])
    return out


def build_bass(sc):
    import concourse.bacc as bacc
    import concourse.tile as tile
    import concourse.mybir as mybir
    from concourse.library_config import mlp

    dt = mybir.dt
    Alu = mybir.AluOpType
    Act = mybir.ActivationFunctionType
    Axis = mybir.AxisListType

    DLO1, DHI1 = sc["DLO1"], sc["DHI1"]
    DLO2, DHI2 = sc["DLO2"], sc["DHI2"]
    SB1, SB2 = sc["SB1"], sc["SB2"]
    HASB1 = sc.get("HASB1", True)
    HASB2 = sc.get("HASB2", True)
    HASBL = sc.get("HASBL", True)
    NG1, NG2 = len(DLO1), len(DLO2)
    XT_COLS = _ceil_to(N + 2, XCHUNK)
    NCHUNK = XT_COLS // XCHUNK
    G128 = NG1 * 128
    T2_ROWS = 1 + G128 + NCORES * G128 + 1

    nc = bacc.Bacc("TRN2", target_bir_lowering=False, debug=False,
                   num_devices=NCORES, num_swdge_queues=4)

    t_xT = nc.dram_tensor("xT", [IN, XT_COLS], dt.bfloat16, kind="ExternalInput")
    t_w1 = nc.dram_tensor("w1ext", [IN, 136], dt.bfloat16, kind="ExternalInput")
    t_w2 = nc.dram_tensor("w2ext", [IN, 36], dt.bfloat16, kind="ExternalInput")
    t_wl = nc.dram_tensor("wl", [HID, OUT], dt.float32, kind="ExternalInput")
    t_b1 = nc.dram_tensor("b1", [128, HEADS * HID], dt.float32,
                          kind="ExternalInput")
    t_b2 = nc.dram_tensor("b2", [128, HID], dt.float32, kind="ExternalInput")
    t_bl = nc.dram_tensor("bl", [GPOOL, OUT], dt.float32, kind="ExternalInput")
    t_rcnt = nc.dram_tensor("rcnt", [HID, GPOOL], dt.float32,
                            kind="ExternalInput")
    t_patch1 = nc.dram_tensor("patch1", [2, 256], dt.bfloat16,
                              kind="ExternalInput")
    t_patch2 = nc.dram_tensor("patch2", [2, 64], dt.float32,
                              kind="ExternalInput")
    t_ident = nc.dram_tensor("ident", [128, 128], dt.bfloat16,
                             kind="ExternalInput")
    t_mpool = nc.dram_tensor("mpool", [NG2 * 128, GPOOL], dt.bfloat16,
                             kind="ExternalInput")
    n1lo = max(8 * sum(DLO1), 8)
    n1hi = max(8 * sum(DHI1), 8)
    n2lo = max(8 * sum(DLO2), 8)
    n2hi = max(8 * sum(DHI2), 8)
    n1all = sum(DLO1) + sum(DHI1)
    if K_IND:
        t_i32 = nc.dram_tensor("idx32", [128, n1all], dt.int32,
                               kind="ExternalInput")
    t_i1lo = nc.dram_tensor("idx1lo", [128, n1lo], dt.int16, kind="ExternalInput")
    t_i1hi = nc.dram_tensor("idx1hi", [128, n1hi], dt.int16, kind="ExternalInput")
    t_i2lo = nc.dram_tensor("idx2lo", [128, n2lo], dt.int16, kind="ExternalInput")
    t_i2hi = nc.dram_tensor("idx2hi", [128, n2hi], dt.int16, kind="ExternalInput")
    t_out = nc.dram_tensor("out", [GPOOL, OUT], dt.float32,
                           kind="ExternalOutput")

    rg = [list(range(NCORES))]
    _qload = [0, 0, 0, 0]

    def nextq(ndesc=1):
        q = min(range(4), key=lambda i: _qload[i])
        _qload[q] += ndesc
        return q

    with tile.TileContext(nc) as tc:
        with (
            tc.tile_pool(name="const", bufs=1) as constp,
            tc.tile_pool(name="dram", bufs=1, space="DRAM") as dramp,
        ):
            nc.gpsimd.load_library(mlp)
            def emit_gather(out_t, tab, idx_t, o0, nb, elem, estep=None,
                            idxp=None, nbmax=None):
                it = idxp.tile([128, 8 * nbmax], dt.int16, tag="idx")
                nc.sync.dma_start(it[:, :8 * nb],
                                  idx_t[:, 8 * o0:8 * (o0 + nb)])
                nc.gpsimd.dma_gather(
                    out_t[:, :nb, :], tab,
                    it[:, :8 * nb],
                    128 * nb, 128 * nb, elem, elem_step=estep,
                    single_packet=bool(K_SP), queue_num=nextq(nb))

            table1 = dramp.tile([XT_COLS, 256], dt.bfloat16, tag="table1")
            table2 = dramp.tile([_ceil_to(T2_ROWS, 4), 64], dt.float32,
                                tag="table2")
            cc_in = dramp.tile([HID, GPOOL], dt.float32, tag="ccin")
            cc_out = dramp.tile([NCORES * HID, GPOOL], dt.float32, tag="ccout")

            w1_t = constp.tile([IN, 136], dt.bfloat16)
            nc.sync.dma_start(w1_t[:], t_w1[:])
            w2_t = constp.tile([IN, 36], dt.bfloat16)
            nc.sync.dma_start(w2_t[:], t_w2[:])
            wl_t = constp.tile([HID, OUT], dt.float32)
            nc.sync.dma_start(wl_t[:], t_wl[:])
            b1_t = constp.tile([128, HEADS * HID], dt.float32)
            nc.sync.dma_start(b1_t[:], t_b1[:])
            b2_t = constp.tile([128, HID], dt.float32)
            nc.sync.dma_start(b2_t[:], t_b2[:])
            bl_t = constp.tile([GPOOL, OUT], dt.float32)
            nc.sync.dma_start(bl_t[:], t_bl[:])
            rc_t = constp.tile([HID, GPOOL], dt.float32)
            nc.sync.dma_start(rc_t[:], t_rcnt[:])
            id_t = constp.tile([128, 128], dt.bfloat16)
            nc.sync.dma_start(id_t[:], t_ident[:])
            # all gather indices live in SBUF for the whole run

            # ---------------- phase X: build table1 ----------------
            with (
                tc.tile_pool(name="xload", bufs=3) as xlp,
                tc.tile_pool(name="xout", bufs=3) as xop,
                tc.tile_pool(name="xpsum", bufs=4, space="PSUM") as xpp,
            ):
                for t in range(NCHUNK):
                    xt_t = xlp.tile([IN, XCHUNK], dt.bfloat16, tag="xt")
                    nc.sync.dma_start(xt_t[:],
                                      t_xT[:, t * XCHUNK:(t + 1) * XCHUNK])
                    o_t = xop.tile([128, 4, 256], dt.bfloat16, tag="xo")
                    nc.vector.memset(o_t[:, :, 136:256], 0.0)
                    for k in range(4):
                        ps = xpp.tile([128, 136], dt.float32, tag="xp")
                        nc.tensor.matmul(ps[:], xt_t[:, k * 128:(k + 1) * 128],
                                         w1_t[:], start=True, stop=True)
                        if k % 2 == 0:
                            nc.vector.tensor_copy(o_t[:, k, 0:136], ps[:])
                        else:
                            nc.scalar.activation(o_t[:, k, 0:136], ps[:],
                                                 Act.Copy)
                    nc.sync.dma_start(
                        table1[t * XCHUNK:(t + 1) * XCHUNK, :].rearrange(
                            "(k p) e -> p k e", p=128), o_t[:])
            with tc.tile_pool(name="patchp", bufs=1) as pp:
                p1_t = pp.tile([2, 256], dt.bfloat16)
                nc.sync.dma_start(p1_t[:], t_patch1[:])
                nc.sync.dma_start(table1[LO_MAX:LO_MAX + 1, :], p1_t[0:1, :])
                nc.sync.dma_start(table1[N + 1:N + 2, :], p1_t[1:2, :])
                if PHASES >= 2:
                    p2_t = pp.tile([2, 64], dt.float32)
                    nc.sync.dma_start(p2_t[:], t_patch2[:])
                    nc.sync.dma_start(table2[0:1, :], p2_t[0:1, :])
                    nc.sync.dma_start(table2[T2_ROWS - 1:T2_ROWS, :],
                                      p2_t[1:2, :])

            if PHASES >= 2:
                # ---------------- phase L1: edges ----------------
                tab1_lo = table1[0:LO_MAX + 1, :]
                tab1_hi = table1[LO_MAX + 1:N + 2, :]
                Dmax1 = max(DLO1[g] + DHI1[g] for g in range(NG1))
                NBLO1 = max(sum(DLO1[g] for g in sb) for sb in SB1)
                NBHI1 = max(max(sum(DHI1[g] for g in sb) for sb in SB1), 1)
                olo = np.concatenate([[0], np.cumsum(DLO1)]).astype(int)
                ohi = np.concatenate([[0], np.cumsum(DHI1)]).astype(int)
                H1 = (NG1 // 2) * 128   # first AllGather covers groups < NG1//2
                t2main1 = table2[1 + G128:1 + G128 + NCORES * H1, :]
                t2main2 = table2[1 + G128 + NCORES * H1:1 + G128 +
                                 NCORES * G128, :]
                with (
                    tc.tile_pool(name="idx1", bufs=4) as idxp,
                    tc.tile_pool(name="gath1", bufs=4) as gathp,
                    tc.tile_pool(name="small1", bufs=3) as smallp,
                    tc.tile_pool(name="epi1", bufs=3) as epip,
                    tc.tile_pool(name="cp1", bufs=3) as cpp,
                    tc.tile_pool(name="agg1", bufs=2, space="PSUM") as aggp,
                    tc.tile_pool(name="psT1", bufs=2, space="PSUM") as psTp,
                    tc.tile_pool(name="ps21", bufs=2, space="PSUM") as ps2p,
                ):
                    for sb in SB1:
                        g0 = sb[0]
                        nlo = sum(DLO1[g] for g in sb)
                        nhi = sum(DHI1[g] for g in sb)
                        glo_t = gathp.tile([128, NBLO1, K_ELEM1],
                                           dt.bfloat16, tag="glo")
                        estep = 256 if K_ELEM1 != 256 else None
                        emit_gather(glo_t, tab1_lo, t_i1lo, olo[g0], nlo,
                                    K_ELEM1, estep, idxp=idxp, nbmax=NBLO1)
                        ghi_t = gathp.tile([128, NBHI1, K_ELEM1], dt.bfloat16,
                                           tag="ghi")
                        if nhi > 0:
                            emit_gather(ghi_t, tab1_hi, t_i1hi, ohi[g0], nhi,
                                        K_ELEM1, estep, idxp=idxp, nbmax=NBHI1)
                        lo_off = 0
                        hi_off = 0
                        for gi, g in enumerate(sb):
                            dlo, dhi = DLO1[g], DHI1[g]
                            D = dlo + dhi
                            if L1STEP < 2:
                                lo_off += dlo
                                hi_off += dhi
                                continue
                            logit_t = smallp.tile([128, Dmax1, 4], dt.float32,
                                                  tag="lg")
                            exb_t = smallp.tile([128, Dmax1, 4], dt.bfloat16,
                                                tag="exb")
                            exs_t = smallp.tile([128, Dmax1, 4], dt.bfloat16,
                                                tag="exs")
                            den_t = smallp.tile([128, 4], dt.float32, tag="dn")
                            rec_t = smallp.tile([128, 4], dt.float32, tag="rc")
                            ald_ap = glo_t[:, lo_off, 132:136]
                            nc.vector.scalar_tensor_tensor(
                                logit_t[:, :dlo, :],
                                glo_t[:, lo_off:lo_off + dlo, 128:132], 0.0,
                                ald_ap.unsqueeze(1).broadcast_to(
                                    (128, dlo, 4)), Alu.add, Alu.add)
                            if dhi > 0:
                                nc.vector.scalar_tensor_tensor(
                                    logit_t[:, dlo:D, :],
                                    ghi_t[:, hi_off:hi_off + dhi, 128:132], 0.0,
                                    ald_ap.unsqueeze(1).broadcast_to(
                                        (128, dhi, 4)), Alu.add, Alu.add)
                            nc.vector.scalar_tensor_tensor(
                                logit_t[:, :D, :], logit_t[:, :D, :], NEG,
                                logit_t[:, :D, :], Alu.mult, Alu.max)
                            nc.scalar.activation(exb_t[:, :D, :],
                                                 logit_t[:, :D, :], Act.Exp)
                            nc.vector.tensor_reduce(
                                den_t[:], exb_t[:, :D, :].transpose([0, 2, 1]),
                                axis=Axis.X, op=Alu.add)
                            nc.vector.reciprocal(rec_t[:], den_t[:])
                            nc.vector.tensor_tensor(
                                exs_t[:, :D, :], exb_t[:, :D, :],
                                rec_t[:].unsqueeze(1).broadcast_to(
                                    (128, D, 4)), Alu.mult)
                            if L1STEP < 3:
                                lo_off += dlo
                                hi_off += dhi
                                continue
                            h_lo = glo_t[:, lo_off:lo_off + dlo, 0:128]
                            h_lo = h_lo.rearrange("p b (h c) -> p b h c", h=4)
                            nc.vector.tensor_tensor(
                                h_lo, h_lo,
                                exs_t[:, :dlo, :].unsqueeze(3).broadcast_to(
                                    (128, dlo, 4, HID)), Alu.mult)
                            if dhi > 0:
                                h_hi = ghi_t[:, hi_off:hi_off + dhi, 0:128]
                                h_hi = h_hi.rearrange("p b (h c) -> p b h c",
                                                      h=4)
                                nc.vector.tensor_tensor(
                                    h_hi, h_hi,
                                    exs_t[:, dlo:D, :].unsqueeze(3).broadcast_to(
                                        (128, dhi, 4, HID)), Alu.mult)
                            if L1STEP < 4:
                                lo_off += dlo
                                hi_off += dhi
                                continue
                            segs = [(0, lo_off, dlo)]
                            if dhi > 0:
                                segs.append((1, hi_off, dhi))
                            chunks = _chunks_desc(segs, WPACK1)
                            lanes = chunks[0][2]
                            agg = aggp.tile([128, WPACK1, 128], dt.float32,
                                            tag="agg")
                            for ci, (ti, off, k) in enumerate(chunks):
                                src_t = glo_t if ti == 0 else ghi_t
                                nc.tensor.matmul(
                                    agg[:, 0:k, :],
                                    id_t[:], src_t[:, off:off + k, 0:128],
                                    start=(ci == 0),
                                    stop=(ci == len(chunks) - 1))
                            if L1STEP < 5:
                                lo_off += dlo
                                hi_off += dhi
                                continue
                            scaled_t = epip.tile([128, 128], dt.float32,
                                                 tag="sd")
                            if lanes == 1:
                                nc.vector.tensor_copy(scaled_t[:],
                                                      agg[:, 0, :])
                            else:
                                nc.vector.tensor_reduce(
                                    scaled_t[:],
                                    agg[:, 0:lanes, :].transpose([0, 2, 1]),
                                    axis=Axis.X, op=Alu.add)
                            if HASB1:
                                nc.vector.tensor_tensor(
                                    scaled_t[:], scaled_t[:], b1_t[:], Alu.add)
                            tmp_t = epip.tile([128, 128], dt.float32, tag="tm")
                            nc.scalar.activation(tmp_t[:], scaled_t[:], Act.Relu,
                                                 scale=-1.0)
                            nc.scalar.activation(tmp_t[:], tmp_t[:], Act.Exp,
                                                 scale=-1.0)
                            elu_t = epip.tile([128, 128], dt.bfloat16, tag="el")
                            nc.vector.scalar_tensor_tensor(
                                elu_t[:], tmp_t[:], -1.0, scaled_t[:],
                                Alu.add, Alu.max)
                            # inline pass-2: h1' @ W2ext -> table2 front rows
                            psT = psTp.tile([128, 128], dt.bfloat16, tag="pt")
                            nc.tensor.transpose(psT[:], elu_t[:], id_t[:])
                            eluT_t = epip.tile([128, 128], dt.bfloat16,
                                               tag="et")
                            nc.scalar.activation(eluT_t[:], psT[:], Act.Copy)
                            ps2 = ps2p.tile([128, 36], dt.float32, tag="p2")
                            nc.tensor.matmul(ps2[:], eluT_t[:], w2_t[:],
                                             start=True, stop=True)
                            cp_t = cpp.tile([128, 64], dt.float32, tag="cp")
                            if g % 2 == 0:
                                nc.scalar.activation(cp_t[:, 0:36], ps2[:],
                                                     Act.Copy)
                            else:
                                nc.vector.tensor_copy(cp_t[:, 0:36], ps2[:])
                            nc.sync.dma_start(
                                table2[1 + 128 * g:1 + 128 * (g + 1), :],
                                cp_t[:])
                            lo_off += dlo
                            hi_off += dhi
                            if PHASES >= 3 and L1STEP >= 5:
                                if 128 * (g + 1) == H1:
                                    nc.gpsimd.collective_compute(
                                        "AllGather", Alu.bypass,
                                        replica_groups=rg,
                                        ins=[table2[1:1 + H1, :]],
                                        outs=[t2main1[:, :]])
                    if PHASES >= 3 and L1STEP >= 5:
                        nc.gpsimd.collective_compute(
                            "AllGather", Alu.bypass, replica_groups=rg,
                            ins=[table2[1 + H1:1 + G128, :]],
                            outs=[t2main2[:, :]])

            if PHASES >= 4:
                # ---------------- phase L2: edges + pool ----------------
                B2 = 1 + G128 + NCORES * ((NG1 // 2) * 128)
                tab2_lo = table2[0:B2, :]
                tab2_hi = table2[B2:T2_ROWS, :]
                Dmax2 = max(DLO2[g] + DHI2[g] for g in range(NG2))
                NBLO2 = max(sum(DLO2[g] for g in sb) for sb in SB2)
                NBHI2 = max(max(sum(DHI2[g] for g in sb) for sb in SB2), 1)
                olo2 = np.concatenate([[0], np.cumsum(DLO2)]).astype(int)
                ohi2 = np.concatenate([[0], np.cumsum(DHI2)]).astype(int)
                with (
                    tc.tile_pool(name="idx2", bufs=4) as idxp,
                    tc.tile_pool(name="gath2", bufs=6) as gathp,
                    tc.tile_pool(name="small2", bufs=4) as smallp,
                    tc.tile_pool(name="epi2", bufs=3) as epip,
                    tc.tile_pool(name="agg2", bufs=2, space="PSUM") as aggp,
                    tc.tile_pool(name="poolps", bufs=1, space="PSUM") as poolpp,
                    tc.tile_pool(name="mp2", bufs=3) as mpp,
                ):
                    poolps = poolpp.tile([HID, GPOOL], dt.float32)
                    h2p_all = mpp.tile([128, NG2, HID], dt.bfloat16,
                                       tag="h2pall", bufs=1)
                    for sb in SB2:
                        g0 = sb[0]
                        nlo = sum(DLO2[g] for g in sb)
                        nhi = sum(DHI2[g] for g in sb)
                        glo_t = gathp.tile([128, NBLO2, 64], dt.float32,
                                           tag="glo")
                        emit_gather(glo_t, tab2_lo, t_i2lo, olo2[g0], nlo, 64,
                                    idxp=idxp, nbmax=NBLO2)
                        ghi_t = gathp.tile([128, NBHI2, 64], dt.float32,
                                           tag="ghi")
                        if nhi > 0:
                            emit_gather(ghi_t, tab2_hi, t_i2hi, ohi2[g0], nhi,
                                        64, idxp=idxp, nbmax=NBHI2)
                        lo_off = 0
                        hi_off = 0
                        for gi, g in enumerate(sb):
                            dlo, dhi = DLO2[g], DHI2[g]
                            D = dlo + dhi
                            logit_t = smallp.tile([128, Dmax2, 1], dt.float32,
                                                  tag="lg")
                            exf_t = smallp.tile([128, Dmax2, 1], dt.float32,
                                                tag="exf")
                            exs_t = smallp.tile([128, Dmax2, 1], dt.float32,
                                                tag="exs")
                            den_t = smallp.tile([128, 1], dt.float32, tag="dn")
                            rec_t = smallp.tile([128, 1], dt.float32, tag="rc")
                            ald_ap = glo_t[:, lo_off, 33:34]
                            nc.vector.scalar_tensor_tensor(
                                logit_t[:, :dlo, :],
                                glo_t[:, lo_off:lo_off + dlo, 32:33], 0.0,
                                ald_ap.unsqueeze(1).broadcast_to(
                                    (128, dlo, 1)), Alu.add, Alu.add)
                            if dhi > 0:
                                nc.vector.scalar_tensor_tensor(
                                    logit_t[:, dlo:D, :],
                                    ghi_t[:, hi_off:hi_off + dhi, 32:33], 0.0,
                                    ald_ap.unsqueeze(1).broadcast_to(
                                        (128, dhi, 1)), Alu.add, Alu.add)
                            nc.vector.scalar_tensor_tensor(
                                logit_t[:, :D, :], logit_t[:, :D, :], NEG,
                                logit_t[:, :D, :], Alu.mult, Alu.max)
                            nc.scalar.activation(exf_t[:, :D, :],
                                                 logit_t[:, :D, :], Act.Exp)
                            nc.vector.tensor_reduce(
                                den_t[:], exf_t[:, :D, :].transpose([0, 2, 1]),
                                axis=Axis.X, op=Alu.add)
                            nc.vector.reciprocal(rec_t[:], den_t[:])
                            nc.vector.tensor_tensor(
                                exs_t[:, :D, :], exf_t[:, :D, :],
                                rec_t[:].unsqueeze(1).broadcast_to(
                                    (128, D, 1)), Alu.mult)
                            exh_t = smallp.tile([128, Dmax2, HID], dt.bfloat16,
                                                tag="exh")
                            nc.vector.tensor_tensor(
                                exh_t[:, :dlo, :],
                                glo_t[:, lo_off:lo_off + dlo, 0:HID],
                                exs_t[:, :dlo, :].broadcast_to(
                                    (128, dlo, HID)), Alu.mult)
                            if dhi > 0:
                                nc.vector.tensor_tensor(
                                    exh_t[:, dlo:D, :],
                                    ghi_t[:, hi_off:hi_off + dhi, 0:HID],
                                    exs_t[:, dlo:D, :].broadcast_to(
                                        (128, dhi, HID)), Alu.mult)
                            chunks = _chunks_desc([(0, 0, D)], WPACK2)
                            lanes = chunks[0][2]
                            agg = aggp.tile([128, WPACK2, HID], dt.float32,
                                            tag="agg")
                            for ci, (ti, off, k) in enumerate(chunks):
                                nc.tensor.matmul(
                                    agg[:, 0:k, :],
                                    id_t[:], exh_t[:, off:off + k, :],
                                    start=(ci == 0),
                                    stop=(ci == len(chunks) - 1))
                            scaled_t = epip.tile([128, HID], dt.float32,
                                                 tag="sd")
                            if lanes == 1:
                                nc.vector.tensor_copy(scaled_t[:],
                                                      agg[:, 0, :])
                            else:
                                nc.vector.tensor_reduce(
                                    scaled_t[:],
                                    agg[:, 0:lanes, :].transpose([0, 2, 1]),
                                    axis=Axis.X, op=Alu.add)
                            if HASB2:
                                nc.vector.tensor_tensor(
                                    scaled_t[:], scaled_t[:], b2_t[:], Alu.add)
                            tmp_t = epip.tile([128, HID], dt.float32, tag="tm")
                            nc.scalar.activation(tmp_t[:], scaled_t[:], Act.Relu,
                                                 scale=-1.0)
                            nc.scalar.activation(tmp_t[:], tmp_t[:], Act.Exp,
                                                 scale=-1.0)
                            nc.vector.scalar_tensor_tensor(
                                h2p_all[:, g, :], tmp_t[:], -1.0, scaled_t[:],
                                Alu.add, Alu.max)
                            lo_off += dlo
                            hi_off += dhi

                    for g in range(NG2):
                        mp_t = mpp.tile([128, GPOOL], dt.bfloat16, tag="mp")
                        nc.sync.dma_start(mp_t[:],
                                          t_mpool[g * 128:(g + 1) * 128, :])
                        nc.tensor.matmul(poolps[:], h2p_all[:, g, :], mp_t[:],
                                         start=(g == 0), stop=(g == NG2 - 1))
                    # ------------- pool + final linear -------------
                    with tc.tile_pool(name="fin", bufs=1) as finp, \
                            tc.tile_pool(name="finps", bufs=1, space="PSUM") as fpp:
                        poolsb = finp.tile([HID, GPOOL], dt.float32)
                        nc.vector.tensor_copy(poolsb[:], poolps[:])
                        nc.sync.dma_start(cc_in[:, :], poolsb[:])
                        nc.gpsimd.collective_compute(
                            "AllGather", Alu.bypass, replica_groups=rg,
                            ins=[cc_in[:, :]], outs=[cc_out[:, :]])
                        parts_t = finp.tile([HID, NCORES, GPOOL], dt.float32)
                        nc.sync.dma_start(
                            parts_t[:],
                            cc_out[:, :].rearrange("(c p) e -> p c e",
                                                   p=HID))
                        psum_t = finp.tile([HID, GPOOL], dt.float32)
                        nc.vector.tensor_reduce(
                            psum_t[:], parts_t[:].transpose([0, 2, 1]),
                            axis=Axis.X, op=Alu.add)
                        mean_t = finp.tile([HID, GPOOL], dt.float32)
                        nc.vector.tensor_tensor(
                            mean_t[:], psum_t[:],
                            rc_t[:], Alu.mult)
                        psO = fpp.tile([GPOOL, OUT], dt.float32)
                        nc.tensor.matmul(psO[:], mean_t[:], wl_t[:], start=True,
                                         stop=True)
                        out_t = finp.tile([GPOOL, OUT], dt.float32)
                        if HASBL:
                            nc.vector.tensor_tensor(out_t[:], psO[:], bl_t[:],
                                                    Alu.add)
                        else:
                            nc.vector.tensor_copy(out_t[:], psO[:])
                        nc.sync.dma_start(t_out[:, :], out_t[:])
            if PHASES < 4:
                _emit_dummy_out(nc, tc, t_out, dt)


    nc.compile()
    return nc


def core_inputs(prep, c):
    cd = prep["cores"][c]

    def padcols(a, cols):
        if a.shape[1] == cols:
            return a
        out = np.zeros((a.shape[0], cols), a.dtype)
        out[:, :a.shape[1]] = a
        return out

    n1lo = max(8 * sum(prep["DLO1"]), 8)
    n1hi = max(8 * sum(prep["DHI1"]), 8)
    n2lo = max(8 * sum(prep["DLO2"]), 8)
    n2hi = max(8 * sum(prep["DHI2"]), 8)
    return dict(
        xT=np.ascontiguousarray(cd["xT"]),
        w1ext=prep["W1ext"], w2ext=prep["W2ext"], wl=prep["Wl"],
        b1=prep["b1"], b2=prep["b2"], bl=prep["bl"], rcnt=prep["rcnt"],
        patch1=prep["patch1"], patch2=prep["patch2"], ident=prep["ident"],
        mpool=np.ascontiguousarray(cd["mpool"]),
        **({"idx32": cd["w_idx32"]} if K_IND else {}),
        idx1lo=padcols(cd["w_idx1lo"], n1lo),
        idx1hi=padcols(cd["w_idx1hi"], n1hi),
        idx2lo=padcols(cd["w_idx2lo"], n2lo),
        idx2hi=padcols(cd["w_idx2hi"], n2hi),
    )


_CACHE = {}


def kernel(**inputs):
    from concourse.bass_utils import run_bass_kernel_spmd

    inputs = {k: np.asarray(v) for k, v in inputs.items()}
    prep = host_prep(**inputs)
    sc = make_sched(prep)
    key = str(sc)
    if key not in _CACHE:
        _CACHE[key] = build_bass(sc)
    nc = _CACHE[key]
    in_maps = [core_inputs(prep, c) for c in range(NCORES)]
    res = run_bass_kernel_spmd(nc, in_maps, list(range(NCORES)))
    return np.asarray(res.results[0]["out"], np.float32)


# revision 25
# speedup vs baseline: 1.1116x; 1.1116x over previous
# Self-contained 8-core Trainium2 Bass kernel for the 2-layer GAT + mean-pool
# problem (nn_GAT_83820581749190).
#
# Sharding: destination nodes (and all their incident edges) are partitioned
# across the 8 cores, so each layer's attention softmax and aggregation
# complete locally per core. Each core builds a replicated layer-1 feature
# table [h1 | al_src | al_dst] (bf16, 512-byte rows) in HBM with a replicated
# x @ W1ext matmul, edge-gathers rows with the GPSIMD dma_gather custom op
# (int16 indices force a lo/hi table-half split), computes the edge softmax
# without segment-max (logits are small enough that exp cannot overflow), and
# aggregates per-destination with wide identity-matmul PSUM accumulation
# (destinations sit on partitions via degree-bucketed groups of 128; up to 4
# degree-lanes per matmul, reduced on Vector). Each destination's self-loop
# edge is forced to be the first lo-half edge, so the gathered block 0 carries
# the per-destination a_dst logits for free. Layer-1 group outputs are pushed
# through W2ext inline and written to a layer-2 feature table in L1-slot
# order; two split AllGathers (overlapped with the L1 tail) replicate it.
# Layer 2 repeats the gather/softmax/aggregate pattern (16 lanes per matmul);
# mean-pool is a matmul against a host-built one-hot graph matrix plus a tiny
# AllReduce.
import os
import numpy as np
import ml_dtypes

N = 50000
E = 800000
IN = 128
HID = 32
HEADS = 4
OUT = 10
GPOOL = 64
NEG = 0.2
NCORES = 8
S = N // NCORES
LO_MAX = 32767          # max usable int16 gather index
SPECIAL_ALS = -100.0    # al_src of pad rows: exp(0.2*(-100+ald)) ~ 2e-9
SB_BLOCK_BUDGET = 48    # max gather blocks per superblock
XCHUNK = 512
WPACK1 = 4              # layer-1 psum lanes per wide matmul
WPACK2 = 16             # layer-2 psum lanes per wide matmul
PHASES = int(os.environ.get("K_PHASES", "99"))  # 1=X only, 2=+L1, 3=+exchange, 4=+L2
L1STEP = int(os.environ.get("K_L1STEP", "99"))  # 1=gathers 2=+softmax 3=+exh 4=+agg 5=+epi
K_ELEM1 = int(os.environ.get("K_ELEM1", "256"))  # L1 gather elem (floor experiments)
K_SP = int(os.environ.get("K_SP", "0"))          # single_packet on gathers
K_IND = int(os.environ.get("K_IND", "0"))        # use indirect_dma_start (floor test)

bf16 = ml_dtypes.bfloat16


def _ceil_to(v, m):
    return (v + m - 1) // m * m


# ======================= host prep =========================================

def _boundary_aware_order(deg_lo, deg_hi):
    """Sort ids by (lo desc, hi desc), but fill 128-groups that straddle a
    lo-run boundary from the *small-hi tail* of the next run, keeping
    per-group max_lo + max_hi tight."""
    Sn = len(deg_lo)
    base = np.lexsort((-deg_hi, -deg_lo))
    glo = deg_lo[base]
    runs = []
    i = 0
    while i < Sn:
        j = i
        while j < Sn and glo[j] == glo[i]:
            j += 1
        runs.append(list(base[i:j]))
        i = j
    order = []
    ri = 0
    fronts = [0] * len(runs)
    backs = [len(r) for r in runs]
    while len(order) < Sn:
        while ri < len(runs) and fronts[ri] >= backs[ri]:
            ri += 1
        if ri >= len(runs):
            break
        need = 128 - (len(order) % 128)
        avail = backs[ri] - fronts[ri]
        if avail >= need:
            order.extend(runs[ri][fronts[ri]:fronts[ri] + need])
            fronts[ri] += need
        else:
            order.extend(runs[ri][fronts[ri]:backs[ri]])
            fronts[ri] = backs[ri]
            need -= avail
            rj = ri + 1
            while need > 0 and rj < len(runs):
                a = backs[rj] - fronts[rj]
                t = min(a, need)
                order.extend(reversed(runs[rj][backs[rj] - t:backs[rj]]))
                backs[rj] -= t
                need -= t
                rj += 1
    P = np.asarray(order, np.int64)
    Ppos = np.empty(Sn, np.int64)
    Ppos[P] = np.arange(Sn)
    return P, Ppos


def _run_groups(glo, ghi, max_rows=128):
    Sn = len(glo)
    ng = (Sn + max_rows - 1) // max_rows
    dlo = np.zeros(ng, np.int64)
    dhi = np.zeros(ng, np.int64)
    for g in range(ng):
        s, e = g * max_rows, min((g + 1) * max_rows, Sn)
        dlo[g] = glo[s:e].max()
        dhi[g] = ghi[s:e].max()
    return dlo, dhi


def _build_layer(src, dstl, is_lo):
    deg_lo = np.bincount(dstl[is_lo], minlength=S)
    deg_hi = np.bincount(dstl[~is_lo], minlength=S)
    P, Ppos = _boundary_aware_order(deg_lo, deg_hi)
    dlo, dhi = _run_groups(deg_lo[P], deg_hi[P])
    return dict(src=src, dstl=dstl, is_lo=is_lo, deg_lo=deg_lo, deg_hi=deg_hi,
                P=P, Ppos=Ppos, dlo=dlo, dhi=dhi)


def _emit_slots(l, DLO, DHI, idx_lo_of, idx_hi_of, special_lo, special_hi):
    NG = len(DLO)
    src, is_lo = l["src"], l["is_lo"]
    Ppos = l["Ppos"]
    nreal = len(l["P"])
    slot2cmp = np.full(NG * 128, -1, np.int64)
    slot2cmp[:nreal] = np.arange(nreal)
    idx_lo = [np.full((int(DLO[g]), 128), special_lo, np.int64)
              for g in range(NG)]
    idx_hi = [np.full((int(DHI[g]), 128), special_hi, np.int64)
              for g in range(NG)]
    slot_of_edge = Ppos[l["dstl"]]
    order = np.argsort(slot_of_edge, kind="stable")
    for mask, arrs, idx_fn in ((is_lo, idx_lo, idx_lo_of),
                               (~is_lo, idx_hi, idx_hi_of)):
        m = mask[order]
        so = slot_of_edge[order][m]
        sr = src[order][m]
        jj = np.arange(len(so)) - np.searchsorted(so, so, side="left")
        gg, kk = so // 128, so % 128
        vals = idx_fn(sr)
        for g in range(NG):
            sel = gg == g
            if sel.any():
                arrs[g][jj[sel], kk[sel]] = vals[sel]
    return idx_lo, idx_hi, slot2cmp


def _wrap16(idx):
    """[n] -> [128, n//16] int16: idx i at [i%16, i//16], replicated x8."""
    n = len(idx)
    assert n % 16 == 0
    w = np.ascontiguousarray(np.asarray(idx).reshape(n // 16, 16).T)
    w = w.astype(np.int16)
    return np.tile(w, (8, 1))


def _wrap_groups(arrs):
    segs = [_wrap16(a.reshape(-1)) if a.size else np.zeros((128, 0), np.int16)
            for a in arrs]
    return np.concatenate(segs, axis=1) if segs else np.zeros((128, 0), np.int16)


def host_prep(x, edge_index, batch, W1, a1_src, a1_dst, b1, W2, a2_src, a2_dst,
              b2, Wl, bl):
    x = np.asarray(x, np.float32)
    edge_index = np.asarray(edge_index, np.int64)
    batch = np.asarray(batch, np.int64)
    # self-loops FIRST: each dst's self edge is its first (lo) edge, so the
    # gathered lo block 0 carries the per-dst a_dst logit columns.
    src_all = np.concatenate([np.arange(N, dtype=np.int64), edge_index[0]])
    dst_all = np.concatenate([np.arange(N, dtype=np.int64), edge_index[1]])
    owner = dst_all // S

    a1_src = np.asarray(a1_src, np.float32)
    a1_dst = np.asarray(a1_dst, np.float32)
    W1 = np.asarray(W1, np.float32)
    W2 = np.asarray(W2, np.float32)
    As1 = np.zeros((HEADS * HID, HEADS), np.float32)
    Ad1 = np.zeros((HEADS * HID, HEADS), np.float32)
    for h in range(HEADS):
        As1[h * HID:(h + 1) * HID, h] = a1_src[h]
        Ad1[h * HID:(h + 1) * HID, h] = a1_dst[h]
    W1ext = np.concatenate([W1, W1 @ As1, W1 @ Ad1], axis=1)   # [128,136]
    W2ext = np.concatenate(
        [W2, W2 @ np.asarray(a2_src, np.float32)[0][:, None],
         W2 @ np.asarray(a2_dst, np.float32)[0][:, None],
         np.zeros((HEADS * HID, 2), np.float32)], axis=1)  # [128,36]

    cores = [dict(c=c) for c in range(NCORES)]
    for cd in cores:
        c = cd["c"]
        m = owner == c
        cd["src"] = src_all[m]
        cd["dstl"] = dst_all[m] - c * S

    # ---------- layer 1 ----------
    for cd in cores:
        c = cd["c"]
        pos_of = np.empty(N, np.int64)
        own = np.arange(c * S, (c + 1) * S)
        oth = np.concatenate([np.arange(0, c * S), np.arange((c + 1) * S, N)])
        pos_of[oth] = S + np.arange(N - S)
        pos_of[own] = 0
        is_lo1 = pos_of[cd["src"]] < LO_MAX
        l1 = _build_layer(cd["src"], cd["dstl"], is_lo1)
        pos_of[own] = l1["Ppos"]
        row_of = np.where(pos_of < LO_MAX, pos_of, pos_of + 1)
        cd["l1"] = l1
        cd["row_of"] = row_of
    NG1 = max(len(cd["l1"]["dlo"]) for cd in cores)
    DLO1 = np.zeros(NG1, np.int64)
    DHI1 = np.zeros(NG1, np.int64)
    for cd in cores:
        d = cd["l1"]
        DLO1[:len(d["dlo"])] = np.maximum(DLO1[:len(d["dlo"])], d["dlo"])
        DHI1[:len(d["dhi"])] = np.maximum(DHI1[:len(d["dhi"])], d["dhi"])
    assert (DLO1 >= 1).all()   # self edges are always lo
    for cd in cores:
        r = cd["row_of"]
        cd["idx1_lo"], cd["idx1_hi"], cd["slot2cmp1"] = _emit_slots(
            cd["l1"], DLO1, DHI1,
            lambda s, r=r: r[s], lambda s, r=r: r[s] - (LO_MAX + 1),
            LO_MAX, N + 1 - (LO_MAX + 1))
        # verify self-edge-first: lo block 0 of each group holds the dst's own
        # row for every real slot
        c = cd["c"]
        P1 = cd["l1"]["P"]
        for g in range(NG1):
            e = min(128, S - g * 128)
            if e <= 0:
                break
            dsts = P1[g * 128:g * 128 + e]
            assert (cd["idx1_lo"][g][0, :e] == r[dsts + c * S]).all()

    G128 = NG1 * 128
    H1 = (NG1 // 2) * 128   # first split-AllGather covers slots < H1
    # ---------- layer 2 ----------
    # table2 layout: [0]=patch-lo | [1 : 1+G128) own slots (AllGather input) |
    # [1+G128 : +8*H1) all cores' slots < H1 | [.. : +8*(G128-H1)) all cores'
    # slots >= H1 | [last]=patch-hi. Two contiguous AllGather segments.
    T2_ROWS = 1 + G128 + NCORES * G128 + 1
    B2 = 1 + G128 + NCORES * H1      # lo/hi boundary = start of 2nd AllGather
    assert B2 <= LO_MAX + 1
    row2_main = np.empty(N, np.int64)
    for cd in cores:
        c = cd["c"]
        pp1 = cd["l1"]["Ppos"]
        r = np.where(pp1 < H1,
                     1 + G128 + c * H1 + pp1,
                     1 + G128 + NCORES * H1 + c * (G128 - H1) + (pp1 - H1))
        row2_main[c * S:(c + 1) * S] = r
    for cd in cores:
        c = cd["c"]
        src = cd["src"]
        own_m = (src // S) == c
        row2 = row2_main[src].copy()
        row2[own_m] = 1 + cd["l1"]["Ppos"][src[own_m] - c * S]
        cd["row2"] = row2
        is_lo2 = row2 < B2
        cd["l2"] = _build_layer(src, cd["dstl"], is_lo2)
    NG2 = max(len(cd["l2"]["dlo"]) for cd in cores)
    DLO2 = np.zeros(NG2, np.int64)
    DHI2 = np.zeros(NG2, np.int64)
    for cd in cores:
        d = cd["l2"]
        DLO2[:len(d["dlo"])] = np.maximum(DLO2[:len(d["dlo"])], d["dlo"])
        DHI2[:len(d["dhi"])] = np.maximum(DHI2[:len(d["dhi"])], d["dhi"])
    assert (DLO2 >= 1).all()   # self edges (own rows <= G128) are always lo
    assert T2_ROWS - 1 - B2 <= 32767
    for cd in cores:
        l2 = cd["l2"]
        row2 = cd["row2"]
        pos_in_edges = {}
        # idx fns index by src node id; build per-edge instead: _emit_slots
        # passes src ids, but row2 is per-edge. Map via first occurrence is
        # wrong if a src repeats with different rows -- it cannot: row2 is a
        # function of src id only. Build a per-node map lazily.
        row2_of_node = np.empty(N, np.int64)
        row2_of_node[cd["src"]] = row2
        cd["idx2_lo"], cd["idx2_hi"], cd["slot2cmp2"] = _emit_slots(
            l2, DLO2, DHI2,
            lambda s, r=row2_of_node: r[s],
            lambda s, r=row2_of_node: r[s] - B2,
            0, T2_ROWS - 1 - B2)
        # verify self-edge-first in lo block 0
        c = cd["c"]
        P2 = l2["P"]
        own_row = 1 + cd["l1"]["Ppos"]
        for g in range(NG2):
            e = min(128, S - g * 128)
            if e <= 0:
                break
            dsts = P2[g * 128:g * 128 + e]
            assert (cd["idx2_lo"][g][0, :e] == own_row[dsts]).all()

    # ---------- aux ----------
    cnt = np.bincount(batch, minlength=GPOOL).astype(np.float32)
    recip_cnt = (1.0 / np.maximum(cnt, 1.0)).astype(np.float32)

    for cd in cores:
        c = cd["c"]
        gids = batch[c * S:(c + 1) * S]
        Mp = np.zeros((NG2 * 128, GPOOL), np.float32)
        s2c = cd["slot2cmp2"]
        real = s2c >= 0
        Mp[np.where(real)[0], gids[cd["l2"]["P"][s2c[real]]]] = 1.0
        cd["mpool"] = Mp.astype(bf16)

        xt = np.zeros((IN, _ceil_to(N + 2, XCHUNK)), np.float32)
        xt[:, cd["row_of"]] = x.T
        cd["xT"] = xt.astype(bf16)

        segs = []
        for g in range(NG1):
            a = np.concatenate(
                [cd["idx1_lo"][g],
                 cd["idx1_hi"][g] + (LO_MAX + 1)], axis=0)  # [d, 128] abs rows
            segs.append(np.ascontiguousarray(a.T))          # [128, d]
        cd["w_idx32"] = np.concatenate(segs, axis=1).astype(np.int32)
        cd["w_idx1lo"] = _wrap_groups(cd["idx1_lo"])
        cd["w_idx1hi"] = _wrap_groups(cd["idx1_hi"])
        cd["w_idx2lo"] = _wrap_groups(cd["idx2_lo"])
        cd["w_idx2hi"] = _wrap_groups(cd["idx2_hi"])

    patch1 = np.zeros((2, 256), np.float32)
    patch1[:, 128:132] = SPECIAL_ALS
    patch2 = np.zeros((2, 64), np.float32)
    patch2[:, 32] = SPECIAL_ALS

    return dict(cores=cores,
                DLO1=[int(v) for v in DLO1], DHI1=[int(v) for v in DHI1],
                DLO2=[int(v) for v in DLO2], DHI2=[int(v) for v in DHI2],
                W1ext=W1ext.astype(bf16), W2ext=W2ext.astype(bf16),
                Wl=np.asarray(Wl, np.float32),
                b1=np.tile(np.asarray(b1, np.float32).reshape(1, -1),
                           (128, 1)),
                b2=np.tile(np.asarray(b2, np.float32).reshape(1, -1),
                           (128, 1)),
                bl=np.tile(np.asarray(bl, np.float32).reshape(1, -1),
                           (GPOOL, 1)),
                rcnt=np.tile(recip_cnt.reshape(1, -1), (HID, 1)),
                patch1=patch1.astype(bf16), patch2=patch2,
                ident=np.eye(128, dtype=bf16))


def _pack_superblocks(DLO, DHI, budget=SB_BLOCK_BUDGET):
    sbs, cur, tot = [], [], 0
    for g in range(len(DLO)):
        d = int(DLO[g] + DHI[g])
        if cur and tot + d > budget:
            sbs.append(cur)
            cur, tot = [], 0
        cur.append(g)
        tot += d
    if cur:
        sbs.append(cur)
    return sbs


def make_sched(prep):
    DLO1, DHI1 = prep["DLO1"], prep["DHI1"]
    DLO2, DHI2 = prep["DLO2"], prep["DHI2"]
    assert all(a >= 1 for a in DLO1)
    assert all(a >= 1 for a in DLO2)
    return dict(DLO1=DLO1, DHI1=DHI1, DLO2=DLO2, DHI2=DHI2,
                SB1=_pack_superblocks(DLO1, DHI1),
                SB2=_pack_superblocks(DLO2, DHI2),
                HASB1=bool(np.any(prep["b1"])), HASB2=bool(np.any(prep["b2"])),
                HASBL=bool(np.any(prep["bl"])))


# ======================= bass kernel =======================================

def _emit_dummy_out(nc, tc, t_out, dt):
    with tc.tile_pool(name='dummy', bufs=1) as dp:
        d = dp.tile([GPOOL, OUT], dt.float32)
        nc.vector.memset(d[:], 0.0)
        nc.sync.dma_start(t_out[:, :], d[:])


def _chunks_desc(segs, wpack):
    """segs: list of (tile_idx, off, count). Returns wide-matmul chunks
    (tile_idx, off, k<=wpack) sorted largest-first."""
    out = []
    for ti, off, cnt in segs:
        rem = cnt
        o = off
        while rem > 0:
            k = min(wpack, rem)
            out.append((ti, o, k))
            o += k
            rem -= k
    out.sort(key=lambda t: -t[2])
    return out


def build_bass(sc):
    import concourse.bacc as bacc
    import concourse.tile as tile
    import concourse.mybir as mybir
    from concourse.library_config import mlp

    dt = mybir.dt
    Alu = mybir.AluOpType
    Act = mybir.ActivationFunctionType
    Axis = mybir.AxisListType

    DLO1, DHI1 = sc["DLO1"], sc["DHI1"]
    DLO2, DHI2 = sc["DLO2"], sc["DHI2"]
    SB1, SB2 = sc["SB1"], sc["SB2"]
    HASB1 = sc.get("HASB1", True)
    HASB2 = sc.get("HASB2", True)
    HASBL = sc.get("HASBL", True)
    NG1, NG2 = len(DLO1), len(DLO2)
    XT_COLS = _ceil_to(N + 2, XCHUNK)
    NCHUNK = XT_COLS // XCHUNK
    G128 = NG1 * 128
    T2_ROWS = 1 + G128 + NCORES * G128 + 1

    nc = bacc.Bacc("TRN2", target_bir_lowering=False, debug=False,
                   num_devices=NCORES, num_swdge_queues=4)

    t_xT = nc.dram_tensor("xT", [IN, XT_COLS], dt.bfloat16, kind="ExternalInput")
    t_w1 = nc.dram_tensor("w1ext", [IN, 136], dt.bfloat16, kind="ExternalInput")
    t_w2 = nc.dram_tensor("w2ext", [IN, 36], dt.bfloat16, kind="ExternalInput")
    t_wl = nc.dram_tensor("wl", [HID, OUT], dt.float32, kind="ExternalInput")
    t_b1 = nc.dram_tensor("b1", [128, HEADS * HID], dt.float32,
                          kind="ExternalInput")
    t_b2 = nc.dram_tensor("b2", [128, HID], dt.float32, kind="ExternalInput")
    t_bl = nc.dram_tensor("bl", [GPOOL, OUT], dt.float32, kind="ExternalInput")
    t_rcnt = nc.dram_tensor("rcnt", [HID, GPOOL], dt.float32,
                            kind="ExternalInput")
    t_patch1 = nc.dram_tensor("patch1", [2, 256], dt.bfloat16,
                              kind="ExternalInput")
    t_patch2 = nc.dram_tensor("patch2", [2, 64], dt.float32,
                              kind="ExternalInput")
    t_ident = nc.dram_tensor("ident", [128, 128], dt.bfloat16,
                             kind="ExternalInput")
    t_mpool = nc.dram_tensor("mpool", [NG2 * 128, GPOOL], dt.bfloat16,
                             kind="ExternalInput")
    n1lo = max(8 * sum(DLO1), 8)
    n1hi = max(8 * sum(DHI1), 8)
    n2lo = max(8 * sum(DLO2), 8)
    n2hi = max(8 * sum(DHI2), 8)
    n1all = sum(DLO1) + sum(DHI1)
    if K_IND:
        t_i32 = nc.dram_tensor("idx32", [128, n1all], dt.int32,
                               kind="ExternalInput")
    t_i1lo = nc.dram_tensor("idx1lo", [128, n1lo], dt.int16, kind="ExternalInput")
    t_i1hi = nc.dram_tensor("idx1hi", [128, n1hi], dt.int16, kind="ExternalInput")
    t_i2lo = nc.dram_tensor("idx2lo", [128, n2lo], dt.int16, kind="ExternalInput")
    t_i2hi = nc.dram_tensor("idx2hi", [128, n2hi], dt.int16, kind="ExternalInput")
    t_out = nc.dram_tensor("out", [GPOOL, OUT], dt.float32,
                           kind="ExternalOutput")

    rg = [list(range(NCORES))]
    _qc = [0]

    def nextq(ndesc=1):
        _qc[0] = (_qc[0] + 1) % 4
        return _qc[0]

    with tile.TileContext(nc) as tc:
        with (
            tc.tile_pool(name="const", bufs=1) as constp,
            tc.tile_pool(name="dram", bufs=1, space="DRAM") as dramp,
        ):
            nc.gpsimd.load_library(mlp)
            def emit_gather(out_t, tab, idx_t, o0, nb, elem, estep=None,
                            idxp=None, nbmax=None):
                nc.gpsimd.dma_gather(
                    out_t[:, :nb, :], tab,
                    idx_t[:, 8 * o0:8 * (o0 + nb)],
                    128 * nb, 128 * nb, elem, elem_step=estep,
                    single_packet=bool(K_SP), queue_num=nextq(nb))

            table1 = dramp.tile([XT_COLS, 256], dt.bfloat16, tag="table1")
            table2 = dramp.tile([_ceil_to(T2_ROWS, 4), 64], dt.float32,
                                tag="table2")
            cc_in = dramp.tile([HID, GPOOL], dt.float32, tag="ccin")
            cc_out = dramp.tile([NCORES * HID, GPOOL], dt.float32, tag="ccout")

            w1_t = constp.tile([IN, 136], dt.bfloat16)
            nc.sync.dma_start(w1_t[:], t_w1[:])
            w2_t = constp.tile([IN, 36], dt.bfloat16)
            nc.sync.dma_start(w2_t[:], t_w2[:])
            wl_t = constp.tile([HID, OUT], dt.float32)
            nc.sync.dma_start(wl_t[:], t_wl[:])
            b1_t = constp.tile([128, HEADS * HID], dt.float32)
            nc.sync.dma_start(b1_t[:], t_b1[:])
            b2_t = constp.tile([128, HID], dt.float32)
            nc.sync.dma_start(b2_t[:], t_b2[:])
            bl_t = constp.tile([GPOOL, OUT], dt.float32)
            nc.sync.dma_start(bl_t[:], t_bl[:])
            rc_t = constp.tile([HID, GPOOL], dt.float32)
            nc.sync.dma_start(rc_t[:], t_rcnt[:])
            id_t = constp.tile([128, 128], dt.bfloat16)
            nc.sync.dma_start(id_t[:], t_ident[:])
            if PHASES >= 2:
                i1lo_t = constp.tile([128, n1lo], dt.int16)
                nc.sync.dma_start(i1lo_t[:], t_i1lo[:])
                i1hi_t = constp.tile([128, n1hi], dt.int16)
                nc.sync.dma_start(i1hi_t[:], t_i1hi[:])
            if PHASES >= 4:
                i2lo_t = constp.tile([128, n2lo], dt.int16)
                nc.sync.dma_start(i2lo_t[:], t_i2lo[:])
                i2hi_t = constp.tile([128, n2hi], dt.int16)
                nc.sync.dma_start(i2hi_t[:], t_i2hi[:])
            # all gather indices live in SBUF for the whole run

            # ---------------- phase X: build table1 ----------------
            with (
                tc.tile_pool(name="xload", bufs=3) as xlp,
                tc.tile_pool(name="xout", bufs=3) as xop,
                tc.tile_pool(name="xpsum", bufs=4, space="PSUM") as xpp,
            ):
                for t in range(NCHUNK):
                    xt_t = xlp.tile([IN, XCHUNK], dt.bfloat16, tag="xt")
                    nc.sync.dma_start(xt_t[:],
                                      t_xT[:, t * XCHUNK:(t + 1) * XCHUNK])
                    o_t = xop.tile([128, 4, 256], dt.bfloat16, tag="xo")
                    nc.vector.memset(o_t[:, :, 136:256], 0.0)
                    for k in range(4):
                        ps = xpp.tile([128, 136], dt.float32, tag="xp")
                        nc.tensor.matmul(ps[:], xt_t[:, k * 128:(k + 1) * 128],
                                         w1_t[:], start=True, stop=True)
                        if k % 2 == 0:
                            nc.vector.tensor_copy(o_t[:, k, 0:136], ps[:])
                        else:
                            nc.scalar.activation(o_t[:, k, 0:136], ps[:],
                                                 Act.Copy)
                    nc.sync.dma_start(
                        table1[t * XCHUNK:(t + 1) * XCHUNK, :].rearrange(
                            "(k p) e -> p k e", p=128), o_t[:])
            with tc.tile_pool(name="patchp", bufs=1) as pp:
                p1_t = pp.tile([2, 256], dt.bfloat16)
                nc.sync.dma_start(p1_t[:], t_patch1[:])
                nc.sync.dma_start(table1[LO_MAX:LO_MAX + 1, :], p1_t[0:1, :])
                nc.sync.dma_start(table1[N + 1:N + 2, :], p1_t[1:2, :])
                if PHASES >= 2:
                    p2_t = pp.tile([2, 64], dt.float32)
                    nc.sync.dma_start(p2_t[:], t_patch2[:])
                    nc.sync.dma_start(table2[0:1, :], p2_t[0:1, :])
                    nc.sync.dma_start(table2[T2_ROWS - 1:T2_ROWS, :],
                                      p2_t[1:2, :])

            if PHASES >= 2:
                # ---------------- phase L1: edges ----------------
                tab1_lo = table1[0:LO_MAX + 1, :]
                tab1_hi = table1[LO_MAX + 1:N + 2, :]
                Dmax1 = max(DLO1[g] + DHI1[g] for g in range(NG1))
                NBLO1 = max(sum(DLO1[g] for g in sb) for sb in SB1)
                NBHI1 = max(max(sum(DHI1[g] for g in sb) for sb in SB1), 1)
                olo = np.concatenate([[0], np.cumsum(DLO1)]).astype(int)
                ohi = np.concatenate([[0], np.cumsum(DHI1)]).astype(int)
                H1 = (NG1 // 2) * 128   # first AllGather covers groups < NG1//2
                t2main1 = table2[1 + G128:1 + G128 + NCORES * H1, :]
                t2main2 = table2[1 + G128 + NCORES * H1:1 + G128 +
                                 NCORES * G128, :]
                with (
                    tc.tile_pool(name="gath1", bufs=5) as gathp,
                    tc.tile_pool(name="small1", bufs=3) as smallp,
                    tc.tile_pool(name="epi1", bufs=3) as epip,
                    tc.tile_pool(name="cp1", bufs=3) as cpp,
                    tc.tile_pool(name="agg1", bufs=2, space="PSUM") as aggp,
                    tc.tile_pool(name="psT1", bufs=2, space="PSUM") as psTp,
                    tc.tile_pool(name="ps21", bufs=2, space="PSUM") as ps2p,
                ):
                    for sb in SB1:
                        g0 = sb[0]
                        nlo = sum(DLO1[g] for g in sb)
                        nhi = sum(DHI1[g] for g in sb)
                        glo_t = gathp.tile([128, NBLO1, K_ELEM1],
                                           dt.bfloat16, tag="glo")
                        estep = 256 if K_ELEM1 != 256 else None
                        emit_gather(glo_t, tab1_lo, i1lo_t, olo[g0], nlo,
                                    K_ELEM1, estep)
                        ghi_t = gathp.tile([128, NBHI1, K_ELEM1], dt.bfloat16,
                                           tag="ghi")
                        if nhi > 0:
                            emit_gather(ghi_t, tab1_hi, i1hi_t, ohi[g0], nhi,
                                        K_ELEM1, estep)
                        lo_off = 0
                        hi_off = 0
                        for gi, g in enumerate(sb):
                            dlo, dhi = DLO1[g], DHI1[g]
                            D = dlo + dhi
                            if L1STEP < 2:
                                lo_off += dlo
                                hi_off += dhi
                                continue
                            logit_t = smallp.tile([128, Dmax1, 4], dt.float32,
                                                  tag="lg")
                            exb_t = smallp.tile([128, Dmax1, 4], dt.bfloat16,
                                                tag="exb")
                            exs_t = smallp.tile([128, Dmax1, 4], dt.bfloat16,
                                                tag="exs")
                            den_t = smallp.tile([128, 4], dt.float32, tag="dn")
                            rec_t = smallp.tile([128, 4], dt.float32, tag="rc")
                            ald_ap = glo_t[:, lo_off, 132:136]
                            nc.vector.scalar_tensor_tensor(
                                logit_t[:, :dlo, :],
                                glo_t[:, lo_off:lo_off + dlo, 128:132], 0.0,
                                ald_ap.unsqueeze(1).broadcast_to(
                                    (128, dlo, 4)), Alu.add, Alu.add)
                            if dhi > 0:
                                nc.vector.scalar_tensor_tensor(
                                    logit_t[:, dlo:D, :],
                                    ghi_t[:, hi_off:hi_off + dhi, 128:132], 0.0,
                                    ald_ap.unsqueeze(1).broadcast_to(
                                        (128, dhi, 4)), Alu.add, Alu.add)
                            nc.vector.scalar_tensor_tensor(
                                logit_t[:, :D, :], logit_t[:, :D, :], NEG,
                                logit_t[:, :D, :], Alu.mult, Alu.max)
                            nc.scalar.activation(exb_t[:, :D, :],
                                                 logit_t[:, :D, :], Act.Exp)
                            nc.vector.tensor_reduce(
                                den_t[:], exb_t[:, :D, :].transpose([0, 2, 1]),
                                axis=Axis.X, op=Alu.add)
                            nc.vector.reciprocal(rec_t[:], den_t[:])
                            nc.vector.tensor_tensor(
                                exs_t[:, :D, :], exb_t[:, :D, :],
                                rec_t[:].unsqueeze(1).broadcast_to(
                                    (128, D, 4)), Alu.mult)
                            if L1STEP < 3:
                                lo_off += dlo
                                hi_off += dhi
                                continue
                            h_lo = glo_t[:, lo_off:lo_off + dlo, 0:128]
                            h_lo = h_lo.rearrange("p b (h c) -> p b h c", h=4)
                            nc.vector.tensor_tensor(
                                h_lo, h_lo,
                                exs_t[:, :dlo, :].unsqueeze(3).broadcast_to(
                                    (128, dlo, 4, HID)), Alu.mult)
                            if dhi > 0:
                                h_hi = ghi_t[:, hi_off:hi_off + dhi, 0:128]
                                h_hi = h_hi.rearrange("p b (h c) -> p b h c",
                                                      h=4)
                                nc.vector.tensor_tensor(
                                    h_hi, h_hi,
                                    exs_t[:, dlo:D, :].unsqueeze(3).broadcast_to(
                                        (128, dhi, 4, HID)), Alu.mult)
                            if L1STEP < 4:
                                lo_off += dlo
                                hi_off += dhi
                                continue
                            segs = [(0, lo_off, dlo)]
                            if dhi > 0:
                                segs.append((1, hi_off, dhi))
                            chunks = _chunks_desc(segs, WPACK1)
                            lanes = chunks[0][2]
                            agg = aggp.tile([128, WPACK1, 128], dt.float32,
                                            tag="agg")
                            for ci, (ti, off, k) in enumerate(chunks):
                                src_t = glo_t if ti == 0 else ghi_t
                                nc.tensor.matmul(
                                    agg[:, 0:k, :],
                                    id_t[:], src_t[:, off:off + k, 0:128],
                                    start=(ci == 0),
                                    stop=(ci == len(chunks) - 1))
                            if L1STEP < 5:
                                lo_off += dlo
                                hi_off += dhi
                                continue
                            scaled_t = epip.tile([128, 128], dt.float32,
                                                 tag="sd")
                            if lanes == 1:
                                nc.vector.tensor_copy(scaled_t[:],
                                                      agg[:, 0, :])
                            else:
                                nc.vector.tensor_reduce(
                                    scaled_t[:],
                                    agg[:, 0:lanes, :].transpose([0, 2, 1]),
                                    axis=Axis.X, op=Alu.add)
                            if HASB1:
                                nc.vector.tensor_tensor(
                                    scaled_t[:], scaled_t[:], b1_t[:], Alu.add)
                            tmp_t = epip.tile([128, 128], dt.float32, tag="tm")
                            nc.scalar.activation(tmp_t[:], scaled_t[:], Act.Relu,
                                                 scale=-1.0)
                            nc.scalar.activation(tmp_t[:], tmp_t[:], Act.Exp,
                                                 scale=-1.0)
                            elu_t = epip.tile([128, 128], dt.bfloat16, tag="el")
                            nc.vector.scalar_tensor_tensor(
                                elu_t[:], tmp_t[:], -1.0, scaled_t[:],
                                Alu.add, Alu.max)
                            # inline pass-2: h1' @ W2ext -> table2 front rows
                            psT = psTp.tile([128, 128], dt.bfloat16, tag="pt")
                            nc.tensor.transpose(psT[:], elu_t[:], id_t[:])
                            eluT_t = epip.tile([128, 128], dt.bfloat16,
                                               tag="et")
                            nc.scalar.activation(eluT_t[:], psT[:], Act.Copy)
                            ps2 = ps2p.tile([128, 36], dt.float32, tag="p2")
                            nc.tensor.matmul(ps2[:], eluT_t[:], w2_t[:],
                                             start=True, stop=True)
                            cp_t = cpp.tile([128, 64], dt.float32, tag="cp")
                            if g % 2 == 0:
                                nc.scalar.activation(cp_t[:, 0:36], ps2[:],
                                                     Act.Copy)
                            else:
                                nc.vector.tensor_copy(cp_t[:, 0:36], ps2[:])
                            nc.sync.dma_start(
                                table2[1 + 128 * g:1 + 128 * (g + 1), :],
                                cp_t[:])
                            lo_off += dlo
                            hi_off += dhi
                            if PHASES >= 3 and L1STEP >= 5:
                                if 128 * (g + 1) == H1:
                                    nc.gpsimd.collective_compute(
                                        "AllGather", Alu.bypass,
                                        replica_groups=rg,
                                        ins=[table2[1:1 + H1, :]],
                                        outs=[t2main1[:, :]])
                    if PHASES >= 3 and L1STEP >= 5:
                        nc.gpsimd.collective_compute(
                            "AllGather", Alu.bypass, replica_groups=rg,
                            ins=[table2[1 + H1:1 + G128, :]],
                            outs=[t2main2[:, :]])

            if PHASES >= 4:
                # ---------------- phase L2: edges + pool ----------------
                B2 = 1 + G128 + NCORES * ((NG1 // 2) * 128)
                tab2_lo = table2[0:B2, :]
                tab2_hi = table2[B2:T2_ROWS, :]
                Dmax2 = max(DLO2[g] + DHI2[g] for g in range(NG2))
                NBLO2 = max(sum(DLO2[g] for g in sb) for sb in SB2)
                NBHI2 = max(max(sum(DHI2[g] for g in sb) for sb in SB2), 1)
                olo2 = np.concatenate([[0], np.cumsum(DLO2)]).astype(int)
                ohi2 = np.concatenate([[0], np.cumsum(DHI2)]).astype(int)
                with (
                    tc.tile_pool(name="gath2", bufs=8) as gathp,
                    tc.tile_pool(name="small2", bufs=4) as smallp,
                    tc.tile_pool(name="epi2", bufs=3) as epip,
                    tc.tile_pool(name="agg2", bufs=2, space="PSUM") as aggp,
                    tc.tile_pool(name="poolps", bufs=1, space="PSUM") as poolpp,
                    tc.tile_pool(name="mp2", bufs=3) as mpp,
                ):
                    poolps = poolpp.tile([HID, GPOOL], dt.float32)
                    h2p_all = mpp.tile([128, NG2, HID], dt.bfloat16,
                                       tag="h2pall", bufs=1)
                    for sb in SB2:
                        g0 = sb[0]
                        nlo = sum(DLO2[g] for g in sb)
                        nhi = sum(DHI2[g] for g in sb)
                        glo_t = gathp.tile([128, NBLO2, 64], dt.float32,
                                           tag="glo")
                        emit_gather(glo_t, tab2_lo, i2lo_t, olo2[g0], nlo, 64)
                        ghi_t = gathp.tile([128, NBHI2, 64], dt.float32,
                                           tag="ghi")
                        if nhi > 0:
                            emit_gather(ghi_t, tab2_hi, i2hi_t, ohi2[g0], nhi,
                                        64)
                        lo_off = 0
                        hi_off = 0
                        for gi, g in enumerate(sb):
                            dlo, dhi = DLO2[g], DHI2[g]
                            D = dlo + dhi
                            logit_t = smallp.tile([128, Dmax2, 1], dt.float32,
                                                  tag="lg")
                            exf_t = smallp.tile([128, Dmax2, 1], dt.float32,
                                                tag="exf")
                            exs_t = smallp.tile([128, Dmax2, 1], dt.float32,
                                                tag="exs")
                            den_t = smallp.tile([128, 1], dt.float32, tag="dn")
                            rec_t = smallp.tile([128, 1], dt.float32, tag="rc")
                            ald_ap = glo_t[:, lo_off, 33:34]
                            nc.vector.scalar_tensor_tensor(
                                logit_t[:, :dlo, :],
                                glo_t[:, lo_off:lo_off + dlo, 32:33], 0.0,
                                ald_ap.unsqueeze(1).broadcast_to(
                                    (128, dlo, 1)), Alu.add, Alu.add)
                            if dhi > 0:
                                nc.vector.scalar_tensor_tensor(
                                    logit_t[:, dlo:D, :],
                                    ghi_t[:, hi_off:hi_off + dhi, 32:33], 0.0,
                                    ald_ap.unsqueeze(1).broadcast_to(
                                        (128, dhi, 1)), Alu.add, Alu.add)
                            nc.vector.scalar_tensor_tensor(
                                logit_t[:, :D, :], logit_t[:, :D, :], NEG,
                                logit_t[:, :D, :], Alu.mult, Alu.max)
                            nc.scalar.activation(exf_t[:, :D, :],
                                                 logit_t[:, :D, :], Act.Exp)
                            nc.vector.tensor_reduce(
                                den_t[:], exf_t[:, :D, :].transpose([0, 2, 1]),
                                axis=Axis.X, op=Alu.add)
                            nc.vector.reciprocal(rec_t[:], den_t[:])
                            nc.vector.tensor_tensor(
                                exs_t[:, :D, :], exf_t[:, :D, :],
                                rec_t[:].unsqueeze(1).broadcast_to(
                                    (128, D, 1)), Alu.mult)
                            exh_t = smallp.tile([128, Dmax2, HID], dt.bfloat16,
                                                tag="exh")
                            nc.vector.tensor_tensor(
                                exh_t[:, :dlo, :],
                                glo_t[:, lo_off:lo_off + dlo, 0:HID],
                                exs_t[:, :dlo, :].broadcast_to(
                                    (128, dlo, HID)), Alu.mult)
                            if dhi > 0:
                                nc.vector.tensor_tensor(
                                    exh_t[:, dlo:D, :],
                                    ghi_t[:, hi_off:hi_off + dhi, 0:HID],
                                    exs_t[:, dlo:D, :].broadcast_to(
                                        (128, dhi, HID)), Alu.mult)
                            chunks = _chunks_desc([(0, 0, D)], WPACK2)
                            lanes = chunks[0][2]
                            agg = aggp.tile([128, WPACK2, HID], dt.float32,
                                            tag="agg")
                            for ci, (ti, off, k) in enumerate(chunks):
                                nc.tensor.matmul(
                                    agg[:, 0:k, :],
                                    id_t[:], exh_t[:, off:off + k, :],
                                    start=(ci == 0),
                                    stop=(ci == len(chunks) - 1))
                            scaled_t = epip.tile([128, HID], dt.float32,
                                                 tag="sd")
                            if lanes == 1:
                                nc.vector.tensor_copy(scaled_t[:],
                                                      agg[:, 0, :])
                            else:
                                nc.vector.tensor_reduce(
                                    scaled_t[:],
                                    agg[:, 0:lanes, :].transpose([0, 2, 1]),
                                    axis=Axis.X, op=Alu.add)
                            if HASB2:
                                nc.vector.tensor_tensor(
                                    scaled_t[:], scaled_t[:], b2_t[:], Alu.add)
                            tmp_t = epip.tile([128, HID], dt.float32, tag="tm")
                            nc.scalar.activation(tmp_t[:], scaled_t[:], Act.Relu,
                                                 scale=-1.0)
                            nc.scalar.activation(tmp_t[:], tmp_t[:], Act.Exp,
                                                 scale=-1.0)
                            nc.vector.scalar_tensor_tensor(
                                h2p_all[:, g, :], tmp_t[:], -1.0, scaled_t[:],
                                Alu.add, Alu.max)
                            lo_off += dlo
                            hi_off += dhi

                    for g in range(NG2):
                        mp_t = mpp.tile([128, GPOOL], dt.bfloat16, tag="mp")
                        nc.sync.dma_start(mp_t[:],
                                          t_mpool[g * 128:(g + 1) * 128, :])
                        nc.tensor.matmul(poolps[:], h2p_all[:, g, :], mp_t[:],
                                         start=(g == 0), stop=(g == NG2 - 1))
                    # ------------- pool + final linear -------------
                    with tc.tile_pool(name="fin", bufs=1) as finp, \
                            tc.tile_pool(name="finps", bufs=1, space="PSUM") as fpp:
                        poolsb = finp.tile([HID, GPOOL], dt.float32)
                        nc.vector.tensor_copy(poolsb[:], poolps[:])
                        nc.sync.dma_start(cc_in[:, :], poolsb[:])
                        nc.gpsimd.collective_compute(
                            "AllGather", Alu.bypass, replica_groups=rg,
                            ins=[cc_in[:, :]], outs=[cc_out[:, :]])
                        parts_t = finp.tile([HID, NCORES, GPOOL], dt.float32)
                        nc.sync.dma_start(
                            parts_t[:],
                            cc_out[:, :].rearrange("(c p) e -> p c e",
                                                   p=HID))
                        psum_t = finp.tile([HID, GPOOL], dt.float32)
                        nc.vector.tensor_reduce(
                            psum_t[:], parts_t[:].transpose([0, 2, 1]),
                            axis=Axis.X, op=Alu.add)
                        mean_t = finp.tile([HID, GPOOL], dt.float32)
                        nc.vector.tensor_tensor(
                            mean_t[:], psum_t[:],
                            rc_t[:], Alu.mult)
                        psO = fpp.tile([GPOOL, OUT], dt.float32)
                        nc.tensor.matmul(psO[:], mean_t[:], wl_t[:], start=True,
                                         stop=True)
                        out_t = finp.tile([GPOOL, OUT], dt.float32)
                        if HASBL:
                            nc.vector.tensor_tensor(out_t[:], psO[:], bl_t[:],
                                                    Alu.add)
                        else:
                            nc.vector.tensor_copy(out_t[:], psO[:])
                        nc.sync.dma_start(t_out[:, :], out_t[:])
            if PHASES < 4:
                _emit_dummy_out(nc, tc, t_out, dt)


    nc.compile()
    return nc


def core_inputs(prep, c):
    cd = prep["cores"][c]

    def padcols(a, cols):
        if a.shape[1] == cols:
            return a
        out = np.zeros((a.shape[0], cols), a.dtype)
        out[:, :a.shape[1]] = a
        return out

    n1lo = max(8 * sum(prep["DLO1"]), 8)
    n1hi = max(8 * sum(prep["DHI1"]), 8)
    n2lo = max(8 * sum(prep["DLO2"]), 8)
    n2hi = max(8 * sum(prep["DHI2"]), 8)
    return dict(
        xT=np.ascontiguousarray(cd["xT"]),
        w1ext=prep["W1ext"], w2ext=prep["W2ext"], wl=prep["Wl"],
        b1=prep["b1"], b2=prep["b2"], bl=prep["bl"], rcnt=prep["rcnt"],
        patch1=prep["patch1"], patch2=prep["patch2"], ident=prep["ident"],
        mpool=np.ascontiguousarray(cd["mpool"]),
        **({"idx32": cd["w_idx32"]} if K_IND else {}),
        idx1lo=padcols(cd["w_idx1lo"], n1lo),
        idx1hi=padcols(cd["w_idx1hi"], n1hi),
        idx2lo=padcols(cd["w_idx2lo"], n2lo),
        idx2hi=padcols(cd["w_idx2hi"], n2hi),
    )


_CACHE = {}


def kernel(**inputs):
    from concourse.bass_utils import run_bass_kernel_spmd

    inputs = {k: np.asarray(v) for k, v in inputs.items()}
    prep = host_prep(**inputs)
    sc = make_sched(prep)
    key = str(sc)
    if key not in _CACHE:
        _CACHE[key] = build_bass(sc)
    nc = _CACHE[key]
    in_maps = [core_inputs(prep, c) for c in range(NCORES)]
    res = run_bass_kernel_spmd(nc, in_maps, list(range(NCORES)))
    return np.asarray(res.results[0]["out"], np.float32)


# revision 26
# speedup vs baseline: 1.1132x; 1.0014x over previous
# Self-contained 8-core Trainium2 Bass kernel for the 2-layer GAT + mean-pool
# problem (nn_GAT_83820581749190).
#
# Sharding: destination nodes (and all their incident edges) are partitioned
# across the 8 cores, so each layer's attention softmax and aggregation
# complete locally per core. Each core builds a replicated layer-1 feature
# table [h1 | al_src | al_dst] (bf16, 512-byte rows) in HBM with a replicated
# x @ W1ext matmul, edge-gathers rows with the GPSIMD dma_gather custom op
# (int16 indices force a lo/hi table-half split), computes the edge softmax
# without segment-max (logits are small enough that exp cannot overflow), and
# aggregates per-destination with wide identity-matmul PSUM accumulation
# (destinations sit on partitions via degree-bucketed groups of 128; up to 4
# degree-lanes per matmul, reduced on Vector). Each destination's self-loop
# edge is forced to be the first lo-half edge, so the gathered block 0 carries
# the per-destination a_dst logits for free. Layer-1 group outputs are pushed
# through W2ext inline and written to a layer-2 feature table in L1-slot
# order; two split AllGathers (overlapped with the L1 tail) replicate it.
# Layer 2 repeats the gather/softmax/aggregate pattern (16 lanes per matmul);
# mean-pool is a matmul against a host-built one-hot graph matrix plus a tiny
# AllReduce.
import os
import numpy as np
import ml_dtypes

N = 50000
E = 800000
IN = 128
HID = 32
HEADS = 4
OUT = 10
GPOOL = 64
NEG = 0.2
NCORES = 8
S = N // NCORES
LO_MAX = 32767          # max usable int16 gather index
SPECIAL_ALS = -100.0    # al_src of pad rows: exp(0.2*(-100+ald)) ~ 2e-9
SB_BLOCK_BUDGET = int(os.environ.get("K_BUD", "48"))  # max gather blocks per superblock
XCHUNK = 512
WPACK1 = 4              # layer-1 psum lanes per wide matmul
WPACK2 = 16             # layer-2 psum lanes per wide matmul
PHASES = int(os.environ.get("K_PHASES", "99"))  # 1=X only, 2=+L1, 3=+exchange, 4=+L2
L1STEP = int(os.environ.get("K_L1STEP", "99"))  # 1=gathers 2=+softmax 3=+exh 4=+agg 5=+epi
K_ELEM1 = int(os.environ.get("K_ELEM1", "256"))  # L1 gather elem (floor experiments)
K_SP = int(os.environ.get("K_SP", "0"))          # single_packet on gathers
K_IND = int(os.environ.get("K_IND", "0"))        # use indirect_dma_start (floor test)

bf16 = ml_dtypes.bfloat16


def _ceil_to(v, m):
    return (v + m - 1) // m * m


# ======================= host prep =========================================

def _boundary_aware_order(deg_lo, deg_hi):
    """Sort ids by (lo desc, hi desc), but fill 128-groups that straddle a
    lo-run boundary from the *small-hi tail* of the next run, keeping
    per-group max_lo + max_hi tight."""
    Sn = len(deg_lo)
    base = np.lexsort((-deg_hi, -deg_lo))
    glo = deg_lo[base]
    runs = []
    i = 0
    while i < Sn:
        j = i
        while j < Sn and glo[j] == glo[i]:
            j += 1
        runs.append(list(base[i:j]))
        i = j
    order = []
    ri = 0
    fronts = [0] * len(runs)
    backs = [len(r) for r in runs]
    while len(order) < Sn:
        while ri < len(runs) and fronts[ri] >= backs[ri]:
            ri += 1
        if ri >= len(runs):
            break
        need = 128 - (len(order) % 128)
        avail = backs[ri] - fronts[ri]
        if avail >= need:
            order.extend(runs[ri][fronts[ri]:fronts[ri] + need])
            fronts[ri] += need
        else:
            order.extend(runs[ri][fronts[ri]:backs[ri]])
            fronts[ri] = backs[ri]
            need -= avail
            rj = ri + 1
            while need > 0 and rj < len(runs):
                a = backs[rj] - fronts[rj]
                t = min(a, need)
                order.extend(reversed(runs[rj][backs[rj] - t:backs[rj]]))
                backs[rj] -= t
                need -= t
                rj += 1
    P = np.asarray(order, np.int64)
    Ppos = np.empty(Sn, np.int64)
    Ppos[P] = np.arange(Sn)
    return P, Ppos


def _run_groups(glo, ghi, max_rows=128):
    Sn = len(glo)
    ng = (Sn + max_rows - 1) // max_rows
    dlo = np.zeros(ng, np.int64)
    dhi = np.zeros(ng, np.int64)
    for g in range(ng):
        s, e = g * max_rows, min((g + 1) * max_rows, Sn)
        dlo[g] = glo[s:e].max()
        dhi[g] = ghi[s:e].max()
    return dlo, dhi


def _build_layer(src, dstl, is_lo):
    deg_lo = np.bincount(dstl[is_lo], minlength=S)
    deg_hi = np.bincount(dstl[~is_lo], minlength=S)
    P, Ppos = _boundary_aware_order(deg_lo, deg_hi)
    dlo, dhi = _run_groups(deg_lo[P], deg_hi[P])
    return dict(src=src, dstl=dstl, is_lo=is_lo, deg_lo=deg_lo, deg_hi=deg_hi,
                P=P, Ppos=Ppos, dlo=dlo, dhi=dhi)


def _emit_slots(l, DLO, DHI, idx_lo_of, idx_hi_of, special_lo, special_hi):
    NG = len(DLO)
    src, is_lo = l["src"], l["is_lo"]
    Ppos = l["Ppos"]
    nreal = len(l["P"])
    slot2cmp = np.full(NG * 128, -1, np.int64)
    slot2cmp[:nreal] = np.arange(nreal)
    idx_lo = [np.full((int(DLO[g]), 128), special_lo, np.int64)
              for g in range(NG)]
    idx_hi = [np.full((int(DHI[g]), 128), special_hi, np.int64)
              for g in range(NG)]
    slot_of_edge = Ppos[l["dstl"]]
    order = np.argsort(slot_of_edge, kind="stable")
    for mask, arrs, idx_fn in ((is_lo, idx_lo, idx_lo_of),
                               (~is_lo, idx_hi, idx_hi_of)):
        m = mask[order]
        so = slot_of_edge[order][m]
        sr = src[order][m]
        jj = np.arange(len(so)) - np.searchsorted(so, so, side="left")
        gg, kk = so // 128, so % 128
        vals = idx_fn(sr)
        for g in range(NG):
            sel = gg == g
            if sel.any():
                arrs[g][jj[sel], kk[sel]] = vals[sel]
    return idx_lo, idx_hi, slot2cmp


def _wrap16(idx):
    """[n] -> [128, n//16] int16: idx i at [i%16, i//16], replicated x8."""
    n = len(idx)
    assert n % 16 == 0
    w = np.ascontiguousarray(np.asarray(idx).reshape(n // 16, 16).T)
    w = w.astype(np.int16)
    return np.tile(w, (8, 1))


def _wrap_groups(arrs):
    segs = [_wrap16(a.reshape(-1)) if a.size else np.zeros((128, 0), np.int16)
            for a in arrs]
    return np.concatenate(segs, axis=1) if segs else np.zeros((128, 0), np.int16)


def host_prep(x, edge_index, batch, W1, a1_src, a1_dst, b1, W2, a2_src, a2_dst,
              b2, Wl, bl):
    x = np.asarray(x, np.float32)
    edge_index = np.asarray(edge_index, np.int64)
    batch = np.asarray(batch, np.int64)
    # self-loops FIRST: each dst's self edge is its first (lo) edge, so the
    # gathered lo block 0 carries the per-dst a_dst logit columns.
    src_all = np.concatenate([np.arange(N, dtype=np.int64), edge_index[0]])
    dst_all = np.concatenate([np.arange(N, dtype=np.int64), edge_index[1]])
    owner = dst_all // S

    a1_src = np.asarray(a1_src, np.float32)
    a1_dst = np.asarray(a1_dst, np.float32)
    W1 = np.asarray(W1, np.float32)
    W2 = np.asarray(W2, np.float32)
    As1 = np.zeros((HEADS * HID, HEADS), np.float32)
    Ad1 = np.zeros((HEADS * HID, HEADS), np.float32)
    for h in range(HEADS):
        As1[h * HID:(h + 1) * HID, h] = a1_src[h]
        Ad1[h * HID:(h + 1) * HID, h] = a1_dst[h]
    W1ext = np.concatenate([W1, W1 @ As1, W1 @ Ad1], axis=1)   # [128,136]
    W2ext = np.concatenate(
        [W2, W2 @ np.asarray(a2_src, np.float32)[0][:, None],
         W2 @ np.asarray(a2_dst, np.float32)[0][:, None],
         np.zeros((HEADS * HID, 2), np.float32)], axis=1)  # [128,36]

    cores = [dict(c=c) for c in range(NCORES)]
    for cd in cores:
        c = cd["c"]
        m = owner == c
        cd["src"] = src_all[m]
        cd["dstl"] = dst_all[m] - c * S

    # ---------- layer 1 ----------
    for cd in cores:
        c = cd["c"]
        pos_of = np.empty(N, np.int64)
        own = np.arange(c * S, (c + 1) * S)
        oth = np.concatenate([np.arange(0, c * S), np.arange((c + 1) * S, N)])
        pos_of[oth] = S + np.arange(N - S)
        pos_of[own] = 0
        is_lo1 = pos_of[cd["src"]] < LO_MAX
        l1 = _build_layer(cd["src"], cd["dstl"], is_lo1)
        pos_of[own] = l1["Ppos"]
        row_of = np.where(pos_of < LO_MAX, pos_of, pos_of + 1)
        cd["l1"] = l1
        cd["row_of"] = row_of
    NG1 = max(len(cd["l1"]["dlo"]) for cd in cores)
    DLO1 = np.zeros(NG1, np.int64)
    DHI1 = np.zeros(NG1, np.int64)
    for cd in cores:
        d = cd["l1"]
        DLO1[:len(d["dlo"])] = np.maximum(DLO1[:len(d["dlo"])], d["dlo"])
        DHI1[:len(d["dhi"])] = np.maximum(DHI1[:len(d["dhi"])], d["dhi"])
    assert (DLO1 >= 1).all()   # self edges are always lo
    for cd in cores:
        r = cd["row_of"]
        cd["idx1_lo"], cd["idx1_hi"], cd["slot2cmp1"] = _emit_slots(
            cd["l1"], DLO1, DHI1,
            lambda s, r=r: r[s], lambda s, r=r: r[s] - (LO_MAX + 1),
            LO_MAX, N + 1 - (LO_MAX + 1))
        # verify self-edge-first: lo block 0 of each group holds the dst's own
        # row for every real slot
        c = cd["c"]
        P1 = cd["l1"]["P"]
        for g in range(NG1):
            e = min(128, S - g * 128)
            if e <= 0:
                break
            dsts = P1[g * 128:g * 128 + e]
            assert (cd["idx1_lo"][g][0, :e] == r[dsts + c * S]).all()

    G128 = NG1 * 128
    H1 = (NG1 // 2) * 128   # first split-AllGather covers slots < H1
    # ---------- layer 2 ----------
    # table2 layout: [0]=patch-lo | [1 : 1+G128) own slots (AllGather input) |
    # [1+G128 : +8*H1) all cores' slots < H1 | [.. : +8*(G128-H1)) all cores'
    # slots >= H1 | [last]=patch-hi. Two contiguous AllGather segments.
    T2_ROWS = 1 + G128 + NCORES * G128 + 1
    B2 = 1 + G128 + NCORES * H1      # lo/hi boundary = start of 2nd AllGather
    assert B2 <= LO_MAX + 1
    row2_main = np.empty(N, np.int64)
    for cd in cores:
        c = cd["c"]
        pp1 = cd["l1"]["Ppos"]
        r = np.where(pp1 < H1,
                     1 + G128 + c * H1 + pp1,
                     1 + G128 + NCORES * H1 + c * (G128 - H1) + (pp1 - H1))
        row2_main[c * S:(c + 1) * S] = r
    for cd in cores:
        c = cd["c"]
        src = cd["src"]
        own_m = (src // S) == c
        row2 = row2_main[src].copy()
        row2[own_m] = 1 + cd["l1"]["Ppos"][src[own_m] - c * S]
        cd["row2"] = row2
        is_lo2 = row2 < B2
        cd["l2"] = _build_layer(src, cd["dstl"], is_lo2)
    NG2 = max(len(cd["l2"]["dlo"]) for cd in cores)
    DLO2 = np.zeros(NG2, np.int64)
    DHI2 = np.zeros(NG2, np.int64)
    for cd in cores:
        d = cd["l2"]
        DLO2[:len(d["dlo"])] = np.maximum(DLO2[:len(d["dlo"])], d["dlo"])
        DHI2[:len(d["dhi"])] = np.maximum(DHI2[:len(d["dhi"])], d["dhi"])
    assert (DLO2 >= 1).all()   # self edges (own rows <= G128) are always lo
    assert T2_ROWS - 1 - B2 <= 32767
    for cd in cores:
        l2 = cd["l2"]
        row2 = cd["row2"]
        pos_in_edges = {}
        # idx fns index by src node id; build per-edge instead: _emit_slots
        # passes src ids, but row2 is per-edge. Map via first occurrence is
        # wrong if a src repeats with different rows -- it cannot: row2 is a
        # function of src id only. Build a per-node map lazily.
        row2_of_node = np.empty(N, np.int64)
        row2_of_node[cd["src"]] = row2
        cd["idx2_lo"], cd["idx2_hi"], cd["slot2cmp2"] = _emit_slots(
            l2, DLO2, DHI2,
            lambda s, r=row2_of_node: r[s],
            lambda s, r=row2_of_node: r[s] - B2,
            0, T2_ROWS - 1 - B2)
        # verify self-edge-first in lo block 0
        c = cd["c"]
        P2 = l2["P"]
        own_row = 1 + cd["l1"]["Ppos"]
        for g in range(NG2):
            e = min(128, S - g * 128)
            if e <= 0:
                break
            dsts = P2[g * 128:g * 128 + e]
            assert (cd["idx2_lo"][g][0, :e] == own_row[dsts]).all()

    # ---------- aux ----------
    cnt = np.bincount(batch, minlength=GPOOL).astype(np.float32)
    recip_cnt = (1.0 / np.maximum(cnt, 1.0)).astype(np.float32)

    for cd in cores:
        c = cd["c"]
        gids = batch[c * S:(c + 1) * S]
        Mp = np.zeros((NG2 * 128, GPOOL), np.float32)
        s2c = cd["slot2cmp2"]
        real = s2c >= 0
        Mp[np.where(real)[0], gids[cd["l2"]["P"][s2c[real]]]] = 1.0
        cd["mpool"] = Mp.astype(bf16)

        xt = np.zeros((IN, _ceil_to(N + 2, XCHUNK)), np.float32)
        xt[:, cd["row_of"]] = x.T
        cd["xT"] = xt.astype(bf16)

        segs = []
        for g in range(NG1):
            a = np.concatenate(
                [cd["idx1_lo"][g],
                 cd["idx1_hi"][g] + (LO_MAX + 1)], axis=0)  # [d, 128] abs rows
            segs.append(np.ascontiguousarray(a.T))          # [128, d]
        cd["w_idx32"] = np.concatenate(segs, axis=1).astype(np.int32)
        cd["w_idx1lo"] = _wrap_groups(cd["idx1_lo"])
        cd["w_idx1hi"] = _wrap_groups(cd["idx1_hi"])
        cd["w_idx2lo"] = _wrap_groups(cd["idx2_lo"])
        cd["w_idx2hi"] = _wrap_groups(cd["idx2_hi"])

    patch1 = np.zeros((2, 256), np.float32)
    patch1[:, 128:132] = SPECIAL_ALS
    patch2 = np.zeros((2, 64), np.float32)
    patch2[:, 32] = SPECIAL_ALS

    return dict(cores=cores,
                DLO1=[int(v) for v in DLO1], DHI1=[int(v) for v in DHI1],
                DLO2=[int(v) for v in DLO2], DHI2=[int(v) for v in DHI2],
                W1ext=W1ext.astype(bf16), W2ext=W2ext.astype(bf16),
                Wl=np.asarray(Wl, np.float32),
                b1=np.tile(np.asarray(b1, np.float32).reshape(1, -1),
                           (128, 1)),
                b2=np.tile(np.asarray(b2, np.float32).reshape(1, -1),
                           (128, 1)),
                bl=np.tile(np.asarray(bl, np.float32).reshape(1, -1),
                           (GPOOL, 1)),
                rcnt=np.tile(recip_cnt.reshape(1, -1), (HID, 1)),
                patch1=patch1.astype(bf16), patch2=patch2,
                ident=np.eye(128, dtype=bf16))


def _pack_superblocks(DLO, DHI, budget=SB_BLOCK_BUDGET):
    sbs, cur, tot = [], [], 0
    for g in range(len(DLO)):
        d = int(DLO[g] + DHI[g])
        if cur and tot + d > budget:
            sbs.append(cur)
            cur, tot = [], 0
        cur.append(g)
        tot += d
    if cur:
        sbs.append(cur)
    return sbs


def make_sched(prep):
    DLO1, DHI1 = prep["DLO1"], prep["DHI1"]
    DLO2, DHI2 = prep["DLO2"], prep["DHI2"]
    assert all(a >= 1 for a in DLO1)
    assert all(a >= 1 for a in DLO2)
    return dict(DLO1=DLO1, DHI1=DHI1, DLO2=DLO2, DHI2=DHI2,
                SB1=_pack_superblocks(DLO1, DHI1),
                SB2=_pack_superblocks(DLO2, DHI2),
                HASB1=bool(np.any(prep["b1"])), HASB2=bool(np.any(prep["b2"])),
                HASBL=bool(np.any(prep["bl"])))


# ======================= bass kernel =======================================

def _emit_dummy_out(nc, tc, t_out, dt):
    with tc.tile_pool(name='dummy', bufs=1) as dp:
        d = dp.tile([GPOOL, OUT], dt.float32)
        nc.vector.memset(d[:], 0.0)
        nc.sync.dma_start(t_out[:, :], d[:])


def _chunks_desc(segs, wpack):
    """segs: list of (tile_idx, off, count). Returns wide-matmul chunks
    (tile_idx, off, k<=wpack) sorted largest-first."""
    out = []
    for ti, off, cnt in segs:
        rem = cnt
        o = off
        while rem > 0:
            k = min(wpack, rem)
            out.append((ti, o, k))
            o += k
            rem -= k
    out.sort(key=lambda t: -t[2])
    return out


def build_bass(sc):
    import concourse.bacc as bacc
    import concourse.tile as tile
    import concourse.mybir as mybir
    from concourse.library_config import mlp

    dt = mybir.dt
    Alu = mybir.AluOpType
    Act = mybir.ActivationFunctionType
    Axis = mybir.AxisListType

    DLO1, DHI1 = sc["DLO1"], sc["DHI1"]
    DLO2, DHI2 = sc["DLO2"], sc["DHI2"]
    SB1, SB2 = sc["SB1"], sc["SB2"]
    HASB1 = sc.get("HASB1", True)
    HASB2 = sc.get("HASB2", True)
    HASBL = sc.get("HASBL", True)
    NG1, NG2 = len(DLO1), len(DLO2)
    XT_COLS = _ceil_to(N + 2, XCHUNK)
    NCHUNK = XT_COLS // XCHUNK
    G128 = NG1 * 128
    T2_ROWS = 1 + G128 + NCORES * G128 + 1

    nc = bacc.Bacc("TRN2", target_bir_lowering=False, debug=False,
                   num_devices=NCORES, num_swdge_queues=4)

    t_xT = nc.dram_tensor("xT", [IN, XT_COLS], dt.bfloat16, kind="ExternalInput")
    t_w1 = nc.dram_tensor("w1ext", [IN, 136], dt.bfloat16, kind="ExternalInput")
    t_w2 = nc.dram_tensor("w2ext", [IN, 36], dt.bfloat16, kind="ExternalInput")
    t_wl = nc.dram_tensor("wl", [HID, OUT], dt.float32, kind="ExternalInput")
    t_b1 = nc.dram_tensor("b1", [128, HEADS * HID], dt.float32,
                          kind="ExternalInput")
    t_b2 = nc.dram_tensor("b2", [128, HID], dt.float32, kind="ExternalInput")
    t_bl = nc.dram_tensor("bl", [GPOOL, OUT], dt.float32, kind="ExternalInput")
    t_rcnt = nc.dram_tensor("rcnt", [HID, GPOOL], dt.float32,
                            kind="ExternalInput")
    t_patch1 = nc.dram_tensor("patch1", [2, 256], dt.bfloat16,
                              kind="ExternalInput")
    t_patch2 = nc.dram_tensor("patch2", [2, 64], dt.float32,
                              kind="ExternalInput")
    t_ident = nc.dram_tensor("ident", [128, 128], dt.bfloat16,
                             kind="ExternalInput")
    t_mpool = nc.dram_tensor("mpool", [NG2 * 128, GPOOL], dt.bfloat16,
                             kind="ExternalInput")
    n1lo = max(8 * sum(DLO1), 8)
    n1hi = max(8 * sum(DHI1), 8)
    n2lo = max(8 * sum(DLO2), 8)
    n2hi = max(8 * sum(DHI2), 8)
    n1all = sum(DLO1) + sum(DHI1)
    if K_IND:
        t_i32 = nc.dram_tensor("idx32", [128, n1all], dt.int32,
                               kind="ExternalInput")
    t_i1lo = nc.dram_tensor("idx1lo", [128, n1lo], dt.int16, kind="ExternalInput")
    t_i1hi = nc.dram_tensor("idx1hi", [128, n1hi], dt.int16, kind="ExternalInput")
    t_i2lo = nc.dram_tensor("idx2lo", [128, n2lo], dt.int16, kind="ExternalInput")
    t_i2hi = nc.dram_tensor("idx2hi", [128, n2hi], dt.int16, kind="ExternalInput")
    t_out = nc.dram_tensor("out", [GPOOL, OUT], dt.float32,
                           kind="ExternalOutput")

    rg = [list(range(NCORES))]
    _qc = [0]

    def nextq(ndesc=1):
        _qc[0] = (_qc[0] + 1) % 4
        return _qc[0]

    with tile.TileContext(nc) as tc:
        with (
            tc.tile_pool(name="const", bufs=1) as constp,
            tc.tile_pool(name="dram", bufs=1, space="DRAM") as dramp,
        ):
            nc.gpsimd.load_library(mlp)
            def emit_gather(out_t, tab, idx_t, o0, nb, elem, estep=None,
                            idxp=None, nbmax=None):
                nc.gpsimd.dma_gather(
                    out_t[:, :nb, :], tab,
                    idx_t[:, 8 * o0:8 * (o0 + nb)],
                    128 * nb, 128 * nb, elem, elem_step=estep,
                    single_packet=bool(K_SP), queue_num=nextq(nb))

            table1 = dramp.tile([XT_COLS, 256], dt.bfloat16, tag="table1")
            table2 = dramp.tile([_ceil_to(T2_ROWS, 4), 64], dt.float32,
                                tag="table2")
            cc_in = dramp.tile([HID, GPOOL], dt.float32, tag="ccin")
            cc_out = dramp.tile([NCORES * HID, GPOOL], dt.float32, tag="ccout")

            w1_t = constp.tile([IN, 136], dt.bfloat16)
            nc.sync.dma_start(w1_t[:], t_w1[:])
            w2_t = constp.tile([IN, 36], dt.bfloat16)
            nc.sync.dma_start(w2_t[:], t_w2[:])
            wl_t = constp.tile([HID, OUT], dt.float32)
            nc.sync.dma_start(wl_t[:], t_wl[:])
            b1_t = constp.tile([128, HEADS * HID], dt.float32)
            nc.sync.dma_start(b1_t[:], t_b1[:])
            b2_t = constp.tile([128, HID], dt.float32)
            nc.sync.dma_start(b2_t[:], t_b2[:])
            bl_t = constp.tile([GPOOL, OUT], dt.float32)
            nc.sync.dma_start(bl_t[:], t_bl[:])
            rc_t = constp.tile([HID, GPOOL], dt.float32)
            nc.sync.dma_start(rc_t[:], t_rcnt[:])
            id_t = constp.tile([128, 128], dt.bfloat16)
            nc.sync.dma_start(id_t[:], t_ident[:])
            if PHASES >= 2:
                i1lo_t = constp.tile([128, n1lo], dt.int16)
                nc.sync.dma_start(i1lo_t[:], t_i1lo[:])
                i1hi_t = constp.tile([128, n1hi], dt.int16)
                nc.sync.dma_start(i1hi_t[:], t_i1hi[:])
            if PHASES >= 4:
                i2lo_t = constp.tile([128, n2lo], dt.int16)
                nc.sync.dma_start(i2lo_t[:], t_i2lo[:])
                i2hi_t = constp.tile([128, n2hi], dt.int16)
                nc.sync.dma_start(i2hi_t[:], t_i2hi[:])
            # all gather indices live in SBUF for the whole run

            # ---------------- phase X: build table1 ----------------
            with (
                tc.tile_pool(name="xload", bufs=3) as xlp,
                tc.tile_pool(name="xout", bufs=3) as xop,
                tc.tile_pool(name="xpsum", bufs=4, space="PSUM") as xpp,
            ):
                for t in range(NCHUNK):
                    xt_t = xlp.tile([IN, XCHUNK], dt.bfloat16, tag="xt")
                    nc.sync.dma_start(xt_t[:],
                                      t_xT[:, t * XCHUNK:(t + 1) * XCHUNK])
                    o_t = xop.tile([128, 4, 256], dt.bfloat16, tag="xo")
                    nc.vector.memset(o_t[:, :, 136:256], 0.0)
                    for k in range(4):
                        ps = xpp.tile([128, 136], dt.float32, tag="xp")
                        nc.tensor.matmul(ps[:], xt_t[:, k * 128:(k + 1) * 128],
                                         w1_t[:], start=True, stop=True)
                        if k % 2 == 0:
                            nc.vector.tensor_copy(o_t[:, k, 0:136], ps[:])
                        else:
                            nc.scalar.activation(o_t[:, k, 0:136], ps[:],
                                                 Act.Copy)
                    nc.sync.dma_start(
                        table1[t * XCHUNK:(t + 1) * XCHUNK, :].rearrange(
                            "(k p) e -> p k e", p=128), o_t[:])
            with tc.tile_pool(name="patchp", bufs=1) as pp:
                p1_t = pp.tile([2, 256], dt.bfloat16)
                nc.sync.dma_start(p1_t[:], t_patch1[:])
                nc.sync.dma_start(table1[LO_MAX:LO_MAX + 1, :], p1_t[0:1, :])
                nc.sync.dma_start(table1[N + 1:N + 2, :], p1_t[1:2, :])
                if PHASES >= 2:
                    p2_t = pp.tile([2, 64], dt.float32)
                    nc.sync.dma_start(p2_t[:], t_patch2[:])
                    nc.sync.dma_start(table2[0:1, :], p2_t[0:1, :])
                    nc.sync.dma_start(table2[T2_ROWS - 1:T2_ROWS, :],
                                      p2_t[1:2, :])

            if PHASES >= 2:
                # ---------------- phase L1: edges ----------------
                tab1_lo = table1[0:LO_MAX + 1, :]
                tab1_hi = table1[LO_MAX + 1:N + 2, :]
                Dmax1 = max(DLO1[g] + DHI1[g] for g in range(NG1))
                NBLO1 = max(sum(DLO1[g] for g in sb) for sb in SB1)
                NBHI1 = max(max(sum(DHI1[g] for g in sb) for sb in SB1), 1)
                olo = np.concatenate([[0], np.cumsum(DLO1)]).astype(int)
                ohi = np.concatenate([[0], np.cumsum(DHI1)]).astype(int)
                H1 = (NG1 // 2) * 128   # first AllGather covers groups < NG1//2
                t2main1 = table2[1 + G128:1 + G128 + NCORES * H1, :]
                t2main2 = table2[1 + G128 + NCORES * H1:1 + G128 +
                                 NCORES * G128, :]
                with (
                    tc.tile_pool(name="gath1", bufs=(3 if SB_BLOCK_BUDGET > 56 else 5)) as gathp,
                    tc.tile_pool(name="small1", bufs=3) as smallp,
                    tc.tile_pool(name="epi1", bufs=3) as epip,
                    tc.tile_pool(name="cp1", bufs=3) as cpp,
                    tc.tile_pool(name="agg1", bufs=2, space="PSUM") as aggp,
                    tc.tile_pool(name="psT1", bufs=2, space="PSUM") as psTp,
                    tc.tile_pool(name="ps21", bufs=2, space="PSUM") as ps2p,
                ):
                    for sb in SB1:
                        g0 = sb[0]
                        nlo = sum(DLO1[g] for g in sb)
                        nhi = sum(DHI1[g] for g in sb)
                        glo_t = gathp.tile([128, NBLO1, K_ELEM1],
                                           dt.bfloat16, tag="glo")
                        estep = 256 if K_ELEM1 != 256 else None
                        emit_gather(glo_t, tab1_lo, i1lo_t, olo[g0], nlo,
                                    K_ELEM1, estep)
                        ghi_t = gathp.tile([128, NBHI1, K_ELEM1], dt.bfloat16,
                                           tag="ghi")
                        if nhi > 0:
                            emit_gather(ghi_t, tab1_hi, i1hi_t, ohi[g0], nhi,
                                        K_ELEM1, estep)
                        lo_off = 0
                        hi_off = 0
                        for gi, g in enumerate(sb):
                            dlo, dhi = DLO1[g], DHI1[g]
                            D = dlo + dhi
                            if L1STEP < 2:
                                lo_off += dlo
                                hi_off += dhi
                                continue
                            logit_t = smallp.tile([128, Dmax1, 4], dt.float32,
                                                  tag="lg")
                            exb_t = smallp.tile([128, Dmax1, 4], dt.bfloat16,
                                                tag="exb")
                            exs_t = smallp.tile([128, Dmax1, 4], dt.bfloat16,
                                                tag="exs")
                            den_t = smallp.tile([128, 4], dt.float32, tag="dn")
                            rec_t = smallp.tile([128, 4], dt.float32, tag="rc")
                            ald_ap = glo_t[:, lo_off, 132:136]
                            nc.vector.scalar_tensor_tensor(
                                logit_t[:, :dlo, :],
                                glo_t[:, lo_off:lo_off + dlo, 128:132], 0.0,
                                ald_ap.unsqueeze(1).broadcast_to(
                                    (128, dlo, 4)), Alu.add, Alu.add)
                            if dhi > 0:
                                nc.vector.scalar_tensor_tensor(
                                    logit_t[:, dlo:D, :],
                                    ghi_t[:, hi_off:hi_off + dhi, 128:132], 0.0,
                                    ald_ap.unsqueeze(1).broadcast_to(
                                        (128, dhi, 4)), Alu.add, Alu.add)
                            nc.vector.scalar_tensor_tensor(
                                logit_t[:, :D, :], logit_t[:, :D, :], NEG,
                                logit_t[:, :D, :], Alu.mult, Alu.max)
                            nc.scalar.activation(exb_t[:, :D, :],
                                                 logit_t[:, :D, :], Act.Exp)
                            nc.vector.tensor_reduce(
                                den_t[:], exb_t[:, :D, :].transpose([0, 2, 1]),
                                axis=Axis.X, op=Alu.add)
                            nc.vector.reciprocal(rec_t[:], den_t[:])
                            nc.vector.tensor_tensor(
                                exs_t[:, :D, :], exb_t[:, :D, :],
                                rec_t[:].unsqueeze(1).broadcast_to(
                                    (128, D, 4)), Alu.mult)
                            if L1STEP < 3:
                                lo_off += dlo
                                hi_off += dhi
                                continue
                            h_lo = glo_t[:, lo_off:lo_off + dlo, 0:128]
                            h_lo = h_lo.rearrange("p b (h c) -> p b h c", h=4)
                            nc.vector.tensor_tensor(
                                h_lo, h_lo,
                                exs_t[:, :dlo, :].unsqueeze(3).broadcast_to(
                                    (128, dlo, 4, HID)), Alu.mult)
                            if dhi > 0:
                                h_hi = ghi_t[:, hi_off:hi_off + dhi, 0:128]
                                h_hi = h_hi.rearrange("p b (h c) -> p b h c",
                                                      h=4)
                                nc.vector.tensor_tensor(
                                    h_hi, h_hi,
                                    exs_t[:, dlo:D, :].unsqueeze(3).broadcast_to(
                                        (128, dhi, 4, HID)), Alu.mult)
                            if L1STEP < 4:
                                lo_off += dlo
                                hi_off += dhi
                                continue
                            segs = [(0, lo_off, dlo)]
                            if dhi > 0:
                                segs.append((1, hi_off, dhi))
                            chunks = _chunks_desc(segs, WPACK1)
                            lanes = chunks[0][2]
                            agg = aggp.tile([128, WPACK1, 128], dt.float32,
                                            tag="agg")
                            for ci, (ti, off, k) in enumerate(chunks):
                                src_t = glo_t if ti == 0 else ghi_t
                                nc.tensor.matmul(
                                    agg[:, 0:k, :],
                                    id_t[:], src_t[:, off:off + k, 0:128],
                                    start=(ci == 0),
                                    stop=(ci == len(chunks) - 1))
                            if L1STEP < 5:
                                lo_off += dlo
                                hi_off += dhi
                                continue
                            scaled_t = epip.tile([128, 128], dt.float32,
                                                 tag="sd")
                            if lanes == 1:
                                nc.vector.tensor_copy(scaled_t[:],
                                                      agg[:, 0, :])
                            else:
                                nc.vector.tensor_reduce(
                                    scaled_t[:],
                                    agg[:, 0:lanes, :].transpose([0, 2, 1]),
                                    axis=Axis.X, op=Alu.add)
                            if HASB1:
                                nc.vector.tensor_tensor(
                                    scaled_t[:], scaled_t[:], b1_t[:], Alu.add)
                            tmp_t = epip.tile([128, 128], dt.float32, tag="tm")
                            nc.scalar.activation(tmp_t[:], scaled_t[:], Act.Relu,
                                                 scale=-1.0)
                            nc.scalar.activation(tmp_t[:], tmp_t[:], Act.Exp,
                                                 scale=-1.0)
                            elu_t = epip.tile([128, 128], dt.bfloat16, tag="el")
                            nc.vector.scalar_tensor_tensor(
                                elu_t[:], tmp_t[:], -1.0, scaled_t[:],
                                Alu.add, Alu.max)
                            # inline pass-2: h1' @ W2ext -> table2 front rows
                            psT = psTp.tile([128, 128], dt.bfloat16, tag="pt")
                            nc.tensor.transpose(psT[:], elu_t[:], id_t[:])
                            eluT_t = epip.tile([128, 128], dt.bfloat16,
                                               tag="et")
                            nc.scalar.activation(eluT_t[:], psT[:], Act.Copy)
                            ps2 = ps2p.tile([128, 36], dt.float32, tag="p2")
                            nc.tensor.matmul(ps2[:], eluT_t[:], w2_t[:],
                                             start=True, stop=True)
                            cp_t = cpp.tile([128, 64], dt.float32, tag="cp")
                            if g % 2 == 0:
                                nc.scalar.activation(cp_t[:, 0:36], ps2[:],
                                                     Act.Copy)
                            else:
                                nc.vector.tensor_copy(cp_t[:, 0:36], ps2[:])
                            nc.sync.dma_start(
                                table2[1 + 128 * g:1 + 128 * (g + 1), :],
                                cp_t[:])
                            lo_off += dlo
                            hi_off += dhi
                            if PHASES >= 3 and L1STEP >= 5:
                                if 128 * (g + 1) == H1:
                                    nc.gpsimd.collective_compute(
                                        "AllGather", Alu.bypass,
                                        replica_groups=rg,
                                        ins=[table2[1:1 + H1, :]],
                                        outs=[t2main1[:, :]])
                    if PHASES >= 3 and L1STEP >= 5:
                        nc.gpsimd.collective_compute(
                            "AllGather", Alu.bypass, replica_groups=rg,
                            ins=[table2[1 + H1:1 + G128, :]],
                            outs=[t2main2[:, :]])

            if PHASES >= 4:
                # ---------------- phase L2: edges + pool ----------------
                B2 = 1 + G128 + NCORES * ((NG1 // 2) * 128)
                tab2_lo = table2[0:B2, :]
                tab2_hi = table2[B2:T2_ROWS, :]
                Dmax2 = max(DLO2[g] + DHI2[g] for g in range(NG2))
                NBLO2 = max(sum(DLO2[g] for g in sb) for sb in SB2)
                NBHI2 = max(max(sum(DHI2[g] for g in sb) for sb in SB2), 1)
                olo2 = np.concatenate([[0], np.cumsum(DLO2)]).astype(int)
                ohi2 = np.concatenate([[0], np.cumsum(DHI2)]).astype(int)
                with (
                    tc.tile_pool(name="gath2", bufs=(5 if SB_BLOCK_BUDGET > 56 else 8)) as gathp,
                    tc.tile_pool(name="small2", bufs=4) as smallp,
                    tc.tile_pool(name="epi2", bufs=3) as epip,
                    tc.tile_pool(name="agg2", bufs=2, space="PSUM") as aggp,
                    tc.tile_pool(name="poolps", bufs=1, space="PSUM") as poolpp,
                    tc.tile_pool(name="mp2", bufs=3) as mpp,
                ):
                    poolps = poolpp.tile([HID, GPOOL], dt.float32)
                    h2p_all = mpp.tile([128, NG2, HID], dt.bfloat16,
                                       tag="h2pall", bufs=1)
                    for sb in SB2:
                        g0 = sb[0]
                        nlo = sum(DLO2[g] for g in sb)
                        nhi = sum(DHI2[g] for g in sb)
                        glo_t = gathp.tile([128, NBLO2, 64], dt.float32,
                                           tag="glo")
                        emit_gather(glo_t, tab2_lo, i2lo_t, olo2[g0], nlo, 64)
                        ghi_t = gathp.tile([128, NBHI2, 64], dt.float32,
                                           tag="ghi")
                        if nhi > 0:
                            emit_gather(ghi_t, tab2_hi, i2hi_t, ohi2[g0], nhi,
                                        64)
                        lo_off = 0
                        hi_off = 0
                        for gi, g in enumerate(sb):
                            dlo, dhi = DLO2[g], DHI2[g]
                            D = dlo + dhi
                            logit_t = smallp.tile([128, Dmax2, 1], dt.float32,
                                                  tag="lg")
                            exf_t = smallp.tile([128, Dmax2, 1], dt.float32,
                                                tag="exf")
                            exs_t = smallp.tile([128, Dmax2, 1], dt.float32,
                                                tag="exs")
                            den_t = smallp.tile([128, 1], dt.float32, tag="dn")
                            rec_t = smallp.tile([128, 1], dt.float32, tag="rc")
                            ald_ap = glo_t[:, lo_off, 33:34]
                            nc.vector.scalar_tensor_tensor(
                                logit_t[:, :dlo, :],
                                glo_t[:, lo_off:lo_off + dlo, 32:33], 0.0,
                                ald_ap.unsqueeze(1).broadcast_to(
                                    (128, dlo, 1)), Alu.add, Alu.add)
                            if dhi > 0:
                                nc.vector.scalar_tensor_tensor(
                                    logit_t[:, dlo:D, :],
                                    ghi_t[:, hi_off:hi_off + dhi, 32:33], 0.0,
                                    ald_ap.unsqueeze(1).broadcast_to(
                                        (128, dhi, 1)), Alu.add, Alu.add)
                            nc.vector.scalar_tensor_tensor(
                                logit_t[:, :D, :], logit_t[:, :D, :], NEG,
                                logit_t[:, :D, :], Alu.mult, Alu.max)
                            nc.scalar.activation(exf_t[:, :D, :],
                                                 logit_t[:, :D, :], Act.Exp)
                            nc.vector.tensor_reduce(
                                den_t[:], exf_t[:, :D, :].transpose([0, 2, 1]),
                                axis=Axis.X, op=Alu.add)
                            nc.vector.reciprocal(rec_t[:], den_t[:])
                            nc.vector.tensor_tensor(
                                exs_t[:, :D, :], exf_t[:, :D, :],
                                rec_t[:].unsqueeze(1).broadcast_to(
                                    (128, D, 1)), Alu.mult)
                            exh_t = smallp.tile([128, Dmax2, HID], dt.bfloat16,
                                                tag="exh")
                            nc.vector.tensor_tensor(
                                exh_t[:, :dlo, :],
                                glo_t[:, lo_off:lo_off + dlo, 0:HID],
                                exs_t[:, :dlo, :].broadcast_to(
                                    (128, dlo, HID)), Alu.mult)
                            if dhi > 0:
                                nc.vector.tensor_tensor(
                                    exh_t[:, dlo:D, :],
                                    ghi_t[:, hi_off:hi_off + dhi, 0:HID],
                                    exs_t[:, dlo:D, :].broadcast_to(
                                        (128, dhi, HID)), Alu.mult)
                            chunks = _chunks_desc([(0, 0, D)], WPACK2)
                            lanes = chunks[0][2]
                            agg = aggp.tile([128, WPACK2, HID], dt.float32,
                                            tag="agg")
                            for ci, (ti, off, k) in enumerate(chunks):
                                nc.tensor.matmul(
                                    agg[:, 0:k, :],
                                    id_t[:], exh_t[:, off:off + k, :],
                                    start=(ci == 0),
                                    stop=(ci == len(chunks) - 1))
                            scaled_t = epip.tile([128, HID], dt.float32,
                                                 tag="sd")
                            if lanes == 1:
                                nc.vector.tensor_copy(scaled_t[:],
                                                      agg[:, 0, :])
                            else:
                                nc.vector.tensor_reduce(
                                    scaled_t[:],
                                    agg[:, 0:lanes, :].transpose([0, 2, 1]),
                                    axis=Axis.X, op=Alu.add)
                            if HASB2:
                                nc.vector.tensor_tensor(
                                    scaled_t[:], scaled_t[:], b2_t[:], Alu.add)
                            tmp_t = epip.tile([128, HID], dt.float32, tag="tm")
                            nc.scalar.activation(tmp_t[:], scaled_t[:], Act.Relu,
                                                 scale=-1.0)
                            nc.scalar.activation(tmp_t[:], tmp_t[:], Act.Exp,
                                                 scale=-1.0)
                            nc.vector.scalar_tensor_tensor(
                                h2p_all[:, g, :], tmp_t[:], -1.0, scaled_t[:],
                                Alu.add, Alu.max)
                            lo_off += dlo
                            hi_off += dhi

                    for g in range(NG2):
                        mp_t = mpp.tile([128, GPOOL], dt.bfloat16, tag="mp")
                        nc.sync.dma_start(mp_t[:],
                                          t_mpool[g * 128:(g + 1) * 128, :])
                        nc.tensor.matmul(poolps[:], h2p_all[:, g, :], mp_t[:],
                                         start=(g == 0), stop=(g == NG2 - 1))
                    # ------------- pool + final linear -------------
                    with tc.tile_pool(name="fin", bufs=1) as finp, \
                            tc.tile_pool(name="finps", bufs=1, space="PSUM") as fpp:
                        poolsb = finp.tile([HID, GPOOL], dt.float32)
                        nc.vector.tensor_copy(poolsb[:], poolps[:])
                        nc.sync.dma_start(cc_in[:, :], poolsb[:])
                        nc.gpsimd.collective_compute(
                            "AllGather", Alu.bypass, replica_groups=rg,
                            ins=[cc_in[:, :]], outs=[cc_out[:, :]])
                        parts_t = finp.tile([HID, NCORES, GPOOL], dt.float32)
                        nc.sync.dma_start(
                            parts_t[:],
                            cc_out[:, :].rearrange("(c p) e -> p c e",
                                                   p=HID))
                        psum_t = finp.tile([HID, GPOOL], dt.float32)
                        nc.vector.tensor_reduce(
                            psum_t[:], parts_t[:].transpose([0, 2, 1]),
                            axis=Axis.X, op=Alu.add)
                        mean_t = finp.tile([HID, GPOOL], dt.float32)
                        nc.vector.tensor_tensor(
                            mean_t[:], psum_t[:],
                            rc_t[:], Alu.mult)
                        psO = fpp.tile([GPOOL, OUT], dt.float32)
                        nc.tensor.matmul(psO[:], mean_t[:], wl_t[:], start=True,
                                         stop=True)
                        out_t = finp.tile([GPOOL, OUT], dt.float32)
                        if HASBL:
                            nc.vector.tensor_tensor(out_t[:], psO[:], bl_t[:],
                                                    Alu.add)
                        else:
                            nc.vector.tensor_copy(out_t[:], psO[:])
                        nc.sync.dma_start(t_out[:, :], out_t[:])
            if PHASES < 4:
                _emit_dummy_out(nc, tc, t_out, dt)


    nc.compile()
    return nc


def core_inputs(prep, c):
    cd = prep["cores"][c]

    def padcols(a, cols):
        if a.shape[1] == cols:
            return a
        out = np.zeros((a.shape[0], cols), a.dtype)
        out[:, :a.shape[1]] = a
        return out

    n1lo = max(8 * sum(prep["DLO1"]), 8)
    n1hi = max(8 * sum(prep["DHI1"]), 8)
    n2lo = max(8 * sum(prep["DLO2"]), 8)
    n2hi = max(8 * sum(prep["DHI2"]), 8)
    return dict(
        xT=np.ascontiguousarray(cd["xT"]),
        w1ext=prep["W1ext"], w2ext=prep["W2ext"], wl=prep["Wl"],
        b1=prep["b1"], b2=prep["b2"], bl=prep["bl"], rcnt=prep["rcnt"],
        patch1=prep["patch1"], patch2=prep["patch2"], ident=prep["ident"],
        mpool=np.ascontiguousarray(cd["mpool"]),
        **({"idx32": cd["w_idx32"]} if K_IND else {}),
        idx1lo=padcols(cd["w_idx1lo"], n1lo),
        idx1hi=padcols(cd["w_idx1hi"], n1hi),
        idx2lo=padcols(cd["w_idx2lo"], n2lo),
        idx2hi=padcols(cd["w_idx2hi"], n2hi),
    )


_CACHE = {}


def kernel(**inputs):
    from concourse.bass_utils import run_bass_kernel_spmd

    inputs = {k: np.asarray(v) for k, v in inputs.items()}
    prep = host_prep(**inputs)
    sc = make_sched(prep)
    key = str(sc)
    if key not in _CACHE:
        _CACHE[key] = build_bass(sc)
    nc = _CACHE[key]
    in_maps = [core_inputs(prep, c) for c in range(NCORES)]
    res = run_bass_kernel_spmd(nc, in_maps, list(range(NCORES)))
    return np.asarray(res.results[0]["out"], np.float32)


# revision 27
# speedup vs baseline: 1.1469x; 1.0302x over previous
# Self-contained 8-core Trainium2 Bass kernel for the 2-layer GAT + mean-pool
# problem (nn_GAT_83820581749190).
#
# Sharding: destination nodes (and all their incident edges) are partitioned
# across the 8 cores, so each layer's attention softmax and aggregation
# complete locally per core. Each core builds a replicated layer-1 feature
# table [h1 | al_src | al_dst] (bf16, 512-byte rows) in HBM with a replicated
# x @ W1ext matmul, edge-gathers rows with the GPSIMD dma_gather custom op
# (int16 indices force a lo/hi table-half split), computes the edge softmax
# without segment-max (logits are small enough that exp cannot overflow), and
# aggregates per-destination with wide identity-matmul PSUM accumulation
# (destinations sit on partitions via degree-bucketed groups of 128; up to 4
# degree-lanes per matmul, reduced on Vector). Each destination's self-loop
# edge is forced to be the first lo-half edge, so the gathered block 0 carries
# the per-destination a_dst logits for free. Layer-1 group outputs are pushed
# through W2ext inline and written to a layer-2 feature table in L1-slot
# order; two split AllGathers (overlapped with the L1 tail) replicate it.
# Layer 2 repeats the gather/softmax/aggregate pattern (16 lanes per matmul);
# mean-pool is a matmul against a host-built one-hot graph matrix plus a tiny
# AllReduce.
import os
import numpy as np
import ml_dtypes

N = 50000
E = 800000
IN = 128
HID = 32
HEADS = 4
OUT = 10
GPOOL = 64
NEG = 0.2
NCORES = 8
S = N // NCORES
LO_MAX = 32767          # max usable int16 gather index
SPECIAL_ALS = -100.0    # al_src of pad rows: exp(0.2*(-100+ald)) ~ 2e-9
SB_BLOCK_BUDGET = int(os.environ.get("K_BUD", "48"))  # max gather blocks per superblock
XCHUNK = 512
WPACK1 = 4              # layer-1 psum lanes per wide matmul
WPACK2 = 16             # layer-2 psum lanes per wide matmul
PHASES = int(os.environ.get("K_PHASES", "99"))  # 1=X only, 2=+L1, 3=+exchange, 4=+L2
L1STEP = int(os.environ.get("K_L1STEP", "99"))  # 1=gathers 2=+softmax 3=+exh 4=+agg 5=+epi
K_ELEM1 = int(os.environ.get("K_ELEM1", "256"))  # L1 gather elem (floor experiments)
K_SP = int(os.environ.get("K_SP", "0"))          # single_packet on gathers
K_IND = int(os.environ.get("K_IND", "0"))        # use indirect_dma_start (floor test)

bf16 = ml_dtypes.bfloat16


def _ceil_to(v, m):
    return (v + m - 1) // m * m


# ======================= host prep =========================================

def _boundary_aware_order(deg_lo, deg_hi):
    """Sort ids by (lo desc, hi desc), but fill 128-groups that straddle a
    lo-run boundary from the *small-hi tail* of the next run, keeping
    per-group max_lo + max_hi tight."""
    Sn = len(deg_lo)
    base = np.lexsort((-deg_hi, -deg_lo))
    glo = deg_lo[base]
    runs = []
    i = 0
    while i < Sn:
        j = i
        while j < Sn and glo[j] == glo[i]:
            j += 1
        runs.append(list(base[i:j]))
        i = j
    order = []
    ri = 0
    fronts = [0] * len(runs)
    backs = [len(r) for r in runs]
    while len(order) < Sn:
        while ri < len(runs) and fronts[ri] >= backs[ri]:
            ri += 1
        if ri >= len(runs):
            break
        need = 128 - (len(order) % 128)
        avail = backs[ri] - fronts[ri]
        if avail >= need:
            order.extend(runs[ri][fronts[ri]:fronts[ri] + need])
            fronts[ri] += need
        else:
            order.extend(runs[ri][fronts[ri]:backs[ri]])
            fronts[ri] = backs[ri]
            need -= avail
            rj = ri + 1
            while need > 0 and rj < len(runs):
                a = backs[rj] - fronts[rj]
                t = min(a, need)
                order.extend(reversed(runs[rj][backs[rj] - t:backs[rj]]))
                backs[rj] -= t
                need -= t
                rj += 1
    P = np.asarray(order, np.int64)
    Ppos = np.empty(Sn, np.int64)
    Ppos[P] = np.arange(Sn)
    return P, Ppos


def _run_groups(glo, ghi, max_rows=128):
    Sn = len(glo)
    ng = (Sn + max_rows - 1) // max_rows
    dlo = np.zeros(ng, np.int64)
    dhi = np.zeros(ng, np.int64)
    for g in range(ng):
        s, e = g * max_rows, min((g + 1) * max_rows, Sn)
        dlo[g] = glo[s:e].max()
        dhi[g] = ghi[s:e].max()
    return dlo, dhi


def _build_layer(src, dstl, is_lo):
    deg_lo = np.bincount(dstl[is_lo], minlength=S)
    deg_hi = np.bincount(dstl[~is_lo], minlength=S)
    P, Ppos = _boundary_aware_order(deg_lo, deg_hi)
    dlo, dhi = _run_groups(deg_lo[P], deg_hi[P])
    return dict(src=src, dstl=dstl, is_lo=is_lo, deg_lo=deg_lo, deg_hi=deg_hi,
                P=P, Ppos=Ppos, dlo=dlo, dhi=dhi)


def _emit_slots(l, DLO, DHI, idx_lo_of, idx_hi_of, special_lo, special_hi):
    NG = len(DLO)
    src, is_lo = l["src"], l["is_lo"]
    Ppos = l["Ppos"]
    nreal = len(l["P"])
    slot2cmp = np.full(NG * 128, -1, np.int64)
    slot2cmp[:nreal] = np.arange(nreal)
    idx_lo = [np.full((int(DLO[g]), 128), special_lo, np.int64)
              for g in range(NG)]
    idx_hi = [np.full((int(DHI[g]), 128), special_hi, np.int64)
              for g in range(NG)]
    slot_of_edge = Ppos[l["dstl"]]
    order = np.argsort(slot_of_edge, kind="stable")
    for mask, arrs, idx_fn in ((is_lo, idx_lo, idx_lo_of),
                               (~is_lo, idx_hi, idx_hi_of)):
        m = mask[order]
        so = slot_of_edge[order][m]
        sr = src[order][m]
        jj = np.arange(len(so)) - np.searchsorted(so, so, side="left")
        gg, kk = so // 128, so % 128
        vals = idx_fn(sr)
        for g in range(NG):
            sel = gg == g
            if sel.any():
                arrs[g][jj[sel], kk[sel]] = vals[sel]
    return idx_lo, idx_hi, slot2cmp


def _wrap16(idx):
    """[n] -> [128, n//16] int16: idx i at [i%16, i//16], replicated x8."""
    n = len(idx)
    assert n % 16 == 0
    w = np.ascontiguousarray(np.asarray(idx).reshape(n // 16, 16).T)
    w = w.astype(np.int16)
    return np.tile(w, (8, 1))


def _wrap_groups(arrs):
    segs = [_wrap16(a.reshape(-1)) if a.size else np.zeros((128, 0), np.int16)
            for a in arrs]
    return np.concatenate(segs, axis=1) if segs else np.zeros((128, 0), np.int16)


def host_prep(x, edge_index, batch, W1, a1_src, a1_dst, b1, W2, a2_src, a2_dst,
              b2, Wl, bl):
    x = np.asarray(x, np.float32)
    edge_index = np.asarray(edge_index, np.int64)
    batch = np.asarray(batch, np.int64)
    # self-loops FIRST: each dst's self edge is its first (lo) edge, so the
    # gathered lo block 0 carries the per-dst a_dst logit columns.
    src_all = np.concatenate([np.arange(N, dtype=np.int64), edge_index[0]])
    dst_all = np.concatenate([np.arange(N, dtype=np.int64), edge_index[1]])
    owner = dst_all // S

    a1_src = np.asarray(a1_src, np.float32)
    a1_dst = np.asarray(a1_dst, np.float32)
    W1 = np.asarray(W1, np.float32)
    W2 = np.asarray(W2, np.float32)
    As1 = np.zeros((HEADS * HID, HEADS), np.float32)
    Ad1 = np.zeros((HEADS * HID, HEADS), np.float32)
    for h in range(HEADS):
        As1[h * HID:(h + 1) * HID, h] = a1_src[h]
        Ad1[h * HID:(h + 1) * HID, h] = a1_dst[h]
    W1ext = np.concatenate([W1, W1 @ As1, W1 @ Ad1], axis=1)   # [128,136]
    W2ext = np.concatenate(
        [W2, W2 @ np.asarray(a2_src, np.float32)[0][:, None],
         W2 @ np.asarray(a2_dst, np.float32)[0][:, None],
         np.zeros((HEADS * HID, 2), np.float32)], axis=1)  # [128,36]

    cores = [dict(c=c) for c in range(NCORES)]
    for cd in cores:
        c = cd["c"]
        m = owner == c
        cd["src"] = src_all[m]
        cd["dstl"] = dst_all[m] - c * S

    # ---------- layer 1 ----------
    for cd in cores:
        c = cd["c"]
        pos_of = np.empty(N, np.int64)
        own = np.arange(c * S, (c + 1) * S)
        oth = np.concatenate([np.arange(0, c * S), np.arange((c + 1) * S, N)])
        pos_of[oth] = S + np.arange(N - S)
        pos_of[own] = 0
        is_lo1 = pos_of[cd["src"]] < LO_MAX
        l1 = _build_layer(cd["src"], cd["dstl"], is_lo1)
        pos_of[own] = l1["Ppos"]
        row_of = np.where(pos_of < LO_MAX, pos_of, pos_of + 1)
        cd["l1"] = l1
        cd["row_of"] = row_of
    NG1 = max(len(cd["l1"]["dlo"]) for cd in cores)
    DLO1 = np.zeros(NG1, np.int64)
    DHI1 = np.zeros(NG1, np.int64)
    for cd in cores:
        d = cd["l1"]
        DLO1[:len(d["dlo"])] = np.maximum(DLO1[:len(d["dlo"])], d["dlo"])
        DHI1[:len(d["dhi"])] = np.maximum(DHI1[:len(d["dhi"])], d["dhi"])
    assert (DLO1 >= 1).all()   # self edges are always lo
    for cd in cores:
        r = cd["row_of"]
        cd["idx1_lo"], cd["idx1_hi"], cd["slot2cmp1"] = _emit_slots(
            cd["l1"], DLO1, DHI1,
            lambda s, r=r: r[s], lambda s, r=r: r[s] - (LO_MAX + 1),
            LO_MAX, N + 1 - (LO_MAX + 1))
        # verify self-edge-first: lo block 0 of each group holds the dst's own
        # row for every real slot
        c = cd["c"]
        P1 = cd["l1"]["P"]
        for g in range(NG1):
            e = min(128, S - g * 128)
            if e <= 0:
                break
            dsts = P1[g * 128:g * 128 + e]
            assert (cd["idx1_lo"][g][0, :e] == r[dsts + c * S]).all()

    G128 = NG1 * 128
    H1 = (NG1 // 2) * 128   # first split-AllGather covers slots < H1
    # ---------- layer 2 ----------
    # table2 layout: [0]=patch-lo | [1 : 1+G128) own slots (AllGather input) |
    # [1+G128 : +8*H1) all cores' slots < H1 | [.. : +8*(G128-H1)) all cores'
    # slots >= H1 | [last]=patch-hi. Two contiguous AllGather segments.
    T2_ROWS = 1 + G128 + NCORES * G128 + 1
    B2 = 1 + G128 + NCORES * H1      # lo/hi boundary = start of 2nd AllGather
    assert B2 <= LO_MAX + 1
    row2_main = np.empty(N, np.int64)
    for cd in cores:
        c = cd["c"]
        pp1 = cd["l1"]["Ppos"]
        r = np.where(pp1 < H1,
                     1 + G128 + c * H1 + pp1,
                     1 + G128 + NCORES * H1 + c * (G128 - H1) + (pp1 - H1))
        row2_main[c * S:(c + 1) * S] = r
    for cd in cores:
        c = cd["c"]
        src = cd["src"]
        own_m = (src // S) == c
        row2 = row2_main[src].copy()
        row2[own_m] = 1 + cd["l1"]["Ppos"][src[own_m] - c * S]
        cd["row2"] = row2
        is_lo2 = row2 < B2
        cd["l2"] = _build_layer(src, cd["dstl"], is_lo2)
    NG2 = max(len(cd["l2"]["dlo"]) for cd in cores)
    DLO2 = np.zeros(NG2, np.int64)
    DHI2 = np.zeros(NG2, np.int64)
    for cd in cores:
        d = cd["l2"]
        DLO2[:len(d["dlo"])] = np.maximum(DLO2[:len(d["dlo"])], d["dlo"])
        DHI2[:len(d["dhi"])] = np.maximum(DHI2[:len(d["dhi"])], d["dhi"])
    assert (DLO2 >= 1).all()   # self edges (own rows <= G128) are always lo
    assert T2_ROWS - 1 - B2 <= 32767
    for cd in cores:
        l2 = cd["l2"]
        row2 = cd["row2"]
        pos_in_edges = {}
        # idx fns index by src node id; build per-edge instead: _emit_slots
        # passes src ids, but row2 is per-edge. Map via first occurrence is
        # wrong if a src repeats with different rows -- it cannot: row2 is a
        # function of src id only. Build a per-node map lazily.
        row2_of_node = np.empty(N, np.int64)
        row2_of_node[cd["src"]] = row2
        cd["idx2_lo"], cd["idx2_hi"], cd["slot2cmp2"] = _emit_slots(
            l2, DLO2, DHI2,
            lambda s, r=row2_of_node: r[s],
            lambda s, r=row2_of_node: r[s] - B2,
            0, T2_ROWS - 1 - B2)
        # verify self-edge-first in lo block 0
        c = cd["c"]
        P2 = l2["P"]
        own_row = 1 + cd["l1"]["Ppos"]
        for g in range(NG2):
            e = min(128, S - g * 128)
            if e <= 0:
                break
            dsts = P2[g * 128:g * 128 + e]
            assert (cd["idx2_lo"][g][0, :e] == own_row[dsts]).all()

    # ---------- aux ----------
    cnt = np.bincount(batch, minlength=GPOOL).astype(np.float32)
    recip_cnt = (1.0 / np.maximum(cnt, 1.0)).astype(np.float32)

    for cd in cores:
        c = cd["c"]
        gids = batch[c * S:(c + 1) * S]
        Mp = np.zeros((NG2 * 128, GPOOL), np.float32)
        s2c = cd["slot2cmp2"]
        real = s2c >= 0
        Mp[np.where(real)[0], gids[cd["l2"]["P"][s2c[real]]]] = 1.0
        cd["mpool"] = Mp.astype(bf16)

        xt = np.zeros((IN, _ceil_to(N + 2, XCHUNK)), np.float32)
        xt[:, cd["row_of"]] = x.T
        cd["xT"] = xt.astype(bf16)

        segs = []
        for g in range(NG1):
            a = np.concatenate(
                [cd["idx1_lo"][g],
                 cd["idx1_hi"][g] + (LO_MAX + 1)], axis=0)  # [d, 128] abs rows
            segs.append(np.ascontiguousarray(a.T))          # [128, d]
        cd["w_idx32"] = np.concatenate(segs, axis=1).astype(np.int32)
        SB1 = _pack_superblocks(DLO1, DHI1)
        SB2 = _pack_superblocks(DLO2, DHI2)

        def lay(arrs, DL, SBs, offs, total):
            out = np.zeros((128, 8 * total), np.int16)
            for sbi, sb in enumerate(SBs):
                seg = _wrap_groups([arrs[g] for g in sb])
                out[:, 8 * offs[sbi]:8 * offs[sbi] + seg.shape[1]] = seg
            return out

        lo1, hi1, tl1, th1 = _sb_idx_layout(DLO1, DHI1, SB1)
        lo2, hi2, tl2, th2 = _sb_idx_layout(DLO2, DHI2, SB2)
        cd["w_idx1lo"] = lay(cd["idx1_lo"], DLO1, SB1, lo1, tl1)
        cd["w_idx1hi"] = lay(cd["idx1_hi"], DHI1, SB1, hi1, th1)
        cd["w_idx2lo"] = lay(cd["idx2_lo"], DLO2, SB2, lo2, tl2)
        cd["w_idx2hi"] = lay(cd["idx2_hi"], DHI2, SB2, hi2, th2)

    patch1 = np.zeros((2, 256), np.float32)
    patch1[:, 128:132] = SPECIAL_ALS
    patch2 = np.zeros((2, 64), np.float32)
    patch2[:, 32] = SPECIAL_ALS

    return dict(cores=cores,
                DLO1=[int(v) for v in DLO1], DHI1=[int(v) for v in DHI1],
                DLO2=[int(v) for v in DLO2], DHI2=[int(v) for v in DHI2],
                W1ext=W1ext.astype(bf16), W2ext=W2ext.astype(bf16),
                Wl=np.asarray(Wl, np.float32),
                b1=np.tile(np.asarray(b1, np.float32).reshape(1, -1),
                           (128, 1)),
                b2=np.tile(np.asarray(b2, np.float32).reshape(1, -1),
                           (128, 1)),
                bl=np.tile(np.asarray(bl, np.float32).reshape(1, -1),
                           (GPOOL, 1)),
                rcnt=np.tile(recip_cnt.reshape(1, -1), (HID, 1)),
                patch1=patch1.astype(bf16), patch2=patch2,
                ident=np.eye(128, dtype=bf16))




def _sb_idx_layout(DLO, DHI, SBs):
    """Per-superblock 64-col-aligned offsets for lo/hi idx segments.
    Returns (lo_offs, hi_offs, lo_total, hi_total) in 8-col block units."""
    lo_offs, hi_offs = [], []
    lo_cur = hi_cur = 0
    for sb in SBs:
        nlo = sum(int(DLO[g]) for g in sb)
        nhi = sum(int(DHI[g]) for g in sb)
        lo_offs.append(lo_cur)
        hi_offs.append(hi_cur)
        lo_cur += (nlo + 7) // 8 * 8
        hi_cur += (nhi + 7) // 8 * 8
    return lo_offs, hi_offs, max(lo_cur, 1), max(hi_cur, 1)


def _pack_superblocks(DLO, DHI, budget=SB_BLOCK_BUDGET):
    sbs, cur, tot = [], [], 0
    for g in range(len(DLO)):
        d = int(DLO[g] + DHI[g])
        if cur and tot + d > budget:
            sbs.append(cur)
            cur, tot = [], 0
        cur.append(g)
        tot += d
    if cur:
        sbs.append(cur)
    return sbs


def make_sched(prep):
    DLO1, DHI1 = prep["DLO1"], prep["DHI1"]
    DLO2, DHI2 = prep["DLO2"], prep["DHI2"]
    assert all(a >= 1 for a in DLO1)
    assert all(a >= 1 for a in DLO2)
    return dict(DLO1=DLO1, DHI1=DHI1, DLO2=DLO2, DHI2=DHI2,
                SB1=_pack_superblocks(DLO1, DHI1),
                SB2=_pack_superblocks(DLO2, DHI2),
                HASB1=bool(np.any(prep["b1"])), HASB2=bool(np.any(prep["b2"])),
                HASBL=bool(np.any(prep["bl"])))


# ======================= bass kernel =======================================

def _emit_dummy_out(nc, tc, t_out, dt):
    with tc.tile_pool(name='dummy', bufs=1) as dp:
        d = dp.tile([GPOOL, OUT], dt.float32)
        nc.vector.memset(d[:], 0.0)
        nc.sync.dma_start(t_out[:, :], d[:])


def _chunks_desc(segs, wpack):
    """segs: list of (tile_idx, off, count). Returns wide-matmul chunks
    (tile_idx, off, k<=wpack) sorted largest-first."""
    out = []
    for ti, off, cnt in segs:
        rem = cnt
        o = off
        while rem > 0:
            k = min(wpack, rem)
            out.append((ti, o, k))
            o += k
            rem -= k
    out.sort(key=lambda t: -t[2])
    return out


def build_bass(sc):
    import concourse.bacc as bacc
    import concourse.tile as tile
    import concourse.mybir as mybir
    from concourse.library_config import mlp

    dt = mybir.dt
    Alu = mybir.AluOpType
    Act = mybir.ActivationFunctionType
    Axis = mybir.AxisListType

    DLO1, DHI1 = sc["DLO1"], sc["DHI1"]
    DLO2, DHI2 = sc["DLO2"], sc["DHI2"]
    SB1, SB2 = sc["SB1"], sc["SB2"]
    HASB1 = sc.get("HASB1", True)
    HASB2 = sc.get("HASB2", True)
    HASBL = sc.get("HASBL", True)
    NG1, NG2 = len(DLO1), len(DLO2)
    XT_COLS = _ceil_to(N + 2, XCHUNK)
    NCHUNK = XT_COLS // XCHUNK
    G128 = NG1 * 128
    T2_ROWS = 1 + G128 + NCORES * G128 + 1

    nc = bacc.Bacc("TRN2", target_bir_lowering=False, debug=False,
                   num_devices=NCORES, num_swdge_queues=4)

    t_xT = nc.dram_tensor("xT", [IN, XT_COLS], dt.bfloat16, kind="ExternalInput")
    t_w1 = nc.dram_tensor("w1ext", [IN, 136], dt.bfloat16, kind="ExternalInput")
    t_w2 = nc.dram_tensor("w2ext", [IN, 36], dt.bfloat16, kind="ExternalInput")
    t_wl = nc.dram_tensor("wl", [HID, OUT], dt.float32, kind="ExternalInput")
    t_b1 = nc.dram_tensor("b1", [128, HEADS * HID], dt.float32,
                          kind="ExternalInput")
    t_b2 = nc.dram_tensor("b2", [128, HID], dt.float32, kind="ExternalInput")
    t_bl = nc.dram_tensor("bl", [GPOOL, OUT], dt.float32, kind="ExternalInput")
    t_rcnt = nc.dram_tensor("rcnt", [HID, GPOOL], dt.float32,
                            kind="ExternalInput")
    t_patch1 = nc.dram_tensor("patch1", [2, 256], dt.bfloat16,
                              kind="ExternalInput")
    t_patch2 = nc.dram_tensor("patch2", [2, 64], dt.float32,
                              kind="ExternalInput")
    t_ident = nc.dram_tensor("ident", [128, 128], dt.bfloat16,
                             kind="ExternalInput")
    t_mpool = nc.dram_tensor("mpool", [NG2 * 128, GPOOL], dt.bfloat16,
                             kind="ExternalInput")
    LO1OFF, HI1OFF, tl1, th1 = _sb_idx_layout(DLO1, DHI1, SB1)
    LO2OFF, HI2OFF, tl2, th2 = _sb_idx_layout(DLO2, DHI2, SB2)
    n1lo, n1hi, n2lo, n2hi = 8 * tl1, 8 * th1, 8 * tl2, 8 * th2
    n1all = sum(DLO1) + sum(DHI1)
    if K_IND:
        t_i32 = nc.dram_tensor("idx32", [128, n1all], dt.int32,
                               kind="ExternalInput")
    t_i1lo = nc.dram_tensor("idx1lo", [128, n1lo], dt.int16, kind="ExternalInput")
    t_i1hi = nc.dram_tensor("idx1hi", [128, n1hi], dt.int16, kind="ExternalInput")
    t_i2lo = nc.dram_tensor("idx2lo", [128, n2lo], dt.int16, kind="ExternalInput")
    t_i2hi = nc.dram_tensor("idx2hi", [128, n2hi], dt.int16, kind="ExternalInput")
    t_out = nc.dram_tensor("out", [GPOOL, OUT], dt.float32,
                           kind="ExternalOutput")

    rg = [list(range(NCORES))]
    _qc = [0]

    def nextq(ndesc=1):
        _qc[0] = (_qc[0] + 1) % 4
        return _qc[0]

    with tile.TileContext(nc) as tc:
        with (
            tc.tile_pool(name="const", bufs=1) as constp,
            tc.tile_pool(name="dram", bufs=1, space="DRAM") as dramp,
        ):
            nc.gpsimd.load_library(mlp)
            def emit_gather(out_t, tab, idx_t, o0, nb, elem, estep=None,
                            idxp=None, nbmax=None):
                nc.gpsimd.dma_gather(
                    out_t[:, :nb, :], tab,
                    idx_t[:, 8 * o0:8 * (o0 + nb)],
                    128 * nb, 128 * nb, elem, elem_step=estep,
                    single_packet=bool(K_SP), queue_num=nextq(nb))

            table1 = dramp.tile([XT_COLS, 256], dt.bfloat16, tag="table1")
            table2 = dramp.tile([_ceil_to(T2_ROWS, 4), 64], dt.float32,
                                tag="table2")
            cc_in = dramp.tile([HID, GPOOL], dt.float32, tag="ccin")
            cc_out = dramp.tile([NCORES * HID, GPOOL], dt.float32, tag="ccout")

            w1_t = constp.tile([IN, 136], dt.bfloat16)
            nc.sync.dma_start(w1_t[:], t_w1[:])
            w2_t = constp.tile([IN, 36], dt.bfloat16)
            nc.sync.dma_start(w2_t[:], t_w2[:])
            wl_t = constp.tile([HID, OUT], dt.float32)
            nc.sync.dma_start(wl_t[:], t_wl[:])
            b1_t = constp.tile([128, HEADS * HID], dt.float32)
            nc.sync.dma_start(b1_t[:], t_b1[:])
            b2_t = constp.tile([128, HID], dt.float32)
            nc.sync.dma_start(b2_t[:], t_b2[:])
            bl_t = constp.tile([GPOOL, OUT], dt.float32)
            nc.sync.dma_start(bl_t[:], t_bl[:])
            rc_t = constp.tile([HID, GPOOL], dt.float32)
            nc.sync.dma_start(rc_t[:], t_rcnt[:])
            id_t = constp.tile([128, 128], dt.bfloat16)
            nc.sync.dma_start(id_t[:], t_ident[:])
            if PHASES >= 2:
                i1lo_t = constp.tile([128, n1lo], dt.int16)
                nc.sync.dma_start(i1lo_t[:], t_i1lo[:])
                i1hi_t = constp.tile([128, n1hi], dt.int16)
                nc.sync.dma_start(i1hi_t[:], t_i1hi[:])
            if PHASES >= 4:
                i2lo_t = constp.tile([128, n2lo], dt.int16)
                nc.sync.dma_start(i2lo_t[:], t_i2lo[:])
                i2hi_t = constp.tile([128, n2hi], dt.int16)
                nc.sync.dma_start(i2hi_t[:], t_i2hi[:])
            # all gather indices live in SBUF for the whole run

            # ---------------- phase X: build table1 ----------------
            with (
                tc.tile_pool(name="xload", bufs=3) as xlp,
                tc.tile_pool(name="xout", bufs=3) as xop,
                tc.tile_pool(name="xpsum", bufs=4, space="PSUM") as xpp,
            ):
                for t in range(NCHUNK):
                    xt_t = xlp.tile([IN, XCHUNK], dt.bfloat16, tag="xt")
                    nc.sync.dma_start(xt_t[:],
                                      t_xT[:, t * XCHUNK:(t + 1) * XCHUNK])
                    o_t = xop.tile([128, 4, 256], dt.bfloat16, tag="xo")
                    nc.vector.memset(o_t[:, :, 136:256], 0.0)
                    for k in range(4):
                        ps = xpp.tile([128, 136], dt.float32, tag="xp")
                        nc.tensor.matmul(ps[:], xt_t[:, k * 128:(k + 1) * 128],
                                         w1_t[:], start=True, stop=True)
                        if k % 2 == 0:
                            nc.vector.tensor_copy(o_t[:, k, 0:136], ps[:])
                        else:
                            nc.scalar.activation(o_t[:, k, 0:136], ps[:],
                                                 Act.Copy)
                    nc.sync.dma_start(
                        table1[t * XCHUNK:(t + 1) * XCHUNK, :].rearrange(
                            "(k p) e -> p k e", p=128), o_t[:])
            with tc.tile_pool(name="patchp", bufs=1) as pp:
                p1_t = pp.tile([2, 256], dt.bfloat16)
                nc.sync.dma_start(p1_t[:], t_patch1[:])
                nc.sync.dma_start(table1[LO_MAX:LO_MAX + 1, :], p1_t[0:1, :])
                nc.sync.dma_start(table1[N + 1:N + 2, :], p1_t[1:2, :])
                if PHASES >= 2:
                    p2_t = pp.tile([2, 64], dt.float32)
                    nc.sync.dma_start(p2_t[:], t_patch2[:])
                    nc.sync.dma_start(table2[0:1, :], p2_t[0:1, :])
                    nc.sync.dma_start(table2[T2_ROWS - 1:T2_ROWS, :],
                                      p2_t[1:2, :])

            if PHASES >= 2:
                # ---------------- phase L1: edges ----------------
                tab1_lo = table1[0:LO_MAX + 1, :]
                tab1_hi = table1[LO_MAX + 1:N + 2, :]
                Dmax1 = max(DLO1[g] + DHI1[g] for g in range(NG1))
                NBLO1 = max(sum(DLO1[g] for g in sb) for sb in SB1)
                NBHI1 = max(max(sum(DHI1[g] for g in sb) for sb in SB1), 1)
                olo = np.concatenate([[0], np.cumsum(DLO1)]).astype(int)
                ohi = np.concatenate([[0], np.cumsum(DHI1)]).astype(int)
                H1 = (NG1 // 2) * 128   # first AllGather covers groups < NG1//2
                t2main1 = table2[1 + G128:1 + G128 + NCORES * H1, :]
                t2main2 = table2[1 + G128 + NCORES * H1:1 + G128 +
                                 NCORES * G128, :]
                with (
                    tc.tile_pool(name="gath1", bufs=(3 if SB_BLOCK_BUDGET > 56 else 5)) as gathp,
                    tc.tile_pool(name="small1", bufs=3) as smallp,
                    tc.tile_pool(name="epi1", bufs=3) as epip,
                    tc.tile_pool(name="cp1", bufs=3) as cpp,
                    tc.tile_pool(name="agg1", bufs=2, space="PSUM") as aggp,
                    tc.tile_pool(name="psT1", bufs=2, space="PSUM") as psTp,
                    tc.tile_pool(name="ps21", bufs=2, space="PSUM") as ps2p,
                ):
                    for sbi, sb in enumerate(SB1):
                        g0 = sb[0]
                        nlo = sum(DLO1[g] for g in sb)
                        nhi = sum(DHI1[g] for g in sb)
                        glo_t = gathp.tile([128, NBLO1, K_ELEM1],
                                           dt.bfloat16, tag="glo")
                        estep = 256 if K_ELEM1 != 256 else None
                        emit_gather(glo_t, tab1_lo, i1lo_t, LO1OFF[sbi], nlo,
                                    K_ELEM1, estep)
                        ghi_t = gathp.tile([128, NBHI1, K_ELEM1], dt.bfloat16,
                                           tag="ghi")
                        if nhi > 0:
                            emit_gather(ghi_t, tab1_hi, i1hi_t, HI1OFF[sbi],
                                        nhi, K_ELEM1, estep)
                        lo_off = 0
                        hi_off = 0
                        for gi, g in enumerate(sb):
                            dlo, dhi = DLO1[g], DHI1[g]
                            D = dlo + dhi
                            if L1STEP < 2:
                                lo_off += dlo
                                hi_off += dhi
                                continue
                            logit_t = smallp.tile([128, Dmax1, 4], dt.float32,
                                                  tag="lg")
                            exb_t = smallp.tile([128, Dmax1, 4], dt.bfloat16,
                                                tag="exb")
                            exs_t = smallp.tile([128, Dmax1, 4], dt.bfloat16,
                                                tag="exs")
                            den_t = smallp.tile([128, 4], dt.float32, tag="dn")
                            rec_t = smallp.tile([128, 4], dt.float32, tag="rc")
                            ald_ap = glo_t[:, lo_off, 132:136]
                            nc.vector.scalar_tensor_tensor(
                                logit_t[:, :dlo, :],
                                glo_t[:, lo_off:lo_off + dlo, 128:132], 0.0,
                                ald_ap.unsqueeze(1).broadcast_to(
                                    (128, dlo, 4)), Alu.add, Alu.add)
                            if dhi > 0:
                                nc.vector.scalar_tensor_tensor(
                                    logit_t[:, dlo:D, :],
                                    ghi_t[:, hi_off:hi_off + dhi, 128:132], 0.0,
                                    ald_ap.unsqueeze(1).broadcast_to(
                                        (128, dhi, 4)), Alu.add, Alu.add)
                            nc.vector.scalar_tensor_tensor(
                                logit_t[:, :D, :], logit_t[:, :D, :], NEG,
                                logit_t[:, :D, :], Alu.mult, Alu.max)
                            nc.scalar.activation(exb_t[:, :D, :],
                                                 logit_t[:, :D, :], Act.Exp)
                            nc.vector.tensor_reduce(
                                den_t[:], exb_t[:, :D, :].transpose([0, 2, 1]),
                                axis=Axis.X, op=Alu.add)
                            nc.vector.reciprocal(rec_t[:], den_t[:])
                            nc.vector.tensor_tensor(
                                exs_t[:, :D, :], exb_t[:, :D, :],
                                rec_t[:].unsqueeze(1).broadcast_to(
                                    (128, D, 4)), Alu.mult)
                            if L1STEP < 3:
                                lo_off += dlo
                                hi_off += dhi
                                continue
                            h_lo = glo_t[:, lo_off:lo_off + dlo, 0:128]
                            h_lo = h_lo.rearrange("p b (h c) -> p b h c", h=4)
                            nc.vector.tensor_tensor(
                                h_lo, h_lo,
                                exs_t[:, :dlo, :].unsqueeze(3).broadcast_to(
                                    (128, dlo, 4, HID)), Alu.mult)
                            if dhi > 0:
                                h_hi = ghi_t[:, hi_off:hi_off + dhi, 0:128]
                                h_hi = h_hi.rearrange("p b (h c) -> p b h c",
                                                      h=4)
                                nc.vector.tensor_tensor(
                                    h_hi, h_hi,
                                    exs_t[:, dlo:D, :].unsqueeze(3).broadcast_to(
                                        (128, dhi, 4, HID)), Alu.mult)
                            if L1STEP < 4:
                                lo_off += dlo
                                hi_off += dhi
                                continue
                            segs = [(0, lo_off, dlo)]
                            if dhi > 0:
                                segs.append((1, hi_off, dhi))
                            chunks = _chunks_desc(segs, WPACK1)
                            lanes = chunks[0][2]
                            agg = aggp.tile([128, WPACK1, 128], dt.float32,
                                            tag="agg")
                            for ci, (ti, off, k) in enumerate(chunks):
                                src_t = glo_t if ti == 0 else ghi_t
                                nc.tensor.matmul(
                                    agg[:, 0:k, :],
                                    id_t[:], src_t[:, off:off + k, 0:128],
                                    start=(ci == 0),
                                    stop=(ci == len(chunks) - 1))
                            if L1STEP < 5:
                                lo_off += dlo
                                hi_off += dhi
                                continue
                            scaled_t = epip.tile([128, 128], dt.float32,
                                                 tag="sd")
                            if lanes == 1:
                                nc.vector.tensor_copy(scaled_t[:],
                                                      agg[:, 0, :])
                            else:
                                nc.vector.tensor_reduce(
                                    scaled_t[:],
                                    agg[:, 0:lanes, :].transpose([0, 2, 1]),
                                    axis=Axis.X, op=Alu.add)
                            if HASB1:
                                nc.vector.tensor_tensor(
                                    scaled_t[:], scaled_t[:], b1_t[:], Alu.add)
                            tmp_t = epip.tile([128, 128], dt.float32, tag="tm")
                            nc.scalar.activation(tmp_t[:], scaled_t[:], Act.Relu,
                                                 scale=-1.0)
                            nc.scalar.activation(tmp_t[:], tmp_t[:], Act.Exp,
                                                 scale=-1.0)
                            elu_t = epip.tile([128, 128], dt.bfloat16, tag="el")
                            nc.vector.scalar_tensor_tensor(
                                elu_t[:], tmp_t[:], -1.0, scaled_t[:],
                                Alu.add, Alu.max)
                            # inline pass-2: h1' @ W2ext -> table2 front rows
                            psT = psTp.tile([128, 128], dt.bfloat16, tag="pt")
                            nc.tensor.transpose(psT[:], elu_t[:], id_t[:])
                            eluT_t = epip.tile([128, 128], dt.bfloat16,
                                               tag="et")
                            nc.scalar.activation(eluT_t[:], psT[:], Act.Copy)
                            ps2 = ps2p.tile([128, 36], dt.float32, tag="p2")
                            nc.tensor.matmul(ps2[:], eluT_t[:], w2_t[:],
                                             start=True, stop=True)
                            cp_t = cpp.tile([128, 64], dt.float32, tag="cp")
                            if g % 2 == 0:
                                nc.scalar.activation(cp_t[:, 0:36], ps2[:],
                                                     Act.Copy)
                            else:
                                nc.vector.tensor_copy(cp_t[:, 0:36], ps2[:])
                            nc.sync.dma_start(
                                table2[1 + 128 * g:1 + 128 * (g + 1), :],
                                cp_t[:])
                            lo_off += dlo
                            hi_off += dhi
                            if PHASES >= 3 and L1STEP >= 5:
                                if 128 * (g + 1) == H1:
                                    nc.gpsimd.collective_compute(
                                        "AllGather", Alu.bypass,
                                        replica_groups=rg,
                                        ins=[table2[1:1 + H1, :]],
                                        outs=[t2main1[:, :]])
                    if PHASES >= 3 and L1STEP >= 5:
                        nc.gpsimd.collective_compute(
                            "AllGather", Alu.bypass, replica_groups=rg,
                            ins=[table2[1 + H1:1 + G128, :]],
                            outs=[t2main2[:, :]])

            if PHASES >= 4:
                # ---------------- phase L2: edges + pool ----------------
                B2 = 1 + G128 + NCORES * ((NG1 // 2) * 128)
                tab2_lo = table2[0:B2, :]
                tab2_hi = table2[B2:T2_ROWS, :]
                Dmax2 = max(DLO2[g] + DHI2[g] for g in range(NG2))
                NBLO2 = max(sum(DLO2[g] for g in sb) for sb in SB2)
                NBHI2 = max(max(sum(DHI2[g] for g in sb) for sb in SB2), 1)
                olo2 = np.concatenate([[0], np.cumsum(DLO2)]).astype(int)
                ohi2 = np.concatenate([[0], np.cumsum(DHI2)]).astype(int)
                with (
                    tc.tile_pool(name="gath2", bufs=(5 if SB_BLOCK_BUDGET > 56 else 8)) as gathp,
                    tc.tile_pool(name="small2", bufs=4) as smallp,
                    tc.tile_pool(name="epi2", bufs=3) as epip,
                    tc.tile_pool(name="agg2", bufs=2, space="PSUM") as aggp,
                    tc.tile_pool(name="poolps", bufs=1, space="PSUM") as poolpp,
                    tc.tile_pool(name="mp2", bufs=3) as mpp,
                ):
                    poolps = poolpp.tile([HID, GPOOL], dt.float32)
                    h2p_all = mpp.tile([128, NG2, HID], dt.bfloat16,
                                       tag="h2pall", bufs=1)
                    for sbi, sb in enumerate(SB2):
                        g0 = sb[0]
                        nlo = sum(DLO2[g] for g in sb)
                        nhi = sum(DHI2[g] for g in sb)
                        glo_t = gathp.tile([128, NBLO2, 64], dt.float32,
                                           tag="glo")
                        emit_gather(glo_t, tab2_lo, i2lo_t, LO2OFF[sbi], nlo,
                                    64)
                        ghi_t = gathp.tile([128, NBHI2, 64], dt.float32,
                                           tag="ghi")
                        if nhi > 0:
                            emit_gather(ghi_t, tab2_hi, i2hi_t, HI2OFF[sbi],
                                        nhi, 64)
                        lo_off = 0
                        hi_off = 0
                        for gi, g in enumerate(sb):
                            dlo, dhi = DLO2[g], DHI2[g]
                            D = dlo + dhi
                            logit_t = smallp.tile([128, Dmax2, 1], dt.float32,
                                                  tag="lg")
                            exf_t = smallp.tile([128, Dmax2, 1], dt.float32,
                                                tag="exf")
                            exs_t = smallp.tile([128, Dmax2, 1], dt.float32,
                                                tag="exs")
                            den_t = smallp.tile([128, 1], dt.float32, tag="dn")
                            rec_t = smallp.tile([128, 1], dt.float32, tag="rc")
                            ald_ap = glo_t[:, lo_off, 33:34]
                            nc.vector.scalar_tensor_tensor(
                                logit_t[:, :dlo, :],
                                glo_t[:, lo_off:lo_off + dlo, 32:33], 0.0,
                                ald_ap.unsqueeze(1).broadcast_to(
                                    (128, dlo, 1)), Alu.add, Alu.add)
                            if dhi > 0:
                                nc.vector.scalar_tensor_tensor(
                                    logit_t[:, dlo:D, :],
                                    ghi_t[:, hi_off:hi_off + dhi, 32:33], 0.0,
                                    ald_ap.unsqueeze(1).broadcast_to(
                                        (128, dhi, 1)), Alu.add, Alu.add)
                            nc.vector.scalar_tensor_tensor(
                                logit_t[:, :D, :], logit_t[:, :D, :], NEG,
                                logit_t[:, :D, :], Alu.mult, Alu.max)
                            nc.scalar.activation(exf_t[:, :D, :],
                                                 logit_t[:, :D, :], Act.Exp)
                            nc.vector.tensor_reduce(
                                den_t[:], exf_t[:, :D, :].transpose([0, 2, 1]),
                                axis=Axis.X, op=Alu.add)
                            nc.vector.reciprocal(rec_t[:], den_t[:])
                            nc.vector.tensor_tensor(
                                exs_t[:, :D, :], exf_t[:, :D, :],
                                rec_t[:].unsqueeze(1).broadcast_to(
                                    (128, D, 1)), Alu.mult)
                            exh_t = smallp.tile([128, Dmax2, HID], dt.bfloat16,
                                                tag="exh")
                            nc.vector.tensor_tensor(
                                exh_t[:, :dlo, :],
                                glo_t[:, lo_off:lo_off + dlo, 0:HID],
                                exs_t[:, :dlo, :].broadcast_to(
                                    (128, dlo, HID)), Alu.mult)
                            if dhi > 0:
                                nc.vector.tensor_tensor(
                                    exh_t[:, dlo:D, :],
                                    ghi_t[:, hi_off:hi_off + dhi, 0:HID],
                                    exs_t[:, dlo:D, :].broadcast_to(
                                        (128, dhi, HID)), Alu.mult)
                            chunks = _chunks_desc([(0, 0, D)], WPACK2)
                            lanes = chunks[0][2]
                            agg = aggp.tile([128, WPACK2, HID], dt.float32,
                                            tag="agg")
                            for ci, (ti, off, k) in enumerate(chunks):
                                nc.tensor.matmul(
                                    agg[:, 0:k, :],
                                    id_t[:], exh_t[:, off:off + k, :],
                                    start=(ci == 0),
                                    stop=(ci == len(chunks) - 1))
                            scaled_t = epip.tile([128, HID], dt.float32,
                                                 tag="sd")
                            if lanes == 1:
                                nc.vector.tensor_copy(scaled_t[:],
                                                      agg[:, 0, :])
                            else:
                                nc.vector.tensor_reduce(
                                    scaled_t[:],
                                    agg[:, 0:lanes, :].transpose([0, 2, 1]),
                                    axis=Axis.X, op=Alu.add)
                            if HASB2:
                                nc.vector.tensor_tensor(
                                    scaled_t[:], scaled_t[:], b2_t[:], Alu.add)
                            tmp_t = epip.tile([128, HID], dt.float32, tag="tm")
                            nc.scalar.activation(tmp_t[:], scaled_t[:], Act.Relu,
                                                 scale=-1.0)
                            nc.scalar.activation(tmp_t[:], tmp_t[:], Act.Exp,
                                                 scale=-1.0)
                            nc.vector.scalar_tensor_tensor(
                                h2p_all[:, g, :], tmp_t[:], -1.0, scaled_t[:],
                                Alu.add, Alu.max)
                            lo_off += dlo
                            hi_off += dhi

                    for g in range(NG2):
                        mp_t = mpp.tile([128, GPOOL], dt.bfloat16, tag="mp")
                        nc.sync.dma_start(mp_t[:],
                                          t_mpool[g * 128:(g + 1) * 128, :])
                        nc.tensor.matmul(poolps[:], h2p_all[:, g, :], mp_t[:],
                                         start=(g == 0), stop=(g == NG2 - 1))
                    # ------------- pool + final linear -------------
                    with tc.tile_pool(name="fin", bufs=1) as finp, \
                            tc.tile_pool(name="finps", bufs=1, space="PSUM") as fpp:
                        poolsb = finp.tile([HID, GPOOL], dt.float32)
                        nc.vector.tensor_copy(poolsb[:], poolps[:])
                        nc.sync.dma_start(cc_in[:, :], poolsb[:])
                        nc.gpsimd.collective_compute(
                            "AllGather", Alu.bypass, replica_groups=rg,
                            ins=[cc_in[:, :]], outs=[cc_out[:, :]])
                        parts_t = finp.tile([HID, NCORES, GPOOL], dt.float32)
                        nc.sync.dma_start(
                            parts_t[:],
                            cc_out[:, :].rearrange("(c p) e -> p c e",
                                                   p=HID))
                        psum_t = finp.tile([HID, GPOOL], dt.float32)
                        nc.vector.tensor_reduce(
                            psum_t[:], parts_t[:].transpose([0, 2, 1]),
                            axis=Axis.X, op=Alu.add)
                        mean_t = finp.tile([HID, GPOOL], dt.float32)
                        nc.vector.tensor_tensor(
                            mean_t[:], psum_t[:],
                            rc_t[:], Alu.mult)
                        psO = fpp.tile([GPOOL, OUT], dt.float32)
                        nc.tensor.matmul(psO[:], mean_t[:], wl_t[:], start=True,
                                         stop=True)
                        out_t = finp.tile([GPOOL, OUT], dt.float32)
                        if HASBL:
                            nc.vector.tensor_tensor(out_t[:], psO[:], bl_t[:],
                                                    Alu.add)
                        else:
                            nc.vector.tensor_copy(out_t[:], psO[:])
                        nc.sync.dma_start(t_out[:, :], out_t[:])
            if PHASES < 4:
                _emit_dummy_out(nc, tc, t_out, dt)


    nc.compile()
    return nc


def core_inputs(prep, c):
    cd = prep["cores"][c]

    def padcols(a, cols):
        if a.shape[1] == cols:
            return a
        out = np.zeros((a.shape[0], cols), a.dtype)
        out[:, :a.shape[1]] = a
        return out

    SB1 = _pack_superblocks(prep["DLO1"], prep["DHI1"])
    SB2 = _pack_superblocks(prep["DLO2"], prep["DHI2"])
    _, _, tl1, th1 = _sb_idx_layout(prep["DLO1"], prep["DHI1"], SB1)
    _, _, tl2, th2 = _sb_idx_layout(prep["DLO2"], prep["DHI2"], SB2)
    n1lo, n1hi, n2lo, n2hi = 8 * tl1, 8 * th1, 8 * tl2, 8 * th2
    return dict(
        xT=np.ascontiguousarray(cd["xT"]),
        w1ext=prep["W1ext"], w2ext=prep["W2ext"], wl=prep["Wl"],
        b1=prep["b1"], b2=prep["b2"], bl=prep["bl"], rcnt=prep["rcnt"],
        patch1=prep["patch1"], patch2=prep["patch2"], ident=prep["ident"],
        mpool=np.ascontiguousarray(cd["mpool"]),
        **({"idx32": cd["w_idx32"]} if K_IND else {}),
        idx1lo=padcols(cd["w_idx1lo"], n1lo),
        idx1hi=padcols(cd["w_idx1hi"], n1hi),
        idx2lo=padcols(cd["w_idx2lo"], n2lo),
        idx2hi=padcols(cd["w_idx2hi"], n2hi),
    )


_CACHE = {}


def kernel(**inputs):
    from concourse.bass_utils import run_bass_kernel_spmd

    inputs = {k: np.asarray(v) for k, v in inputs.items()}
    prep = host_prep(**inputs)
    sc = make_sched(prep)
    key = str(sc)
    if key not in _CACHE:
        _CACHE[key] = build_bass(sc)
    nc = _CACHE[key]
    in_maps = [core_inputs(prep, c) for c in range(NCORES)]
    res = run_bass_kernel_spmd(nc, in_maps, list(range(NCORES)))
    return np.asarray(res.results[0]["out"], np.float32)


# revision 28
# speedup vs baseline: 1.1469x; 1.0000x over previous
# Self-contained 8-core Trainium2 Bass kernel for the 2-layer GAT + mean-pool
# problem (nn_GAT_83820581749190).
#
# Sharding: destination nodes (and all their incident edges) are partitioned
# across the 8 cores, so each layer's attention softmax and aggregation
# complete locally per core. Each core builds a replicated layer-1 feature
# table [h1 | al_src | al_dst] (bf16, 512-byte rows) in HBM with a replicated
# x @ W1ext matmul, edge-gathers rows with the GPSIMD dma_gather custom op
# (int16 indices force a lo/hi table-half split), computes the edge softmax
# without segment-max (logits are small enough that exp cannot overflow), and
# aggregates per-destination with wide identity-matmul PSUM accumulation
# (destinations sit on partitions via degree-bucketed groups of 128; up to 4
# degree-lanes per matmul, reduced on Vector). Each destination's self-loop
# edge is forced to be the first lo-half edge, so the gathered block 0 carries
# the per-destination a_dst logits for free. Layer-1 group outputs are pushed
# through W2ext inline and written to a layer-2 feature table in L1-slot
# order; two split AllGathers (overlapped with the L1 tail) replicate it.
# Layer 2 repeats the gather/softmax/aggregate pattern (16 lanes per matmul);
# mean-pool is a matmul against a host-built one-hot graph matrix plus a tiny
# AllReduce.
import os
import numpy as np
import ml_dtypes

N = 50000
E = 800000
IN = 128
HID = 32
HEADS = 4
OUT = 10
GPOOL = 64
NEG = 0.2
NCORES = 8
S = N // NCORES
LO_MAX = 32767          # max usable int16 gather index
SPECIAL_ALS = -100.0    # al_src of pad rows: exp(0.2*(-100+ald)) ~ 2e-9
SB_BLOCK_BUDGET = int(os.environ.get("K_BUD", "48"))  # max gather blocks per superblock
XCHUNK = 512
WPACK1 = 4              # layer-1 psum lanes per wide matmul
WPACK2 = 16             # layer-2 psum lanes per wide matmul
PHASES = int(os.environ.get("K_PHASES", "99"))  # 1=X only, 2=+L1, 3=+exchange, 4=+L2
L1STEP = int(os.environ.get("K_L1STEP", "99"))  # 1=gathers 2=+softmax 3=+exh 4=+agg 5=+epi
K_ELEM1 = int(os.environ.get("K_ELEM1", "256"))  # L1 gather elem (floor experiments)
K_SP = int(os.environ.get("K_SP", "0"))          # single_packet on gathers
K_IND = int(os.environ.get("K_IND", "0"))        # use indirect_dma_start (floor test)

bf16 = ml_dtypes.bfloat16


def _ceil_to(v, m):
    return (v + m - 1) // m * m


# ======================= host prep =========================================

def _boundary_aware_order(deg_lo, deg_hi):
    """Sort ids by (lo desc, hi desc), but fill 128-groups that straddle a
    lo-run boundary from the *small-hi tail* of the next run, keeping
    per-group max_lo + max_hi tight."""
    Sn = len(deg_lo)
    base = np.lexsort((-deg_hi, -deg_lo))
    glo = deg_lo[base]
    runs = []
    i = 0
    while i < Sn:
        j = i
        while j < Sn and glo[j] == glo[i]:
            j += 1
        runs.append(list(base[i:j]))
        i = j
    order = []
    ri = 0
    fronts = [0] * len(runs)
    backs = [len(r) for r in runs]
    while len(order) < Sn:
        while ri < len(runs) and fronts[ri] >= backs[ri]:
            ri += 1
        if ri >= len(runs):
            break
        need = 128 - (len(order) % 128)
        avail = backs[ri] - fronts[ri]
        if avail >= need:
            order.extend(runs[ri][fronts[ri]:fronts[ri] + need])
            fronts[ri] += need
        else:
            order.extend(runs[ri][fronts[ri]:backs[ri]])
            fronts[ri] = backs[ri]
            need -= avail
            rj = ri + 1
            while need > 0 and rj < len(runs):
                a = backs[rj] - fronts[rj]
                t = min(a, need)
                order.extend(reversed(runs[rj][backs[rj] - t:backs[rj]]))
                backs[rj] -= t
                need -= t
                rj += 1
    P = np.asarray(order, np.int64)
    Ppos = np.empty(Sn, np.int64)
    Ppos[P] = np.arange(Sn)
    return P, Ppos


def _run_groups(glo, ghi, max_rows=128):
    Sn = len(glo)
    ng = (Sn + max_rows - 1) // max_rows
    dlo = np.zeros(ng, np.int64)
    dhi = np.zeros(ng, np.int64)
    for g in range(ng):
        s, e = g * max_rows, min((g + 1) * max_rows, Sn)
        dlo[g] = glo[s:e].max()
        dhi[g] = ghi[s:e].max()
    return dlo, dhi


def _build_layer(src, dstl, is_lo):
    deg_lo = np.bincount(dstl[is_lo], minlength=S)
    deg_hi = np.bincount(dstl[~is_lo], minlength=S)
    P, Ppos = _boundary_aware_order(deg_lo, deg_hi)
    dlo, dhi = _run_groups(deg_lo[P], deg_hi[P])
    return dict(src=src, dstl=dstl, is_lo=is_lo, deg_lo=deg_lo, deg_hi=deg_hi,
                P=P, Ppos=Ppos, dlo=dlo, dhi=dhi)


def _emit_slots(l, DLO, DHI, idx_lo_of, idx_hi_of, special_lo, special_hi):
    NG = len(DLO)
    src, is_lo = l["src"], l["is_lo"]
    Ppos = l["Ppos"]
    nreal = len(l["P"])
    slot2cmp = np.full(NG * 128, -1, np.int64)
    slot2cmp[:nreal] = np.arange(nreal)
    idx_lo = [np.full((int(DLO[g]), 128), special_lo, np.int64)
              for g in range(NG)]
    idx_hi = [np.full((int(DHI[g]), 128), special_hi, np.int64)
              for g in range(NG)]
    slot_of_edge = Ppos[l["dstl"]]
    order = np.argsort(slot_of_edge, kind="stable")
    for mask, arrs, idx_fn in ((is_lo, idx_lo, idx_lo_of),
                               (~is_lo, idx_hi, idx_hi_of)):
        m = mask[order]
        so = slot_of_edge[order][m]
        sr = src[order][m]
        jj = np.arange(len(so)) - np.searchsorted(so, so, side="left")
        gg, kk = so // 128, so % 128
        vals = idx_fn(sr)
        for g in range(NG):
            sel = gg == g
            if sel.any():
                arrs[g][jj[sel], kk[sel]] = vals[sel]
    return idx_lo, idx_hi, slot2cmp


def _wrap16(idx):
    """[n] -> [128, n//16] int16: idx i at [i%16, i//16], replicated x8."""
    n = len(idx)
    assert n % 16 == 0
    w = np.ascontiguousarray(np.asarray(idx).reshape(n // 16, 16).T)
    w = w.astype(np.int16)
    return np.tile(w, (8, 1))


def _wrap_groups(arrs):
    segs = [_wrap16(a.reshape(-1)) if a.size else np.zeros((128, 0), np.int16)
            for a in arrs]
    return np.concatenate(segs, axis=1) if segs else np.zeros((128, 0), np.int16)


def host_prep(x, edge_index, batch, W1, a1_src, a1_dst, b1, W2, a2_src, a2_dst,
              b2, Wl, bl):
    x = np.asarray(x, np.float32)
    edge_index = np.asarray(edge_index, np.int64)
    batch = np.asarray(batch, np.int64)
    # self-loops FIRST: each dst's self edge is its first (lo) edge, so the
    # gathered lo block 0 carries the per-dst a_dst logit columns.
    src_all = np.concatenate([np.arange(N, dtype=np.int64), edge_index[0]])
    dst_all = np.concatenate([np.arange(N, dtype=np.int64), edge_index[1]])
    owner = dst_all // S

    a1_src = np.asarray(a1_src, np.float32)
    a1_dst = np.asarray(a1_dst, np.float32)
    W1 = np.asarray(W1, np.float32)
    W2 = np.asarray(W2, np.float32)
    As1 = np.zeros((HEADS * HID, HEADS), np.float32)
    Ad1 = np.zeros((HEADS * HID, HEADS), np.float32)
    for h in range(HEADS):
        As1[h * HID:(h + 1) * HID, h] = a1_src[h]
        Ad1[h * HID:(h + 1) * HID, h] = a1_dst[h]
    W1ext = np.concatenate([W1, W1 @ As1, W1 @ Ad1], axis=1)   # [128,136]
    W2ext = np.concatenate(
        [W2, W2 @ np.asarray(a2_src, np.float32)[0][:, None],
         W2 @ np.asarray(a2_dst, np.float32)[0][:, None],
         np.zeros((HEADS * HID, 2), np.float32)], axis=1)  # [128,36]

    cores = [dict(c=c) for c in range(NCORES)]
    for cd in cores:
        c = cd["c"]
        m = owner == c
        cd["src"] = src_all[m]
        cd["dstl"] = dst_all[m] - c * S

    # ---------- layer 1 ----------
    for cd in cores:
        c = cd["c"]
        pos_of = np.empty(N, np.int64)
        own = np.arange(c * S, (c + 1) * S)
        oth = np.concatenate([np.arange(0, c * S), np.arange((c + 1) * S, N)])
        pos_of[oth] = S + np.arange(N - S)
        pos_of[own] = 0
        is_lo1 = pos_of[cd["src"]] < LO_MAX
        l1 = _build_layer(cd["src"], cd["dstl"], is_lo1)
        pos_of[own] = l1["Ppos"]
        row_of = np.where(pos_of < LO_MAX, pos_of, pos_of + 1)
        cd["l1"] = l1
        cd["row_of"] = row_of
    NG1 = max(len(cd["l1"]["dlo"]) for cd in cores)
    DLO1 = np.zeros(NG1, np.int64)
    DHI1 = np.zeros(NG1, np.int64)
    for cd in cores:
        d = cd["l1"]
        DLO1[:len(d["dlo"])] = np.maximum(DLO1[:len(d["dlo"])], d["dlo"])
        DHI1[:len(d["dhi"])] = np.maximum(DHI1[:len(d["dhi"])], d["dhi"])
    assert (DLO1 >= 1).all()   # self edges are always lo
    for cd in cores:
        r = cd["row_of"]
        cd["idx1_lo"], cd["idx1_hi"], cd["slot2cmp1"] = _emit_slots(
            cd["l1"], DLO1, DHI1,
            lambda s, r=r: r[s], lambda s, r=r: r[s] - (LO_MAX + 1),
            LO_MAX, N + 1 - (LO_MAX + 1))
        # verify self-edge-first: lo block 0 of each group holds the dst's own
        # row for every real slot
        c = cd["c"]
        P1 = cd["l1"]["P"]
        for g in range(NG1):
            e = min(128, S - g * 128)
            if e <= 0:
                break
            dsts = P1[g * 128:g * 128 + e]
            assert (cd["idx1_lo"][g][0, :e] == r[dsts + c * S]).all()

    G128 = NG1 * 128
    # first split-AllGather covers slots < H1 (max groups s.t. lo rows fit int16)
    H1 = min(NG1 - 1, (LO_MAX + 1 - 1 - G128) // (128 * NCORES)) * 128
    # ---------- layer 2 ----------
    # table2 layout: [0]=patch-lo | [1 : 1+G128) own slots (AllGather input) |
    # [1+G128 : +8*H1) all cores' slots < H1 | [.. : +8*(G128-H1)) all cores'
    # slots >= H1 | [last]=patch-hi. Two contiguous AllGather segments.
    T2_ROWS = 1 + G128 + NCORES * G128 + 1
    B2 = 1 + G128 + NCORES * H1      # lo/hi boundary = start of 2nd AllGather
    assert B2 <= LO_MAX + 1
    row2_main = np.empty(N, np.int64)
    for cd in cores:
        c = cd["c"]
        pp1 = cd["l1"]["Ppos"]
        r = np.where(pp1 < H1,
                     1 + G128 + c * H1 + pp1,
                     1 + G128 + NCORES * H1 + c * (G128 - H1) + (pp1 - H1))
        row2_main[c * S:(c + 1) * S] = r
    for cd in cores:
        c = cd["c"]
        src = cd["src"]
        own_m = (src // S) == c
        row2 = row2_main[src].copy()
        row2[own_m] = 1 + cd["l1"]["Ppos"][src[own_m] - c * S]
        cd["row2"] = row2
        is_lo2 = row2 < B2
        cd["l2"] = _build_layer(src, cd["dstl"], is_lo2)
    NG2 = max(len(cd["l2"]["dlo"]) for cd in cores)
    DLO2 = np.zeros(NG2, np.int64)
    DHI2 = np.zeros(NG2, np.int64)
    for cd in cores:
        d = cd["l2"]
        DLO2[:len(d["dlo"])] = np.maximum(DLO2[:len(d["dlo"])], d["dlo"])
        DHI2[:len(d["dhi"])] = np.maximum(DHI2[:len(d["dhi"])], d["dhi"])
    assert (DLO2 >= 1).all()   # self edges (own rows <= G128) are always lo
    assert T2_ROWS - 1 - B2 <= 32767
    for cd in cores:
        l2 = cd["l2"]
        row2 = cd["row2"]
        pos_in_edges = {}
        # idx fns index by src node id; build per-edge instead: _emit_slots
        # passes src ids, but row2 is per-edge. Map via first occurrence is
        # wrong if a src repeats with different rows -- it cannot: row2 is a
        # function of src id only. Build a per-node map lazily.
        row2_of_node = np.empty(N, np.int64)
        row2_of_node[cd["src"]] = row2
        cd["idx2_lo"], cd["idx2_hi"], cd["slot2cmp2"] = _emit_slots(
            l2, DLO2, DHI2,
            lambda s, r=row2_of_node: r[s],
            lambda s, r=row2_of_node: r[s] - B2,
            0, T2_ROWS - 1 - B2)
        # verify self-edge-first in lo block 0
        c = cd["c"]
        P2 = l2["P"]
        own_row = 1 + cd["l1"]["Ppos"]
        for g in range(NG2):
            e = min(128, S - g * 128)
            if e <= 0:
                break
            dsts = P2[g * 128:g * 128 + e]
            assert (cd["idx2_lo"][g][0, :e] == own_row[dsts]).all()

    # ---------- aux ----------
    cnt = np.bincount(batch, minlength=GPOOL).astype(np.float32)
    recip_cnt = (1.0 / np.maximum(cnt, 1.0)).astype(np.float32)

    for cd in cores:
        c = cd["c"]
        gids = batch[c * S:(c + 1) * S]
        Mp = np.zeros((NG2 * 128, GPOOL), np.float32)
        s2c = cd["slot2cmp2"]
        real = s2c >= 0
        Mp[np.where(real)[0], gids[cd["l2"]["P"][s2c[real]]]] = 1.0
        cd["mpool"] = Mp.astype(bf16)

        xt = np.zeros((IN, _ceil_to(N + 2, XCHUNK)), np.float32)
        xt[:, cd["row_of"]] = x.T
        cd["xT"] = xt.astype(bf16)

        segs = []
        for g in range(NG1):
            a = np.concatenate(
                [cd["idx1_lo"][g],
                 cd["idx1_hi"][g] + (LO_MAX + 1)], axis=0)  # [d, 128] abs rows
            segs.append(np.ascontiguousarray(a.T))          # [128, d]
        cd["w_idx32"] = np.concatenate(segs, axis=1).astype(np.int32)
        SB1 = _pack_superblocks(DLO1, DHI1)
        SB2 = _pack_superblocks(DLO2, DHI2)

        def lay(arrs, DL, SBs, offs, total):
            out = np.zeros((128, 8 * total), np.int16)
            for sbi, sb in enumerate(SBs):
                seg = _wrap_groups([arrs[g] for g in sb])
                out[:, 8 * offs[sbi]:8 * offs[sbi] + seg.shape[1]] = seg
            return out

        lo1, hi1, tl1, th1 = _sb_idx_layout(DLO1, DHI1, SB1)
        lo2, hi2, tl2, th2 = _sb_idx_layout(DLO2, DHI2, SB2)
        cd["w_idx1lo"] = lay(cd["idx1_lo"], DLO1, SB1, lo1, tl1)
        cd["w_idx1hi"] = lay(cd["idx1_hi"], DHI1, SB1, hi1, th1)
        cd["w_idx2lo"] = lay(cd["idx2_lo"], DLO2, SB2, lo2, tl2)
        cd["w_idx2hi"] = lay(cd["idx2_hi"], DHI2, SB2, hi2, th2)

    patch1 = np.zeros((2, 256), np.float32)
    patch1[:, 128:132] = SPECIAL_ALS
    patch2 = np.zeros((2, 64), np.float32)
    patch2[:, 32] = SPECIAL_ALS

    return dict(cores=cores,
                DLO1=[int(v) for v in DLO1], DHI1=[int(v) for v in DHI1],
                DLO2=[int(v) for v in DLO2], DHI2=[int(v) for v in DHI2],
                W1ext=W1ext.astype(bf16), W2ext=W2ext.astype(bf16),
                Wl=np.asarray(Wl, np.float32),
                b1=np.tile(np.asarray(b1, np.float32).reshape(1, -1),
                           (128, 1)),
                b2=np.tile(np.asarray(b2, np.float32).reshape(1, -1),
                           (128, 1)),
                bl=np.tile(np.asarray(bl, np.float32).reshape(1, -1),
                           (GPOOL, 1)),
                rcnt=np.tile(recip_cnt.reshape(1, -1), (HID, 1)),
                patch1=patch1.astype(bf16), patch2=patch2,
                ident=np.eye(128, dtype=bf16))




def _sb_idx_layout(DLO, DHI, SBs):
    """Per-superblock 64-col-aligned offsets for lo/hi idx segments.
    Returns (lo_offs, hi_offs, lo_total, hi_total) in 8-col block units."""
    lo_offs, hi_offs = [], []
    lo_cur = hi_cur = 0
    for sb in SBs:
        nlo = sum(int(DLO[g]) for g in sb)
        nhi = sum(int(DHI[g]) for g in sb)
        lo_offs.append(lo_cur)
        hi_offs.append(hi_cur)
        lo_cur += (nlo + 7) // 8 * 8
        hi_cur += (nhi + 7) // 8 * 8
    return lo_offs, hi_offs, max(lo_cur, 1), max(hi_cur, 1)


def _pack_superblocks(DLO, DHI, budget=SB_BLOCK_BUDGET):
    sbs, cur, tot = [], [], 0
    for g in range(len(DLO)):
        d = int(DLO[g] + DHI[g])
        if cur and tot + d > budget:
            sbs.append(cur)
            cur, tot = [], 0
        cur.append(g)
        tot += d
    if cur:
        sbs.append(cur)
    return sbs


def make_sched(prep):
    DLO1, DHI1 = prep["DLO1"], prep["DHI1"]
    DLO2, DHI2 = prep["DLO2"], prep["DHI2"]
    assert all(a >= 1 for a in DLO1)
    assert all(a >= 1 for a in DLO2)
    return dict(DLO1=DLO1, DHI1=DHI1, DLO2=DLO2, DHI2=DHI2,
                SB1=_pack_superblocks(DLO1, DHI1),
                SB2=_pack_superblocks(DLO2, DHI2),
                HASB1=bool(np.any(prep["b1"])), HASB2=bool(np.any(prep["b2"])),
                HASBL=bool(np.any(prep["bl"])))


# ======================= bass kernel =======================================

def _emit_dummy_out(nc, tc, t_out, dt):
    with tc.tile_pool(name='dummy', bufs=1) as dp:
        d = dp.tile([GPOOL, OUT], dt.float32)
        nc.vector.memset(d[:], 0.0)
        nc.sync.dma_start(t_out[:, :], d[:])


def _chunks_desc(segs, wpack):
    """segs: list of (tile_idx, off, count). Returns wide-matmul chunks
    (tile_idx, off, k<=wpack) sorted largest-first."""
    out = []
    for ti, off, cnt in segs:
        rem = cnt
        o = off
        while rem > 0:
            k = min(wpack, rem)
            out.append((ti, o, k))
            o += k
            rem -= k
    out.sort(key=lambda t: -t[2])
    return out


def build_bass(sc):
    import concourse.bacc as bacc
    import concourse.tile as tile
    import concourse.mybir as mybir
    from concourse.library_config import mlp

    dt = mybir.dt
    Alu = mybir.AluOpType
    Act = mybir.ActivationFunctionType
    Axis = mybir.AxisListType

    DLO1, DHI1 = sc["DLO1"], sc["DHI1"]
    DLO2, DHI2 = sc["DLO2"], sc["DHI2"]
    SB1, SB2 = sc["SB1"], sc["SB2"]
    HASB1 = sc.get("HASB1", True)
    HASB2 = sc.get("HASB2", True)
    HASBL = sc.get("HASBL", True)
    NG1, NG2 = len(DLO1), len(DLO2)
    XT_COLS = _ceil_to(N + 2, XCHUNK)
    NCHUNK = XT_COLS // XCHUNK
    G128 = NG1 * 128
    T2_ROWS = 1 + G128 + NCORES * G128 + 1

    nc = bacc.Bacc("TRN2", target_bir_lowering=False, debug=False,
                   num_devices=NCORES, num_swdge_queues=4)

    t_xT = nc.dram_tensor("xT", [IN, XT_COLS], dt.bfloat16, kind="ExternalInput")
    t_w1 = nc.dram_tensor("w1ext", [IN, 136], dt.bfloat16, kind="ExternalInput")
    t_w2 = nc.dram_tensor("w2ext", [IN, 36], dt.bfloat16, kind="ExternalInput")
    t_wl = nc.dram_tensor("wl", [HID, OUT], dt.float32, kind="ExternalInput")
    t_b1 = nc.dram_tensor("b1", [128, HEADS * HID], dt.float32,
                          kind="ExternalInput")
    t_b2 = nc.dram_tensor("b2", [128, HID], dt.float32, kind="ExternalInput")
    t_bl = nc.dram_tensor("bl", [GPOOL, OUT], dt.float32, kind="ExternalInput")
    t_rcnt = nc.dram_tensor("rcnt", [HID, GPOOL], dt.float32,
                            kind="ExternalInput")
    t_patch1 = nc.dram_tensor("patch1", [2, 256], dt.bfloat16,
                              kind="ExternalInput")
    t_patch2 = nc.dram_tensor("patch2", [2, 64], dt.float32,
                              kind="ExternalInput")
    t_ident = nc.dram_tensor("ident", [128, 128], dt.bfloat16,
                             kind="ExternalInput")
    t_mpool = nc.dram_tensor("mpool", [NG2 * 128, GPOOL], dt.bfloat16,
                             kind="ExternalInput")
    LO1OFF, HI1OFF, tl1, th1 = _sb_idx_layout(DLO1, DHI1, SB1)
    LO2OFF, HI2OFF, tl2, th2 = _sb_idx_layout(DLO2, DHI2, SB2)
    n1lo, n1hi, n2lo, n2hi = 8 * tl1, 8 * th1, 8 * tl2, 8 * th2
    n1all = sum(DLO1) + sum(DHI1)
    if K_IND:
        t_i32 = nc.dram_tensor("idx32", [128, n1all], dt.int32,
                               kind="ExternalInput")
    t_i1lo = nc.dram_tensor("idx1lo", [128, n1lo], dt.int16, kind="ExternalInput")
    t_i1hi = nc.dram_tensor("idx1hi", [128, n1hi], dt.int16, kind="ExternalInput")
    t_i2lo = nc.dram_tensor("idx2lo", [128, n2lo], dt.int16, kind="ExternalInput")
    t_i2hi = nc.dram_tensor("idx2hi", [128, n2hi], dt.int16, kind="ExternalInput")
    t_out = nc.dram_tensor("out", [GPOOL, OUT], dt.float32,
                           kind="ExternalOutput")

    rg = [list(range(NCORES))]
    _qc = [0]

    def nextq(ndesc=1):
        _qc[0] = (_qc[0] + 1) % 4
        return _qc[0]

    with tile.TileContext(nc) as tc:
        with (
            tc.tile_pool(name="const", bufs=1) as constp,
            tc.tile_pool(name="dram", bufs=1, space="DRAM") as dramp,
        ):
            nc.gpsimd.load_library(mlp)
            def emit_gather(out_t, tab, idx_t, o0, nb, elem, estep=None,
                            idxp=None, nbmax=None):
                nc.gpsimd.dma_gather(
                    out_t[:, :nb, :], tab,
                    idx_t[:, 8 * o0:8 * (o0 + nb)],
                    128 * nb, 128 * nb, elem, elem_step=estep,
                    single_packet=bool(K_SP), queue_num=nextq(nb))

            table1 = dramp.tile([XT_COLS, 256], dt.bfloat16, tag="table1")
            table2 = dramp.tile([_ceil_to(T2_ROWS, 4), 64], dt.float32,
                                tag="table2")
            cc_in = dramp.tile([HID, GPOOL], dt.float32, tag="ccin")
            cc_out = dramp.tile([NCORES * HID, GPOOL], dt.float32, tag="ccout")

            w1_t = constp.tile([IN, 136], dt.bfloat16)
            nc.sync.dma_start(w1_t[:], t_w1[:])
            w2_t = constp.tile([IN, 36], dt.bfloat16)
            nc.sync.dma_start(w2_t[:], t_w2[:])
            wl_t = constp.tile([HID, OUT], dt.float32)
            nc.sync.dma_start(wl_t[:], t_wl[:])
            b1_t = constp.tile([128, HEADS * HID], dt.float32)
            nc.sync.dma_start(b1_t[:], t_b1[:])
            b2_t = constp.tile([128, HID], dt.float32)
            nc.sync.dma_start(b2_t[:], t_b2[:])
            bl_t = constp.tile([GPOOL, OUT], dt.float32)
            nc.sync.dma_start(bl_t[:], t_bl[:])
            rc_t = constp.tile([HID, GPOOL], dt.float32)
            nc.sync.dma_start(rc_t[:], t_rcnt[:])
            id_t = constp.tile([128, 128], dt.bfloat16)
            nc.sync.dma_start(id_t[:], t_ident[:])
            if PHASES >= 2:
                i1lo_t = constp.tile([128, n1lo], dt.int16)
                nc.sync.dma_start(i1lo_t[:], t_i1lo[:])
                i1hi_t = constp.tile([128, n1hi], dt.int16)
                nc.sync.dma_start(i1hi_t[:], t_i1hi[:])
            if PHASES >= 4:
                i2lo_t = constp.tile([128, n2lo], dt.int16)
                nc.sync.dma_start(i2lo_t[:], t_i2lo[:])
                i2hi_t = constp.tile([128, n2hi], dt.int16)
                nc.sync.dma_start(i2hi_t[:], t_i2hi[:])
            # all gather indices live in SBUF for the whole run

            # ---------------- phase X: build table1 ----------------
            with (
                tc.tile_pool(name="xload", bufs=3) as xlp,
                tc.tile_pool(name="xout", bufs=3) as xop,
                tc.tile_pool(name="xpsum", bufs=4, space="PSUM") as xpp,
            ):
                for t in range(NCHUNK):
                    xt_t = xlp.tile([IN, XCHUNK], dt.bfloat16, tag="xt")
                    nc.sync.dma_start(xt_t[:],
                                      t_xT[:, t * XCHUNK:(t + 1) * XCHUNK])
                    o_t = xop.tile([128, 4, 256], dt.bfloat16, tag="xo")
                    nc.vector.memset(o_t[:, :, 136:256], 0.0)
                    for k in range(4):
                        ps = xpp.tile([128, 136], dt.float32, tag="xp")
                        nc.tensor.matmul(ps[:], xt_t[:, k * 128:(k + 1) * 128],
                                         w1_t[:], start=True, stop=True)
                        if k % 2 == 0:
                            nc.vector.tensor_copy(o_t[:, k, 0:136], ps[:])
                        else:
                            nc.scalar.activation(o_t[:, k, 0:136], ps[:],
                                                 Act.Copy)
                    nc.sync.dma_start(
                        table1[t * XCHUNK:(t + 1) * XCHUNK, :].rearrange(
                            "(k p) e -> p k e", p=128), o_t[:])
            with tc.tile_pool(name="patchp", bufs=1) as pp:
                p1_t = pp.tile([2, 256], dt.bfloat16)
                nc.sync.dma_start(p1_t[:], t_patch1[:])
                nc.sync.dma_start(table1[LO_MAX:LO_MAX + 1, :], p1_t[0:1, :])
                nc.sync.dma_start(table1[N + 1:N + 2, :], p1_t[1:2, :])
                if PHASES >= 2:
                    p2_t = pp.tile([2, 64], dt.float32)
                    nc.sync.dma_start(p2_t[:], t_patch2[:])
                    nc.sync.dma_start(table2[0:1, :], p2_t[0:1, :])
                    nc.sync.dma_start(table2[T2_ROWS - 1:T2_ROWS, :],
                                      p2_t[1:2, :])

            if PHASES >= 2:
                # ---------------- phase L1: edges ----------------
                tab1_lo = table1[0:LO_MAX + 1, :]
                tab1_hi = table1[LO_MAX + 1:N + 2, :]
                Dmax1 = max(DLO1[g] + DHI1[g] for g in range(NG1))
                NBLO1 = max(sum(DLO1[g] for g in sb) for sb in SB1)
                NBHI1 = max(max(sum(DHI1[g] for g in sb) for sb in SB1), 1)
                olo = np.concatenate([[0], np.cumsum(DLO1)]).astype(int)
                ohi = np.concatenate([[0], np.cumsum(DHI1)]).astype(int)
                H1 = min(NG1 - 1,
                         (LO_MAX + 1 - 1 - G128) // (128 * NCORES)) * 128
                t2main1 = table2[1 + G128:1 + G128 + NCORES * H1, :]
                t2main2 = table2[1 + G128 + NCORES * H1:1 + G128 +
                                 NCORES * G128, :]
                with (
                    tc.tile_pool(name="gath1", bufs=(3 if SB_BLOCK_BUDGET > 56 else 5)) as gathp,
                    tc.tile_pool(name="small1", bufs=3) as smallp,
                    tc.tile_pool(name="epi1", bufs=3) as epip,
                    tc.tile_pool(name="cp1", bufs=3) as cpp,
                    tc.tile_pool(name="agg1", bufs=2, space="PSUM") as aggp,
                    tc.tile_pool(name="psT1", bufs=2, space="PSUM") as psTp,
                    tc.tile_pool(name="ps21", bufs=2, space="PSUM") as ps2p,
                ):
                    for sbi, sb in enumerate(SB1):
                        g0 = sb[0]
                        nlo = sum(DLO1[g] for g in sb)
                        nhi = sum(DHI1[g] for g in sb)
                        glo_t = gathp.tile([128, NBLO1, K_ELEM1],
                                           dt.bfloat16, tag="glo")
                        estep = 256 if K_ELEM1 != 256 else None
                        emit_gather(glo_t, tab1_lo, i1lo_t, LO1OFF[sbi], nlo,
                                    K_ELEM1, estep)
                        ghi_t = gathp.tile([128, NBHI1, K_ELEM1], dt.bfloat16,
                                           tag="ghi")
                        if nhi > 0:
                            emit_gather(ghi_t, tab1_hi, i1hi_t, HI1OFF[sbi],
                                        nhi, K_ELEM1, estep)
                        lo_off = 0
                        hi_off = 0
                        for gi, g in enumerate(sb):
                            dlo, dhi = DLO1[g], DHI1[g]
                            D = dlo + dhi
                            if L1STEP < 2:
                                lo_off += dlo
                                hi_off += dhi
                                continue
                            logit_t = smallp.tile([128, Dmax1, 4], dt.float32,
                                                  tag="lg")
                            exb_t = smallp.tile([128, Dmax1, 4], dt.bfloat16,
                                                tag="exb")
                            exs_t = smallp.tile([128, Dmax1, 4], dt.bfloat16,
                                                tag="exs")
                            den_t = smallp.tile([128, 4], dt.float32, tag="dn")
                            rec_t = smallp.tile([128, 4], dt.float32, tag="rc")
                            ald_ap = glo_t[:, lo_off, 132:136]
                            nc.vector.scalar_tensor_tensor(
                                logit_t[:, :dlo, :],
                                glo_t[:, lo_off:lo_off + dlo, 128:132], 0.0,
                                ald_ap.unsqueeze(1).broadcast_to(
                                    (128, dlo, 4)), Alu.add, Alu.add)
                            if dhi > 0:
                                nc.vector.scalar_tensor_tensor(
                                    logit_t[:, dlo:D, :],
                                    ghi_t[:, hi_off:hi_off + dhi, 128:132], 0.0,
                                    ald_ap.unsqueeze(1).broadcast_to(
                                        (128, dhi, 4)), Alu.add, Alu.add)
                            nc.vector.scalar_tensor_tensor(
                                logit_t[:, :D, :], logit_t[:, :D, :], NEG,
                                logit_t[:, :D, :], Alu.mult, Alu.max)
                            nc.scalar.activation(exb_t[:, :D, :],
                                                 logit_t[:, :D, :], Act.Exp)
                            nc.vector.tensor_reduce(
                                den_t[:], exb_t[:, :D, :].transpose([0, 2, 1]),
                                axis=Axis.X, op=Alu.add)
                            nc.vector.reciprocal(rec_t[:], den_t[:])
                            nc.vector.tensor_tensor(
                                exs_t[:, :D, :], exb_t[:, :D, :],
                                rec_t[:].unsqueeze(1).broadcast_to(
                                    (128, D, 4)), Alu.mult)
                            if L1STEP < 3:
                                lo_off += dlo
                                hi_off += dhi
                                continue
                            h_lo = glo_t[:, lo_off:lo_off + dlo, 0:128]
                            h_lo = h_lo.rearrange("p b (h c) -> p b h c", h=4)
                            nc.vector.tensor_tensor(
                                h_lo, h_lo,
                                exs_t[:, :dlo, :].unsqueeze(3).broadcast_to(
                                    (128, dlo, 4, HID)), Alu.mult)
                            if dhi > 0:
                                h_hi = ghi_t[:, hi_off:hi_off + dhi, 0:128]
                                h_hi = h_hi.rearrange("p b (h c) -> p b h c",
                                                      h=4)
                                nc.vector.tensor_tensor(
                                    h_hi, h_hi,
                                    exs_t[:, dlo:D, :].unsqueeze(3).broadcast_to(
                                        (128, dhi, 4, HID)), Alu.mult)
                            if L1STEP < 4:
                                lo_off += dlo
                                hi_off += dhi
                                continue
                            segs = [(0, lo_off, dlo)]
                            if dhi > 0:
                                segs.append((1, hi_off, dhi))
                            chunks = _chunks_desc(segs, WPACK1)
                            lanes = chunks[0][2]
                            agg = aggp.tile([128, WPACK1, 128], dt.float32,
                                            tag="agg")
                            for ci, (ti, off, k) in enumerate(chunks):
                                src_t = glo_t if ti == 0 else ghi_t
                                nc.tensor.matmul(
                                    agg[:, 0:k, :],
                                    id_t[:], src_t[:, off:off + k, 0:128],
                                    start=(ci == 0),
                                    stop=(ci == len(chunks) - 1))
                            if L1STEP < 5:
                                lo_off += dlo
                                hi_off += dhi
                                continue
                            scaled_t = epip.tile([128, 128], dt.float32,
                                                 tag="sd")
                            if lanes == 1:
                                nc.vector.tensor_copy(scaled_t[:],
                                                      agg[:, 0, :])
                            else:
                                nc.vector.tensor_reduce(
                                    scaled_t[:],
                                    agg[:, 0:lanes, :].transpose([0, 2, 1]),
                                    axis=Axis.X, op=Alu.add)
                            if HASB1:
                                nc.vector.tensor_tensor(
                                    scaled_t[:], scaled_t[:], b1_t[:], Alu.add)
                            tmp_t = epip.tile([128, 128], dt.float32, tag="tm")
                            nc.scalar.activation(tmp_t[:], scaled_t[:], Act.Relu,
                                                 scale=-1.0)
                            nc.scalar.activation(tmp_t[:], tmp_t[:], Act.Exp,
                                                 scale=-1.0)
                            elu_t = epip.tile([128, 128], dt.bfloat16, tag="el")
                            nc.vector.scalar_tensor_tensor(
                                elu_t[:], tmp_t[:], -1.0, scaled_t[:],
                                Alu.add, Alu.max)
                            # inline pass-2: h1' @ W2ext -> table2 front rows
                            psT = psTp.tile([128, 128], dt.bfloat16, tag="pt")
                            nc.tensor.transpose(psT[:], elu_t[:], id_t[:])
                            eluT_t = epip.tile([128, 128], dt.bfloat16,
                                               tag="et")
                            nc.scalar.activation(eluT_t[:], psT[:], Act.Copy)
                            ps2 = ps2p.tile([128, 36], dt.float32, tag="p2")
                            nc.tensor.matmul(ps2[:], eluT_t[:], w2_t[:],
                                             start=True, stop=True)
                            cp_t = cpp.tile([128, 64], dt.float32, tag="cp")
                            if g % 2 == 0:
                                nc.scalar.activation(cp_t[:, 0:36], ps2[:],
                                                     Act.Copy)
                            else:
                                nc.vector.tensor_copy(cp_t[:, 0:36], ps2[:])
                            nc.sync.dma_start(
                                table2[1 + 128 * g:1 + 128 * (g + 1), :],
                                cp_t[:])
                            lo_off += dlo
                            hi_off += dhi
                            if PHASES >= 3 and L1STEP >= 5:
                                if 128 * (g + 1) == H1:
                                    nc.gpsimd.collective_compute(
                                        "AllGather", Alu.bypass,
                                        replica_groups=rg,
                                        ins=[table2[1:1 + H1, :]],
                                        outs=[t2main1[:, :]])
                    if PHASES >= 3 and L1STEP >= 5:
                        nc.gpsimd.collective_compute(
                            "AllGather", Alu.bypass, replica_groups=rg,
                            ins=[table2[1 + H1:1 + G128, :]],
                            outs=[t2main2[:, :]])

            if PHASES >= 4:
                # ---------------- phase L2: edges + pool ----------------
                B2 = 1 + G128 + NCORES * min(
                    NG1 - 1, (LO_MAX + 1 - 1 - G128) // (128 * NCORES)) * 128
                tab2_lo = table2[0:B2, :]
                tab2_hi = table2[B2:T2_ROWS, :]
                Dmax2 = max(DLO2[g] + DHI2[g] for g in range(NG2))
                NBLO2 = max(sum(DLO2[g] for g in sb) for sb in SB2)
                NBHI2 = max(max(sum(DHI2[g] for g in sb) for sb in SB2), 1)
                olo2 = np.concatenate([[0], np.cumsum(DLO2)]).astype(int)
                ohi2 = np.concatenate([[0], np.cumsum(DHI2)]).astype(int)
                with (
                    tc.tile_pool(name="gath2", bufs=(5 if SB_BLOCK_BUDGET > 56 else 8)) as gathp,
                    tc.tile_pool(name="small2", bufs=4) as smallp,
                    tc.tile_pool(name="epi2", bufs=3) as epip,
                    tc.tile_pool(name="agg2", bufs=2, space="PSUM") as aggp,
                    tc.tile_pool(name="poolps", bufs=1, space="PSUM") as poolpp,
                    tc.tile_pool(name="mp2", bufs=3) as mpp,
                ):
                    poolps = poolpp.tile([HID, GPOOL], dt.float32)
                    h2p_all = mpp.tile([128, NG2, HID], dt.bfloat16,
                                       tag="h2pall", bufs=1)
                    for sbi, sb in enumerate(SB2):
                        g0 = sb[0]
                        nlo = sum(DLO2[g] for g in sb)
                        nhi = sum(DHI2[g] for g in sb)
                        glo_t = gathp.tile([128, NBLO2, 64], dt.float32,
                                           tag="glo")
                        emit_gather(glo_t, tab2_lo, i2lo_t, LO2OFF[sbi], nlo,
                                    64)
                        ghi_t = gathp.tile([128, NBHI2, 64], dt.float32,
                                           tag="ghi")
                        if nhi > 0:
                            emit_gather(ghi_t, tab2_hi, i2hi_t, HI2OFF[sbi],
                                        nhi, 64)
                        lo_off = 0
                        hi_off = 0
                        for gi, g in enumerate(sb):
                            dlo, dhi = DLO2[g], DHI2[g]
                            D = dlo + dhi
                            logit_t = smallp.tile([128, Dmax2, 1], dt.float32,
                                                  tag="lg")
                            exf_t = smallp.tile([128, Dmax2, 1], dt.float32,
                                                tag="exf")
                            exs_t = smallp.tile([128, Dmax2, 1], dt.float32,
                                                tag="exs")
                            den_t = smallp.tile([128, 1], dt.float32, tag="dn")
                            rec_t = smallp.tile([128, 1], dt.float32, tag="rc")
                            ald_ap = glo_t[:, lo_off, 33:34]
                            nc.vector.scalar_tensor_tensor(
                                logit_t[:, :dlo, :],
                                glo_t[:, lo_off:lo_off + dlo, 32:33], 0.0,
                                ald_ap.unsqueeze(1).broadcast_to(
                                    (128, dlo, 1)), Alu.add, Alu.add)
                            if dhi > 0:
                                nc.vector.scalar_tensor_tensor(
                                    logit_t[:, dlo:D, :],
                                    ghi_t[:, hi_off:hi_off + dhi, 32:33], 0.0,
                                    ald_ap.unsqueeze(1).broadcast_to(
                                        (128, dhi, 1)), Alu.add, Alu.add)
                            nc.vector.scalar_tensor_tensor(
                                logit_t[:, :D, :], logit_t[:, :D, :], NEG,
                                logit_t[:, :D, :], Alu.mult, Alu.max)
                            nc.scalar.activation(exf_t[:, :D, :],
                                                 logit_t[:, :D, :], Act.Exp)
                            nc.vector.tensor_reduce(
                                den_t[:], exf_t[:, :D, :].transpose([0, 2, 1]),
                                axis=Axis.X, op=Alu.add)
                            nc.vector.reciprocal(rec_t[:], den_t[:])
                            nc.vector.tensor_tensor(
                                exs_t[:, :D, :], exf_t[:, :D, :],
                                rec_t[:].unsqueeze(1).broadcast_to(
                                    (128, D, 1)), Alu.mult)
                            exh_t = smallp.tile([128, Dmax2, HID], dt.bfloat16,
                                                tag="exh")
                            nc.vector.tensor_tensor(
                                exh_t[:, :dlo, :],
                                glo_t[:, lo_off:lo_off + dlo, 0:HID],
                                exs_t[:, :dlo, :].broadcast_to(
                                    (128, dlo, HID)), Alu.mult)
                            if dhi > 0:
                                nc.vector.tensor_tensor(
                                    exh_t[:, dlo:D, :],
                                    ghi_t[:, hi_off:hi_off + dhi, 0:HID],
                                    exs_t[:, dlo:D, :].broadcast_to(
                                        (128, dhi, HID)), Alu.mult)
                            chunks = _chunks_desc([(0, 0, D)], WPACK2)
                            lanes = chunks[0][2]
                            agg = aggp.tile([128, WPACK2, HID], dt.float32,
                                            tag="agg")
                            for ci, (ti, off, k) in enumerate(chunks):
                                nc.tensor.matmul(
                                    agg[:, 0:k, :],
                                    id_t[:], exh_t[:, off:off + k, :],
                                    start=(ci == 0),
                                    stop=(ci == len(chunks) - 1))
                            scaled_t = epip.tile([128, HID], dt.float32,
                                                 tag="sd")
                            if lanes == 1:
                                nc.vector.tensor_copy(scaled_t[:],
                                                      agg[:, 0, :])
                            else:
                                nc.vector.tensor_reduce(
                                    scaled_t[:],
                                    agg[:, 0:lanes, :].transpose([0, 2, 1]),
                                    axis=Axis.X, op=Alu.add)
                            if HASB2:
                                nc.vector.tensor_tensor(
                                    scaled_t[:], scaled_t[:], b2_t[:], Alu.add)
                            tmp_t = epip.tile([128, HID], dt.float32, tag="tm")
                            nc.scalar.activation(tmp_t[:], scaled_t[:], Act.Relu,
                                                 scale=-1.0)
                            nc.scalar.activation(tmp_t[:], tmp_t[:], Act.Exp,
                                                 scale=-1.0)
                            nc.vector.scalar_tensor_tensor(
                                h2p_all[:, g, :], tmp_t[:], -1.0, scaled_t[:],
                                Alu.add, Alu.max)
                            lo_off += dlo
                            hi_off += dhi

                    for g in range(NG2):
                        mp_t = mpp.tile([128, GPOOL], dt.bfloat16, tag="mp")
                        nc.sync.dma_start(mp_t[:],
                                          t_mpool[g * 128:(g + 1) * 128, :])
                        nc.tensor.matmul(poolps[:], h2p_all[:, g, :], mp_t[:],
                                         start=(g == 0), stop=(g == NG2 - 1))
                    # ------------- pool + final linear -------------
                    with tc.tile_pool(name="fin", bufs=1) as finp, \
                            tc.tile_pool(name="finps", bufs=1, space="PSUM") as fpp:
                        poolsb = finp.tile([HID, GPOOL], dt.float32)
                        nc.vector.tensor_copy(poolsb[:], poolps[:])
                        nc.sync.dma_start(cc_in[:, :], poolsb[:])
                        nc.gpsimd.collective_compute(
                            "AllGather", Alu.bypass, replica_groups=rg,
                            ins=[cc_in[:, :]], outs=[cc_out[:, :]])
                        parts_t = finp.tile([HID, NCORES, GPOOL], dt.float32)
                        nc.sync.dma_start(
                            parts_t[:],
                            cc_out[:, :].rearrange("(c p) e -> p c e",
                                                   p=HID))
                        psum_t = finp.tile([HID, GPOOL], dt.float32)
                        nc.vector.tensor_reduce(
                            psum_t[:], parts_t[:].transpose([0, 2, 1]),
                            axis=Axis.X, op=Alu.add)
                        mean_t = finp.tile([HID, GPOOL], dt.float32)
                        nc.vector.tensor_tensor(
                            mean_t[:], psum_t[:],
                            rc_t[:], Alu.mult)
                        psO = fpp.tile([GPOOL, OUT], dt.float32)
                        nc.tensor.matmul(psO[:], mean_t[:], wl_t[:], start=True,
                                         stop=True)
                        out_t = finp.tile([GPOOL, OUT], dt.float32)
                        if HASBL:
                            nc.vector.tensor_tensor(out_t[:], psO[:], bl_t[:],
                                                    Alu.add)
                        else:
                            nc.vector.tensor_copy(out_t[:], psO[:])
                        nc.sync.dma_start(t_out[:, :], out_t[:])
            if PHASES < 4:
                _emit_dummy_out(nc, tc, t_out, dt)


    nc.compile()
    return nc


def core_inputs(prep, c):
    cd = prep["cores"][c]

    def padcols(a, cols):
        if a.shape[1] == cols:
            return a
        out = np.zeros((a.shape[0], cols), a.dtype)
        out[:, :a.shape[1]] = a
        return out

    SB1 = _pack_superblocks(prep["DLO1"], prep["DHI1"])
    SB2 = _pack_superblocks(prep["DLO2"], prep["DHI2"])
    _, _, tl1, th1 = _sb_idx_layout(prep["DLO1"], prep["DHI1"], SB1)
    _, _, tl2, th2 = _sb_idx_layout(prep["DLO2"], prep["DHI2"], SB2)
    n1lo, n1hi, n2lo, n2hi = 8 * tl1, 8 * th1, 8 * tl2, 8 * th2
    return dict(
        xT=np.ascontiguousarray(cd["xT"]),
        w1ext=prep["W1ext"], w2ext=prep["W2ext"], wl=prep["Wl"],
        b1=prep["b1"], b2=prep["b2"], bl=prep["bl"], rcnt=prep["rcnt"],
        patch1=prep["patch1"], patch2=prep["patch2"], ident=prep["ident"],
        mpool=np.ascontiguousarray(cd["mpool"]),
        **({"idx32": cd["w_idx32"]} if K_IND else {}),
        idx1lo=padcols(cd["w_idx1lo"], n1lo),
        idx1hi=padcols(cd["w_idx1hi"], n1hi),
        idx2lo=padcols(cd["w_idx2lo"], n2lo),
        idx2hi=padcols(cd["w_idx2hi"], n2hi),
    )


_CACHE = {}


def kernel(**inputs):
    from concourse.bass_utils import run_bass_kernel_spmd

    inputs = {k: np.asarray(v) for k, v in inputs.items()}
    prep = host_prep(**inputs)
    sc = make_sched(prep)
    key = str(sc)
    if key not in _CACHE:
        _CACHE[key] = build_bass(sc)
    nc = _CACHE[key]
    in_maps = [core_inputs(prep, c) for c in range(NCORES)]
    res = run_bass_kernel_spmd(nc, in_maps, list(range(NCORES)))
    return np.asarray(res.results[0]["out"], np.float32)
